# revision 1
# baseline (speedup 1.0000x reference)
"""Causal self-attention (B=4, T=2048, C=1024, H=16, Dh=64) on 8 trn2 NeuronCores.

Sharding: core i <-> (batch b = i//2, head-group g = i%2). Each core computes
8 heads of one batch end-to-end (qkv slice, causal attention, partial output
projection); the host sums the head-group/pair-couple partials per batch and
adds bproj. No device collectives.

All matmuls run as float32r (single-pass reduced-precision fp32 on the PE,
full-rate at moving-dim >= 256), accumulating in fp32 PSUM. Attention uses
the transposed-scores layout sT[tk, tq] so no per-block transposes are
needed: softmax denominators come out of the PV matmul via an extra ones
column interleaved into Wv, and are broadcast across partitions with a
partition-step-0 SBUF->SBUF DMA.
"""

import numpy as np

import concourse.bass as bass
import concourse.tile as tile
from concourse import bacc, mybir
from concourse.bass_utils import run_bass_kernel_spmd
from concourse.masks import make_identity

F32 = mybir.dt.float32
F32R = mybir.dt.float32r
F16 = mybir.dt.float16

N_CORES = 8
B, T, C = 4, 2048, 1024
NH_TOT, D = 16, 64
F = 512            # features per core (8 heads)
NH = 8             # local heads
NPAIR = 4          # head pairs (128 feats each)
CCH = C // 128     # 8 contraction chunks
NTT = T // 128     # 16 t tiles
NTB = T // 512     # 4 t blocks (qkv production)
NQB = T // 512     # 4 q blocks (attention)
VW = NH * (D + 1)  # 520: augmented v width
ADD = mybir.AluOpType.add
MULT = mybir.AluOpType.mult


def _emit(tc, aps):
    from contextlib import ExitStack
    nc = tc.nc
    x, wq, wk, wva, bq, bk, wp = (
        aps["x"], aps["wq"], aps["wk"], aps["wva"], aps["bq"], aps["bk"],
        aps["wp"])
    cmask = aps["cmask"]
    out_ab = [aps["out_pa"], aps["out_pb"]]

    # ---- pools (all coexist; ~210KB/partition total) ----
    ctx = ExitStack()
    pp_qk = ctx.enter_context(tc.tile_pool(name="ps_qk", bufs=2, space="PSUM"))
    pp_s = ctx.enter_context(tc.tile_pool(name="ps_s", bufs=2, space="PSUM"))
    pp_pv = ctx.enter_context(tc.tile_pool(name="ps_pv", bufs=2, space="PSUM"))
    po_v = ctx.enter_context(tc.tile_pool(name="v_all", bufs=1))
    po_mask = ctx.enter_context(tc.tile_pool(name="mask", bufs=1))
    po_wv = ctx.enter_context(tc.tile_pool(name="wv", bufs=8))
    po_qkt = ctx.enter_context(tc.tile_pool(name="qkT", bufs=2))
    po_bias = ctx.enter_context(tc.tile_pool(name="bias", bufs=1))
    po_misc = ctx.enter_context(tc.tile_pool(name="misc", bufs=3))
    po_xt = ctx.enter_context(tc.tile_pool(name="xT", bufs=1))
    po_wqk = ctx.enter_context(tc.tile_pool(name="wqk", bufs=8))
    po_yt = ctx.enter_context(tc.tile_pool(name="yT", bufs=3))
    po_exp = ctx.enter_context(tc.tile_pool(name="expT", bufs=4))
    po_rec = ctx.enter_context(tc.tile_pool(name="recip", bufs=3))
    po_den = ctx.enter_context(tc.tile_pool(name="den", bufs=2))
    po_ytmp = ctx.enter_context(tc.tile_pool(name="ytmp", bufs=2))
    po_wp = ctx.enter_context(tc.tile_pool(name="wp", bufs=4))
    po_dram = ctx.enter_context(tc.tile_pool(name="dram_scr", bufs=4,
                                             space="DRAM"))

    mask_sb = po_mask.tile([128, 512], F32, tag="mask")
    nc.sync.dma_start(out=mask_sb[:], in_=cmask[:])
    ident = po_bias.tile([128, 128], F16, tag="ident")
    nc.sync.dma_start(out=ident[:], in_=aps["identin"][:])
    # bva broadcast to all 128 partitions straight from DRAM
    bva_bc = po_bias.tile([128, VW], F32, tag="bva_bc")
    bva2 = aps["bva2"]
    nc.sync.dma_start(out=bva_bc[:], in_=bass.AP(
        tensor=bva2.tensor, offset=bva2.offset,
        ap=[[0, 128]] + [list(a) for a in bva2.ap[1:]]))

    # ---- phase 0: x -> xT via PE transposes ----
    xT = [po_xt.tile([128, T], F16, tag=f"xT{c}", name=f"xT{c}")
          for c in range(CCH)]
    for tt in range(NTT):
        xt_in = po_misc.tile([128, C], F16, tag="miscH", name="xt_in")
        nc.sync.dma_start(out=xt_in[:], in_=x[tt * 128:(tt + 1) * 128, :])
        tsl = slice(tt * 128, (tt + 1) * 128)
        for ca in range(2):
            pst = pp_qk.tile([128, 512], F16, tag="qk", name="pst")
            for j in range(4):
                c = ca * 4 + j
                nc.tensor.transpose(
                    pst[:, j * 128:(j + 1) * 128],
                    xt_in[:, c * 128:(c + 1) * 128],
                    ident[:])
            for j in range(4):
                nc.vector.tensor_copy(
                    xT[ca * 4 + j][:, tsl], pst[:, j * 128:(j + 1) * 128])

    # ---- phase 0b: v (augmented with ones columns, all 8 heads) ----
    v_all = [po_v.tile([128, VW], F16, tag=f"v{tt}", name=f"v{tt}")
             for tt in range(NTT)]
    for half in range(2):
        cs = slice(half * 260, half * 260 + 260)
        wv_sb = []
        for c in range(CCH):
            wt = po_wv.tile([128, 260], F16, tag="wv")
            nc.sync.dma_start(out=wt[:], in_=wva[c * 128:(c + 1) * 128, cs])
            wv_sb.append(wt)
        for tt in range(NTT):
            ps = pp_qk.tile([128, 260], F32, tag="qk")
            for c in range(CCH):
                nc.tensor.matmul(
                    ps[:], xT[c][:, tt * 128:(tt + 1) * 128],
                    wv_sb[c][:], start=(c == 0), stop=(c == CCH - 1))
            nc.vector.tensor_add(v_all[tt][:, cs], ps[:], bva_bc[:, cs])

    # ---- per head pair: qkv -> attention -> partial proj ----
    # Emitted as interleaved work units so the PE instruction stream mixes
    # next-pair qkv (and couple proj) matmuls between attention groups --
    # engines are in-order, so a blocked exp-wait would otherwise stall
    # ready qkv work behind it.

    def prep_qkv(pair):
        psl = slice(pair * 128, (pair + 1) * 128)
        wqk_c = []
        for c in range(CCH):
            wt = po_wqk.tile([128, 256], F16, tag="wqk", name="wt")
            nc.sync.dma_start(out=wt[:, 0:128],
                              in_=wq[c * 128:(c + 1) * 128, psl])
            nc.sync.dma_start(out=wt[:, 128:256],
                              in_=wk[c * 128:(c + 1) * 128, psl])
            wqk_c.append(wt)
        bq_sb = po_bias.tile([128, 1], F32, tag=f"bq{pair}", name=f"bq{pair}")
        nc.sync.dma_start(out=bq_sb[:], in_=bq[psl, :])
        bk_sb = po_bias.tile([128, 1], F32, tag=f"bk{pair}", name=f"bk{pair}")
        nc.sync.dma_start(out=bk_sb[:], in_=bk[psl, :])
        qT = po_qkt.tile([128, T], F16, tag="qT", name="qT")
        kT = po_qkt.tile([128, T], F16, tag="kT", name="kT")
        return dict(wqk=wqk_c, bq=bq_sb, bk=bk_sb, qT=qT, kT=kT)

    def qkv_units(st8):
        units = []
        for tb in range(NTB):
            def unit(tb=tb):
                tsl = slice(tb * 512, (tb + 1) * 512)
                psq = pp_qk.tile([128, 512], F32, tag="qk", name="psq")
                for c in range(CCH):
                    nc.tensor.matmul(psq[:], st8["wqk"][c][:, 0:128],
                                     xT[c][:, tsl],
                                     start=(c == 0), stop=(c == CCH - 1))
                # psum*1/sqrt(D) + bq/sqrt(D)   (bq pre-scaled on host)
                nc.vector.tensor_scalar(
                    out=st8["qT"][:, tsl], in0=psq[:], scalar1=0.125,
                    scalar2=st8["bq"][:], op0=MULT, op1=ADD)
                psk = pp_qk.tile([128, 512], F32, tag="qk", name="psk")
                for c in range(CCH):
                    nc.tensor.matmul(psk[:], st8["wqk"][c][:, 128:256],
                                     xT[c][:, tsl],
                                     start=(c == 0), stop=(c == CCH - 1))
                nc.vector.tensor_scalar(
                    out=st8["kT"][:, tsl], in0=psk[:], scalar1=st8["bk"][:],
                    scalar2=None, op0=ADD)
            units.append(unit)
        return units

    def attn_units(st8, yt):
        qT, kT = st8["qT"], st8["kT"]
        units = []
        for hl in range(2):
            for qb in range(NQB):
                def unit(hl=hl, qb=qb):
                    h = None
                    rq = slice(hl * 64, hl * 64 + 64)
                    pair_h = st8["pair"] * 2 + hl
                    vsl = slice(pair_h * 65, pair_h * 65 + 65)
                    qsl = slice(qb * 512, (qb + 1) * 512)
                    nkt = 4 * qb + 4
                    pv = pp_pv.tile([128, 512], F32, tag="pv", name="pv")

                    def emit_scores(g0, gs, st, offs):
                        for i in range(gs):
                            kt = g0 + i
                            j = kt - 4 * qb
                            off = 128 * j if j > 0 else 0
                            offs.append(off)
                            nc.tensor.matmul(
                                st[:, i * 512 + off:(i + 1) * 512],
                                kT[rq, kt * 128:(kt + 1) * 128],
                                qT[rq, qb * 512 + off:(qb + 1) * 512],
                                start=True, stop=True)
                            if j >= 0:
                                nc.vector.tensor_add(
                                    st[:, i * 512 + off:(i + 1) * 512],
                                    st[:, i * 512 + off:(i + 1) * 512],
                                    mask_sb[:, 0:512 - off])

                    def emit_exp_pv(g0, gs, st, offs):
                        et = po_exp.tile([128, 1024], F16, tag="expT",
                                         name="et")
                        if gs == 2 and offs == [0, 0]:
                            nc.scalar.activation(
                                et[:, 0:1024], st[:, 0:1024],
                                mybir.ActivationFunctionType.Exp)
                        else:
                            for i in range(gs):
                                off = offs[i]
                                nc.scalar.activation(
                                    et[:, i * 512 + off:(i + 1) * 512],
                                    st[:, i * 512 + off:(i + 1) * 512],
                                    mybir.ActivationFunctionType.Exp)
                        for i in range(gs):
                            kt = g0 + i
                            off = offs[i]
                            nc.tensor.matmul(
                                pv[0:65, off:512], v_all[kt][:, vsl],
                                et[:, i * 512 + off:(i + 1) * 512],
                                start=(kt == 0), stop=(kt == nkt - 1))

                    pend = None
                    for g0 in range(0, nkt, 2):
                        gs = min(2, nkt - g0)
                        st = pp_s.tile([128, 1024], F32, tag="s", name="st")
                        offs = []
                        emit_scores(g0, gs, st, offs)
                        if pend is not None:
                            emit_exp_pv(*pend)
                        pend = (g0, gs, st, offs)
                    emit_exp_pv(*pend)
                    # den row 64 -> DRAM bounce broadcast -> recip -> mul
                    den = po_den.tile([128, 512], F32, tag="den", name="den")
                    nc.vector.tensor_copy(den[64:65, :], pv[64:65, :])
                    dscr = po_dram.tile([1, 512], F32, tag="dscr", name="dscr")
                    nc.sync.dma_start(out=dscr[:], in_=den[64:65, :])
                    rec = po_rec.tile([128, 512], F32, tag="recip", name="rec")
                    nc.sync.dma_start(out=rec[0:64, :], in_=bass.AP(
                        tensor=dscr.tensor, offset=dscr[:].offset,
                        ap=[[0, 64]] + [list(a) for a in dscr[:].ap[1:]]))
                    nc.vector.reciprocal_approx_fast(rec[0:64, :],
                                                     rec[0:64, :])
                    if hl == 0:
                        nc.vector.tensor_mul(yt[0:64, qsl], pv[0:64, :],
                                             rec[0:64, :])
                    else:
                        # engines can't cross partitions; bounce via DMA
                        ytmp = po_ytmp.tile([128, 512], F16, tag="ytmp",
                                            name="ytmp")
                        nc.vector.tensor_mul(ytmp[0:64, :], pv[0:64, :],
                                             rec[0:64, :])
                        nc.sync.dma_start(out=yt[64:128, qsl],
                                          in_=ytmp[0:64, :])
                units.append(unit)
        return units

    def prep_proj(couple):
        wp_sb = []
        for pq in range(2):
            for cb in range(2):
                prow = (couple * 2 + pq) * 128
                wt = po_wp.tile([128, 512], F16, tag="wp", name="wpt")
                nc.sync.dma_start(
                    out=wt[:],
                    in_=wp[prow:prow + 128, cb * 512:(cb + 1) * 512])
                wp_sb.append(wt)
        return wp_sb

    def proj_units(couple, wp_sb, yts):
        out_p = out_ab[couple]
        units = []
        for tt0 in range(0, NTT, 2):
            def unit(tt0=tt0):
                for tt in (tt0, tt0 + 1):
                    ot = po_misc.tile([128, C], F32, tag="misc", name="ot")
                    for cb in range(2):
                        ps = pp_qk.tile([128, 512], F32, tag="qk", name="pp")
                        for pq in range(2):
                            nc.tensor.matmul(
                                ps[:],
                                yts[pq][:, tt * 128:(tt + 1) * 128],
                                wp_sb[pq * 2 + cb][:],
                                start=(pq == 0), stop=(pq == 1))
                        nc.vector.tensor_copy(
                            ot[:, cb * 512:(cb + 1) * 512], ps[:])
                    nc.sync.dma_start(
                        out=out_p[tt * 128:(tt + 1) * 128, :], in_=ot[:])
            units.append(unit)
        return units

    def round_robin(*streams):
        streams = [list(s) for s in streams if s]
        while any(streams):
            for s in streams:
                if s:
                    s.pop(0)()

    pair_state = []
    yts = []
    st0 = prep_qkv(0)
    st0["pair"] = 0
    pair_state.append(st0)
    for u in qkv_units(st0):
        u()
    proj_work = {}
    for p in range(NPAIR):
        yt = po_yt.tile([128, T], F16, tag="yT", name="yt")
        yts.append(yt)
        streams = [attn_units(pair_state[p], yt)]
        if p + 1 < NPAIR:
            stn = prep_qkv(p + 1)
            stn["pair"] = p + 1
            pair_state.append(stn)
            streams.append(qkv_units(stn))
        if p == 2:
            wp_sb = prep_proj(0)
            streams.append(proj_units(0, wp_sb, yts[0:2]))
        round_robin(*streams)
    wp_sb = prep_proj(1)
    for u in proj_units(1, wp_sb, yts[2:4]):
        u()

    ctx.close()


_CACHE = {}


def _build():
    if "nc" in _CACHE:
        return _CACHE["nc"]
    nc = bacc.Bacc("TRN2", target_bir_lowering=False, debug=False,
                   enable_asserts=True, num_devices=N_CORES)
    aps = {
        "x": nc.dram_tensor("x", [T, C], F16, kind="ExternalInput").ap(),
        "wq": nc.dram_tensor("wq", [C, F], F16, kind="ExternalInput").ap(),
        "wk": nc.dram_tensor("wk", [C, F], F16, kind="ExternalInput").ap(),
        "wva": nc.dram_tensor("wva", [C, VW], F16, kind="ExternalInput").ap(),
        "bq": nc.dram_tensor("bq", [F, 1], F32, kind="ExternalInput").ap(),
        "bk": nc.dram_tensor("bk", [F, 1], F32, kind="ExternalInput").ap(),
        "bva2": nc.dram_tensor("bva2", [1, VW], F32, kind="ExternalInput").ap(),
        "wp": nc.dram_tensor("wp", [F, C], F16, kind="ExternalInput").ap(),
        "cmask": nc.dram_tensor("cmask", [128, 512], F32,
                                kind="ExternalInput").ap(),
        "identin": nc.dram_tensor("identin", [128, 128], F16,
                                  kind="ExternalInput").ap(),
        "out_pa": nc.dram_tensor("out_pa", [T, C], F32,
                                 kind="ExternalOutput").ap(),
        "out_pb": nc.dram_tensor("out_pb", [T, C], F32,
                                 kind="ExternalOutput").ap(),
    }
    with tile.TileContext(nc) as tc:
        _emit(tc, aps)
    nc.compile()
    _CACHE["nc"] = nc
    return nc


def _make_in_maps(x, Wqkv, bqkv, Wproj):
    x = np.asarray(x, dtype=np.float32)
    Wqkv = np.asarray(Wqkv, dtype=np.float32)
    bqkv = np.asarray(bqkv, dtype=np.float32)
    Wproj = np.asarray(Wproj, dtype=np.float32)

    # triangular causal mask: M[p, f] = 0 if f >= p else -1e9
    p_idx = np.arange(128)[:, None]
    u_idx = np.arange(512)[None, :]
    cmask = np.where(u_idx >= p_idx, 0.0, -1e9).astype(np.float32)

    in_maps = []
    for core in range(N_CORES):
        b, g = divmod(core, 2)
        q0, k0, v0 = 512 * g, C + 512 * g, 2 * C + 512 * g
        wva = np.zeros((C, VW), dtype=np.float32)
        bva = np.zeros((1, VW), dtype=np.float32)
        for h in range(NH):
            src = v0 + D * h
            dst = 65 * h
            # per-head layout [v(64), one]
            wva[:, dst:dst + 64] = Wqkv[:, src:src + 64]
            bva[0, dst:dst + 64] = bqkv[src:src + 64]
            bva[0, dst + 64] = 1.0
        in_maps.append({
            "x": np.ascontiguousarray(x[b]).astype(np.float16),
            "wq": np.ascontiguousarray(Wqkv[:, q0:q0 + F]).astype(np.float16),
            "wk": np.ascontiguousarray(Wqkv[:, k0:k0 + F]).astype(np.float16),
            "wva": wva.astype(np.float16),
            "bq": np.ascontiguousarray(bqkv[q0:q0 + F].reshape(F, 1) * 0.125),
            "bk": np.ascontiguousarray(bqkv[k0:k0 + F].reshape(F, 1)),
            "bva2": bva,
            "wp": np.ascontiguousarray(Wproj[512 * g:512 * g + F, :]).astype(np.float16),
            "cmask": cmask,
            "identin": np.eye(128, dtype=np.float16),
        })
    return in_maps


def run_sharded(x, Wqkv, bqkv, Wproj, bproj, trace=False):
    nc = _build()
    in_maps = _make_in_maps(x, Wqkv, bqkv, Wproj)
    res = run_bass_kernel_spmd(nc, in_maps, core_ids=list(range(N_CORES)),
                               trace=trace)
    bproj = np.asarray(bproj, dtype=np.float32)
    out = np.empty((B, T, C), dtype=np.float32)
    for b in range(B):
        acc = bproj[None, :].astype(np.float32).repeat(T, axis=0)
        for core in (2 * b, 2 * b + 1):
            acc = acc + res.results[core]["out_pa"] + res.results[core]["out_pb"]
        out[b] = acc
    return out, res


def kernel(x, Wqkv, bqkv, Wproj, bproj):
    out, _ = run_sharded(x, Wqkv, bqkv, Wproj, bproj, trace=False)
    return out



# revision 10
# speedup vs baseline: 1.1012x; 1.1012x over previous
"""Causal self-attention (B=4, T=2048, C=1024, H=16, Dh=64) on 8 trn2 NeuronCores.

Sharding: core i <-> (batch b = i//2, head-group g = i%2). Each core computes
8 heads of one batch end-to-end (qkv slice, causal attention, partial output
projection); the host sums the head-group partials per batch and adds bproj.
No device collectives.

v2 layout: x arrives pre-transposed from the host ([C, T]) so no PE
transposes are needed. Scores use the transposed layout sT[tk, tq]; the two
heads of a pair occupy PE row-groups 0-63 / 64-127 and their score matmuls
are emitted adjacently so the 16x 32x32 sub-arrays run them concurrently.
Softmax denominators come from an extra ones column interleaved into v
(M=65 PV matmuls) and are broadcast across partitions with a DRAM-bounce
DMA. Causal masking only touches the 128 diagonal columns of each k-tile.
"""

import numpy as np

import concourse.bass as bass
import concourse.tile as tile
from concourse import bacc, mybir
from concourse.bass_utils import run_bass_kernel_spmd

F32 = mybir.dt.float32
F16 = mybir.dt.float16
ADD = mybir.AluOpType.add
MULT = mybir.AluOpType.mult

N_CORES = 8
B, T, C = 4, 2048, 1024
NH_TOT, D = 16, 64
F = 512            # features per core (8 heads)
NH = 8             # local heads
NPAIR = 4          # head pairs (128 feats each)
CCH = C // 128     # 8 contraction chunks
NTT = T // 128     # 16 t tiles
NTB = T // 512     # 4 t blocks (qkv production)
NQB = T // 512     # 4 q blocks (attention)
VW = NH * (D + 1)  # 520: augmented v width (per-head [v(64), one])


def _emit(tc, aps):
    from contextlib import ExitStack
    nc = tc.nc
    xt, wq, wk, wva, bq, bk, wp = (
        aps["xt"], aps["wq"], aps["wk"], aps["wva"], aps["bq"], aps["bk"],
        aps["wp"])
    cmask = aps["cmask"]
    out_ab = [aps["out_pa"], aps["out_pb"]]

    ctx = ExitStack()
    # PSUM: pp_main 3x[128,1024] = 6 banks, pp_pv 2x[128,512] = 2 banks
    pp_main = ctx.enter_context(tc.tile_pool(name="ps_main", bufs=3,
                                             space="PSUM"))
    pp_pv = ctx.enter_context(tc.tile_pool(name="ps_pv", bufs=2, space="PSUM"))
    po_xt = ctx.enter_context(tc.tile_pool(name="xt", bufs=1))
    po_v = ctx.enter_context(tc.tile_pool(name="v_all", bufs=1))
    po_mask = ctx.enter_context(tc.tile_pool(name="mask", bufs=1))
    po_wva = ctx.enter_context(tc.tile_pool(name="wva", bufs=8))
    po_qkt = ctx.enter_context(tc.tile_pool(name="qkT", bufs=4))
    po_bias = ctx.enter_context(tc.tile_pool(name="bias", bufs=1))
    po_wqk = ctx.enter_context(tc.tile_pool(name="wqk", bufs=8))
    po_yt = ctx.enter_context(tc.tile_pool(name="yT", bufs=4))
    po_et = ctx.enter_context(tc.tile_pool(name="expT", bufs=4))
    po_rec = ctx.enter_context(tc.tile_pool(name="recip", bufs=3))
    po_den = ctx.enter_context(tc.tile_pool(name="den", bufs=2))
    po_ytmp = ctx.enter_context(tc.tile_pool(name="ytmp", bufs=2))
    po_ot = ctx.enter_context(tc.tile_pool(name="ot", bufs=3))
    po_wp = ctx.enter_context(tc.tile_pool(name="wp", bufs=4))
    po_dram = ctx.enter_context(tc.tile_pool(name="dram_scr", bufs=4,
                                             space="DRAM"))

    mask_sb = po_mask.tile([128, 128], F32, tag="mask")
    nc.sync.dma_start(out=mask_sb[:], in_=cmask[:])
    # bva broadcast to all 128 partitions straight from DRAM ([1,VW] src)
    bva_bc = po_bias.tile([128, VW], F32, tag="bva_bc")
    bva2 = aps["bva2"]
    nc.sync.dma_start(out=bva_bc[:], in_=bass.AP(
        tensor=bva2.tensor, offset=bva2.offset,
        ap=[[0, 128]] + [list(a) for a in bva2.ap[1:]]))

    # ---- phase 0: xT straight from DRAM (host pre-transposed) ----
    xT = [po_xt.tile([128, T], F16, tag=f"xT{c}", name=f"xT{c}")
          for c in range(CCH)]
    for c in range(CCH):
        nc.sync.dma_start(out=xT[c][:], in_=xt[c * 128:(c + 1) * 128, :])

    # ---- phase 0b: v tiles [128, 520] = 8 heads x [v(64) | one],
    # wva host-interleaved with ones columns ----
    v16 = [po_v.tile([128, VW], F16, tag=f"v{tt}", name=f"v{tt}")
           for tt in range(NTT)]
    wva_sb = []
    for c in range(CCH):
        wt = po_wva.tile([128, VW], F16, tag="wva")
        nc.sync.dma_start(out=wt[:], in_=wva[c * 128:(c + 1) * 128, :])
        wva_sb.append(wt)

    def v_units():
        units = []
        for tt in range(NTT):
            def unit(tt=tt):
                ps2 = pp_main.tile([128, 1024], F32, tag="main", name="psv")
                for half in range(2):
                    cs = slice(half * 260, half * 260 + 260)
                    ps = ps2[:, half * 512:half * 512 + 260]
                    for c in range(CCH):
                        nc.tensor.matmul(
                            ps, xT[c][:, tt * 128:(tt + 1) * 128],
                            wva_sb[c][:, cs], start=(c == 0),
                            stop=(c == CCH - 1))
                    nc.vector.tensor_add(v16[tt][:, cs], ps, bva_bc[:, cs])
            units.append(unit)
        return units

    # ---- per head pair: qkv -> attention -> partial proj ----
    def prep_qkv(pair):
        psl = slice(pair * 128, (pair + 1) * 128)
        wqk_c = []
        for c in range(CCH):
            wt = po_wqk.tile([128, 256], F16, tag="wqk", name="wt")
            nc.sync.dma_start(out=wt[:, 0:128],
                              in_=wq[c * 128:(c + 1) * 128, psl])
            nc.sync.dma_start(out=wt[:, 128:256],
                              in_=wk[c * 128:(c + 1) * 128, psl])
            wqk_c.append(wt)
        bq_sb = po_bias.tile([128, 1], F32, tag=f"bq{pair}", name=f"bq{pair}")
        nc.sync.dma_start(out=bq_sb[:], in_=bq[psl, :])
        bk_sb = po_bias.tile([128, 1], F32, tag=f"bk{pair}", name=f"bk{pair}")
        nc.sync.dma_start(out=bk_sb[:], in_=bk[psl, :])
        qT = po_qkt.tile([128, T], F16, tag="qT", name="qT")
        kT = po_qkt.tile([128, T], F16, tag="kT", name="kT")
        return dict(wqk=wqk_c, bq=bq_sb, bk=bk_sb, qT=qT, kT=kT)

    def qkv_units(st8):
        units = []
        for tb in range(NTB):
            def unit(tb=tb):
                tsl = slice(tb * 512, (tb + 1) * 512)
                ps2 = pp_main.tile([128, 1024], F32, tag="main", name="psqk")
                psq, psk = ps2[:, 0:512], ps2[:, 512:1024]
                for c in range(CCH):
                    nc.tensor.matmul(psq, st8["wqk"][c][:, 0:128],
                                     xT[c][:, tsl],
                                     start=(c == 0), stop=(c == CCH - 1))
                for c in range(CCH):
                    nc.tensor.matmul(psk, st8["wqk"][c][:, 128:256],
                                     xT[c][:, tsl],
                                     start=(c == 0), stop=(c == CCH - 1))
                # psum*1/sqrt(D) + bq/sqrt(D)   (bq pre-scaled on host)
                nc.vector.tensor_scalar(
                    out=st8["qT"][:, tsl], in0=psq, scalar1=0.125,
                    scalar2=st8["bq"][:], op0=MULT, op1=ADD)
                nc.vector.tensor_scalar(
                    out=st8["kT"][:, tsl], in0=psk, scalar1=st8["bk"][:],
                    scalar2=None, op0=ADD)
            units.append(unit)
        return units

    def attn_units(st8, yt):
        qT, kT = st8["qT"], st8["kT"]
        units = []
        for qb in range(NQB):
            nkt = 4 * qb + 4
            kpairs = [(2 * i, 2 * i + 1) for i in range(nkt // 2)]
            L = len(kpairs)
            state = {"st": {}, "et": {}, "pv": None}

            def offs_of(kp, qb=qb):
                return tuple(max(0, 128 * (kt - 4 * qb)) for kt in kp)

            def emit_scores(i, qb=qb, kpairs=kpairs, state=state,
                            offs_of=offs_of):
                kp = kpairs[i]
                offs = offs_of(kp)
                sts = []
                for s in range(2):
                    sts.append(pp_main.tile([128, 1024], F32, tag="main",
                                            name=f"st{s}"))
                for j in range(2):   # k-tile within pair, outer for adjacency
                    for s in range(2):  # head A/B adjacent -> concurrent
                        rq = slice(s * 64, s * 64 + 64)
                        kt, off = kp[j], offs[j]
                        nc.tensor.matmul(
                            sts[s][:, j * 512 + off:(j + 1) * 512],
                            kT[rq, kt * 128:(kt + 1) * 128],
                            qT[rq, qb * 512 + off:(qb + 1) * 512],
                            start=True, stop=True)
                for s in range(2):
                    for j in range(2):
                        kt, off = kp[j], offs[j]
                        if kt >= 4 * qb:  # diagonal tile: triangular mask
                            nc.vector.tensor_add(
                                sts[s][:, j * 512 + off:j * 512 + off + 128],
                                sts[s][:, j * 512 + off:j * 512 + off + 128],
                                mask_sb[:])
                state["st"][i] = sts

            def emit_exp(i, kpairs=kpairs, state=state, offs_of=offs_of):
                offs = offs_of(kpairs[i])
                lo = offs[0]  # 0 or 256; single op over [lo:1024]
                sts = state["st"].pop(i)
                ets = []
                for s in range(2):
                    et = po_et.tile([128, 1024], F16, tag="expT", name="et")
                    nc.scalar.activation(
                        et[:, lo:1024], sts[s][:, lo:1024],
                        mybir.ActivationFunctionType.Exp)
                    ets.append(et)
                state["et"][i] = ets

            def emit_pv(i, qb=qb, kpairs=kpairs, nkt=nkt, state=state,
                        offs_of=offs_of):
                kp = kpairs[i]
                offs = offs_of(kp)
                ets = state["et"].pop(i)
                pv = state["pv"]
                for s in range(2):
                    pair_h = st8["pair"] * 2 + s
                    vsl = slice(pair_h * 65, pair_h * 65 + 65)
                    for j in range(2):
                        kt, off = kp[j], offs[j]
                        nc.tensor.matmul(
                            pv[s][0:65, off:512], v16[kt][:, vsl],
                            ets[s][:, j * 512 + off:(j + 1) * 512],
                            start=(kt == 0), stop=(kt == nkt - 1))

            def finish(qb=qb, state=state):
                pv = state["pv"]
                qsl = slice(qb * 512, (qb + 1) * 512)
                for s in range(2):
                    den = po_den.tile([128, 512], F32, tag="den", name="den")
                    nc.vector.tensor_copy(den[64:65, :], pv[s][64:65, :])
                    dscr = po_dram.tile([1, 512], F32, tag="dscr",
                                        name="dscr")
                    nc.sync.dma_start(out=dscr[:], in_=den[64:65, :])
                    rec = po_rec.tile([128, 512], F32, tag="recip",
                                      name="rec")
                    nc.sync.dma_start(out=rec[0:64, :], in_=bass.AP(
                        tensor=dscr.tensor, offset=dscr[:].offset,
                        ap=[[0, 64]] + [list(a) for a in dscr[:].ap[1:]]))
                    nc.vector.reciprocal_approx_fast(rec[0:64, :],
                                                     rec[0:64, :])
                    if s == 0:
                        nc.vector.tensor_mul(yt[0:64, qsl], pv[s][0:64, :],
                                             rec[0:64, :])
                    else:
                        # engines can't cross partitions; bounce via DMA
                        ytmp = po_ytmp.tile([128, 512], F16, tag="ytmp",
                                            name="ytmp")
                        nc.vector.tensor_mul(ytmp[0:64, :], pv[s][0:64, :],
                                             rec[0:64, :])
                        nc.sync.dma_start(out=yt[64:128, qsl],
                                          in_=ytmp[0:64, :])

            def unit(i, L=L, state=state, emit_exp=emit_exp,
                     emit_pv=emit_pv, emit_scores=emit_scores,
                     finish=finish):
                # pipeline: exp(i-1) first, then pv(i-2), then scores(i)
                if i == 0:
                    state["pv"] = [
                        pp_pv.tile([128, 512], F32, tag="pv", name=f"pv{s}")
                        for s in range(2)]
                if 1 <= i <= L:
                    emit_exp(i - 1)
                if 2 <= i <= L + 1:
                    emit_pv(i - 2)
                if i < L:
                    emit_scores(i)
                if i == L + 1:
                    finish()

            for i in range(L + 2):
                units.append(lambda i=i, u=unit: u(i))
        return units

    def prep_proj(couple):
        wp_sb = []
        for pq in range(2):
            for cb in range(2):
                prow = (couple * 2 + pq) * 128
                wt = po_wp.tile([128, 512], F16, tag="wp", name="wpt")
                nc.sync.dma_start(
                    out=wt[:],
                    in_=wp[prow:prow + 128, cb * 512:(cb + 1) * 512])
                wp_sb.append(wt)
        return wp_sb

    def proj_units(couple, wp_sb, yts):
        out_p = out_ab[couple]
        units = []
        for tt0 in range(0, NTT, 2):
            def unit(tt0=tt0):
                for tt in (tt0, tt0 + 1):
                    ot = po_ot.tile([128, C], F16, tag="ot", name="ot")
                    ps2 = pp_main.tile([128, 1024], F32, tag="main",
                                       name="pproj")
                    # pq outer: stationary (y tile) reused across cb
                    for pq in range(2):
                        for cb in range(2):
                            nc.tensor.matmul(
                                ps2[:, cb * 512:(cb + 1) * 512],
                                yts[pq][:, tt * 128:(tt + 1) * 128],
                                wp_sb[pq * 2 + cb][:],
                                start=(pq == 0), stop=(pq == 1))
                    for cb in range(2):
                        nc.vector.tensor_copy(
                            ot[:, cb * 512:(cb + 1) * 512],
                            ps2[:, cb * 512:(cb + 1) * 512])
                    nc.sync.dma_start(
                        out=out_p[tt * 128:(tt + 1) * 128, :], in_=ot[:])
            units.append(unit)
        return units

    def round_robin(*streams):
        streams = [list(s) for s in streams if s]
        while any(streams):
            for s in streams:
                if s:
                    s.pop(0)()

    for u in v_units():
        u()
    pair_state = []
    yts = []
    st0 = prep_qkv(0)
    st0["pair"] = 0
    pair_state.append(st0)
    for u in qkv_units(st0):
        u()
    for p in range(NPAIR):
        yt = po_yt.tile([128, T], F16, tag="yT", name="yt")
        yts.append(yt)
        streams = []
        if p + 1 < NPAIR:
            stn = prep_qkv(p + 1)
            stn["pair"] = p + 1
            pair_state.append(stn)
            streams.append(qkv_units(stn))
        if p == 2:
            wp_sb = prep_proj(0)
            streams.append(proj_units(0, wp_sb, yts[0:2]))
        streams.append(attn_units(pair_state[p], yt))
        round_robin(*streams)
    wp_sb = prep_proj(1)
    for u in proj_units(1, wp_sb, yts[2:4]):
        u()

    ctx.close()


_CACHE = {}


def _build():
    if "nc" in _CACHE:
        return _CACHE["nc"]
    nc = bacc.Bacc("TRN2", target_bir_lowering=False, debug=False,
                   enable_asserts=True, num_devices=N_CORES)
    aps = {
        "xt": nc.dram_tensor("xt", [C, T], F16, kind="ExternalInput").ap(),
        "wq": nc.dram_tensor("wq", [C, F], F16, kind="ExternalInput").ap(),
        "wk": nc.dram_tensor("wk", [C, F], F16, kind="ExternalInput").ap(),
        "wva": nc.dram_tensor("wva", [C, VW], F16, kind="ExternalInput").ap(),
        "bq": nc.dram_tensor("bq", [F, 1], F32, kind="ExternalInput").ap(),
        "bk": nc.dram_tensor("bk", [F, 1], F32, kind="ExternalInput").ap(),
        "bva2": nc.dram_tensor("bva2", [1, VW], F32,
                               kind="ExternalInput").ap(),
        "wp": nc.dram_tensor("wp", [F, C], F16, kind="ExternalInput").ap(),
        "cmask": nc.dram_tensor("cmask", [128, 128], F32,
                                kind="ExternalInput").ap(),
        "out_pa": nc.dram_tensor("out_pa", [T, C], F16,
                                 kind="ExternalOutput").ap(),
        "out_pb": nc.dram_tensor("out_pb", [T, C], F16,
                                 kind="ExternalOutput").ap(),
    }
    with tile.TileContext(nc) as tc:
        _emit(tc, aps)
    nc.compile()
    _CACHE["nc"] = nc
    return nc


def _make_in_maps(x, Wqkv, bqkv, Wproj):
    x = np.asarray(x, dtype=np.float32)
    Wqkv = np.asarray(Wqkv, dtype=np.float32)
    bqkv = np.asarray(bqkv, dtype=np.float32)
    Wproj = np.asarray(Wproj, dtype=np.float32)

    # triangular causal mask: M[p, f] = 0 if f >= p else -1e9
    p_idx = np.arange(128)[:, None]
    u_idx = np.arange(128)[None, :]
    cmask = np.where(u_idx >= p_idx, 0.0, -1e9).astype(np.float32)

    in_maps = []
    for core in range(N_CORES):
        b, g = divmod(core, 2)
        q0, k0, v0 = 512 * g, C + 512 * g, 2 * C + 512 * g
        wva = np.zeros((C, VW), dtype=np.float32)
        bva = np.zeros((1, VW), dtype=np.float32)
        for h in range(NH):
            src = v0 + D * h
            dst = 65 * h
            # per-head layout [v(64), one]
            wva[:, dst:dst + 64] = Wqkv[:, src:src + 64]
            bva[0, dst:dst + 64] = bqkv[src:src + 64]
            bva[0, dst + 64] = 1.0
        in_maps.append({
            "xt": np.ascontiguousarray(x[b].T).astype(np.float16),
            "wq": np.ascontiguousarray(Wqkv[:, q0:q0 + F]).astype(np.float16),
            "wk": np.ascontiguousarray(Wqkv[:, k0:k0 + F]).astype(np.float16),
            "wva": np.ascontiguousarray(wva).astype(np.float16),
            "bq": np.ascontiguousarray(bqkv[q0:q0 + F].reshape(F, 1) * 0.125),
            "bk": np.ascontiguousarray(bqkv[k0:k0 + F].reshape(F, 1)),
            "bva2": np.ascontiguousarray(bva),
            "wp": np.ascontiguousarray(
                Wproj[512 * g:512 * g + F, :]).astype(np.float16),
            "cmask": cmask,
        })
    return in_maps


def run_sharded(x, Wqkv, bqkv, Wproj, bproj, trace=False):
    nc = _build()
    in_maps = _make_in_maps(x, Wqkv, bqkv, Wproj)
    res = run_bass_kernel_spmd(nc, in_maps, core_ids=list(range(N_CORES)),
                               trace=trace)
    bproj = np.asarray(bproj, dtype=np.float32)
    out = np.empty((B, T, C), dtype=np.float32)
    for b in range(B):
        acc = bproj[None, :].astype(np.float32).repeat(T, axis=0)
        for core in (2 * b, 2 * b + 1):
            acc = acc + res.results[core]["out_pa"].astype(np.float32) \
                + res.results[core]["out_pb"].astype(np.float32)
        out[b] = acc
    return out, res


def kernel(x, Wqkv, bqkv, Wproj, bproj):
    out, _ = run_sharded(x, Wqkv, bqkv, Wproj, bproj, trace=False)
    return out


# revision 20
# speedup vs baseline: 1.2251x; 1.1126x over previous
"""Causal self-attention (B=4, T=2048, C=1024, H=16, Dh=64) on 8 trn2 NeuronCores.

Sharding: core i <-> (batch b = i//2, head-group g = i%2). Each core computes
8 heads of one batch end-to-end (qkv slice, causal attention, partial output
projection); the host sums the head-group partials per batch and adds bproj.
No device collectives.

v2 layout: x arrives pre-transposed from the host ([C, T]) so no PE
transposes are needed. Scores use the transposed layout sT[tk, tq]; the two
heads of a pair occupy PE row-groups 0-63 / 64-127 and their score matmuls
are emitted adjacently so the 16x 32x32 sub-arrays run them concurrently.
Softmax denominators come from an extra ones column interleaved into v
(M=65 PV matmuls) and are broadcast across partitions with a DRAM-bounce
DMA. Causal masking only touches the 128 diagonal columns of each k-tile.
"""

import numpy as np

import concourse.bass as bass
import concourse.tile as tile
from concourse import bacc, mybir
from concourse.bass_utils import run_bass_kernel_spmd

F32 = mybir.dt.float32
F16 = mybir.dt.float16
F8 = mybir.dt.float8e4
DR = mybir.MatmulPerfMode.DoubleRow
ADD = mybir.AluOpType.add
MULT = mybir.AluOpType.mult
USE_DR = True        # fp8 DoubleRow PV for off-diagonal k-pairs
USE_GPS_DMA = False  # issue bounce DMAs from the GpSimd queue

N_CORES = 8
B, T, C = 4, 2048, 1024
NH_TOT, D = 16, 64
F = 512            # features per core (8 heads)
NH = 8             # local heads
NPAIR = 4          # head pairs (128 feats each)
CCH = C // 128     # 8 contraction chunks
NTT = T // 128     # 16 t tiles
NTB = T // 512     # 4 t blocks (qkv production)
NQB = T // 512     # 4 q blocks (attention)
VW = NH * (D + 1)  # 520: augmented v width (per-head [v(64), one])


def _emit(tc, aps):
    from contextlib import ExitStack
    nc = tc.nc
    xt, wq, wk, wva, bq, bk, wp = (
        aps["xt"], aps["wq"], aps["wk"], aps["wva"], aps["bq"], aps["bk"],
        aps["wp"])
    cmask = aps["cmask"]
    out_ab = [aps["out_pa"], aps["out_pb"]]

    ctx = ExitStack()
    # PSUM: pp_main 3x[128,1024] = 6 banks, pp_pv 2x[128,512] = 2 banks
    pp_main = ctx.enter_context(tc.tile_pool(name="ps_main", bufs=3,
                                             space="PSUM"))
    pp_pv = ctx.enter_context(tc.tile_pool(name="ps_pv", bufs=2, space="PSUM"))
    po_xt = ctx.enter_context(tc.tile_pool(name="xt", bufs=1))
    po_v = ctx.enter_context(tc.tile_pool(name="v_all", bufs=1))
    po_v8 = ctx.enter_context(tc.tile_pool(name="v8", bufs=1))
    po_ysb = ctx.enter_context(tc.tile_pool(name="ysb", bufs=2))
    po_mask = ctx.enter_context(tc.tile_pool(name="mask", bufs=1))
    po_wva = ctx.enter_context(tc.tile_pool(name="wva", bufs=8))
    po_qkt = ctx.enter_context(tc.tile_pool(name="qkT", bufs=4))
    po_bias = ctx.enter_context(tc.tile_pool(name="bias", bufs=1))
    po_wqk = ctx.enter_context(tc.tile_pool(name="wqk", bufs=8))
    po_yt = ctx.enter_context(tc.tile_pool(name="yT", bufs=4))
    po_et = ctx.enter_context(tc.tile_pool(name="expT", bufs=4))
    po_rec = ctx.enter_context(tc.tile_pool(name="recip", bufs=3))
    po_den = ctx.enter_context(tc.tile_pool(name="den", bufs=2))
    po_ytmp = ctx.enter_context(tc.tile_pool(name="ytmp", bufs=2))
    po_ot = ctx.enter_context(tc.tile_pool(name="ot", bufs=3))
    po_wp = ctx.enter_context(tc.tile_pool(name="wp", bufs=4))
    po_dram = ctx.enter_context(tc.tile_pool(name="dram_scr", bufs=4,
                                             space="DRAM"))

    mask_sb = po_mask.tile([128, 128], F32, tag="mask")
    nc.sync.dma_start(out=mask_sb[:], in_=cmask[:])
    # bva broadcast to all 128 partitions straight from DRAM ([1,VW] src)
    bva_bc = po_bias.tile([128, VW], F32, tag="bva_bc")
    bva2 = aps["bva2"]
    nc.sync.dma_start(out=bva_bc[:], in_=bass.AP(
        tensor=bva2.tensor, offset=bva2.offset,
        ap=[[0, 128]] + [list(a) for a in bva2.ap[1:]]))

    # ---- phase 0: xT straight from DRAM (host pre-transposed) ----
    xT = [po_xt.tile([128, T], F16, tag=f"xT{c}", name=f"xT{c}")
          for c in range(CCH)]
    for c in range(CCH):
        nc.sync.dma_start(out=xT[c][:], in_=xt[c * 128:(c + 1) * 128, :])

    # ---- phase 0b: v tiles [128, 520] = 8 heads x [v(64) | one],
    # wva host-interleaved with ones columns.  v8: fp8 copy, k-tile pairs
    # packed for DoubleRow ([0:520] = even kt, [528:1048] = odd kt). ----
    v16 = [po_v.tile([128, VW], F16, tag=f"v{tt}", name=f"v{tt}")
           for tt in range(NTT)]
    v8 = [po_v8.tile([128, 1056], F8, tag=f"v8_{kp}", name=f"v8_{kp}")
          for kp in range(NTT // 2)]
    wva_sb = []
    for c in range(CCH):
        wt = po_wva.tile([128, VW], F16, tag="wva")
        nc.sync.dma_start(out=wt[:], in_=wva[c * 128:(c + 1) * 128, :])
        wva_sb.append(wt)

    def v_units():
        units = []
        for tt in range(NTT):
            def unit(tt=tt):
                ps2 = pp_main.tile([128, 1024], F32, tag="main", name="psv")
                for half in range(2):
                    cs = slice(half * 260, half * 260 + 260)
                    ps = ps2[:, half * 512:half * 512 + 260]
                    for c in range(CCH):
                        nc.tensor.matmul(
                            ps, xT[c][:, tt * 128:(tt + 1) * 128],
                            wva_sb[c][:, cs], start=(c == 0),
                            stop=(c == CCH - 1))
                    nc.vector.tensor_add(v16[tt][:, cs], ps, bva_bc[:, cs])
                if USE_DR:
                    # fp8 copy of the finished v16 tile (SBUF->SBUF)
                    o = (tt % 2) * 528
                    nc.vector.tensor_copy(v8[tt // 2][:, o:o + VW],
                                          v16[tt][:, 0:VW])
            units.append(unit)
        return units

    # ---- per head pair: qkv -> attention -> partial proj ----
    def prep_qkv(pair):
        psl = slice(pair * 128, (pair + 1) * 128)
        wqk_c = []
        for c in range(CCH):
            wt = po_wqk.tile([128, 256], F16, tag="wqk", name="wt")
            nc.sync.dma_start(out=wt[:, 0:128],
                              in_=wq[c * 128:(c + 1) * 128, psl])
            nc.sync.dma_start(out=wt[:, 128:256],
                              in_=wk[c * 128:(c + 1) * 128, psl])
            wqk_c.append(wt)
        bq_sb = po_bias.tile([128, 1], F32, tag=f"bq{pair}", name=f"bq{pair}")
        nc.sync.dma_start(out=bq_sb[:], in_=bq[psl, :])
        bk_sb = po_bias.tile([128, 1], F32, tag=f"bk{pair}", name=f"bk{pair}")
        nc.sync.dma_start(out=bk_sb[:], in_=bk[psl, :])
        qT = po_qkt.tile([128, T], F16, tag="qT", name="qT")
        kT = po_qkt.tile([128, T], F16, tag="kT", name="kT")
        return dict(wqk=wqk_c, bq=bq_sb, bk=bk_sb, qT=qT, kT=kT)

    def qkv_units(st8):
        units = []
        for tb in range(NTB):
            def unit(tb=tb):
                tsl = slice(tb * 512, (tb + 1) * 512)
                ps2 = pp_main.tile([128, 1024], F32, tag="main", name="psqk")
                psq, psk = ps2[:, 0:512], ps2[:, 512:1024]
                for c in range(CCH):
                    nc.tensor.matmul(psq, st8["wqk"][c][:, 0:128],
                                     xT[c][:, tsl],
                                     start=(c == 0), stop=(c == CCH - 1))
                for c in range(CCH):
                    nc.tensor.matmul(psk, st8["wqk"][c][:, 128:256],
                                     xT[c][:, tsl],
                                     start=(c == 0), stop=(c == CCH - 1))
                # psum*1/sqrt(D) + bq/sqrt(D)   (bq pre-scaled on host)
                nc.vector.tensor_scalar(
                    out=st8["qT"][:, tsl], in0=psq, scalar1=0.125,
                    scalar2=st8["bq"][:], op0=MULT, op1=ADD)
                nc.vector.tensor_scalar(
                    out=st8["kT"][:, tsl], in0=psk, scalar1=st8["bk"][:],
                    scalar2=None, op0=ADD)
            units.append(unit)
        return units

    def attn_units(st8, yt):
        qT, kT = st8["qT"], st8["kT"]
        units = []
        for qb in range(NQB):
            nkt = 4 * qb + 4
            kpairs = [(2 * i, 2 * i + 1) for i in range(nkt // 2)]
            L = len(kpairs)
            state = {"st": {}, "et": {}, "pv": None}

            def offs_of(kp, qb=qb):
                return tuple(max(0, 128 * (kt - 4 * qb)) for kt in kp)

            def emit_scores(i, qb=qb, kpairs=kpairs, state=state,
                            offs_of=offs_of):
                kp = kpairs[i]
                offs = offs_of(kp)
                sts = []
                for s in range(2):
                    sts.append(pp_main.tile([128, 1024], F32, tag="main",
                                            name=f"st{s}"))
                for j in range(2):   # k-tile within pair, outer for adjacency
                    for s in range(2):  # head A/B adjacent -> concurrent
                        rq = slice(s * 64, s * 64 + 64)
                        kt, off = kp[j], offs[j]
                        nc.tensor.matmul(
                            sts[s][:, j * 512 + off:(j + 1) * 512],
                            kT[rq, kt * 128:(kt + 1) * 128],
                            qT[rq, qb * 512 + off:(qb + 1) * 512],
                            start=True, stop=True)
                for s in range(2):
                    for j in range(2):
                        kt, off = kp[j], offs[j]
                        if kt >= 4 * qb:  # diagonal tile: triangular mask
                            nc.vector.tensor_add(
                                sts[s][:, j * 512 + off:j * 512 + off + 128],
                                sts[s][:, j * 512 + off:j * 512 + off + 128],
                                mask_sb[:])
                state["st"][i] = sts

            def emit_exp(i, qb=qb, kpairs=kpairs, state=state,
                         offs_of=offs_of):
                kp = kpairs[i]
                offs = offs_of(kp)
                lo = offs[0]  # 0 or 256; single op over [lo:1024]
                full = USE_DR and kp[1] < 4 * qb
                sts = state["st"].pop(i)
                ets = []
                for s in range(2):
                    et = po_et.tile([128, 1024], F8 if full else F16,
                                    tag="expT8" if full else "expT",
                                    name="et")
                    nc.scalar.activation(
                        et[:, lo:1024], sts[s][:, lo:1024],
                        mybir.ActivationFunctionType.Exp)
                    ets.append(et)
                state["et"][i] = ets

            def emit_pv(i, qb=qb, kpairs=kpairs, nkt=nkt, state=state,
                        offs_of=offs_of):
                kp = kpairs[i]
                offs = offs_of(kp)
                full = USE_DR and kp[1] < 4 * qb
                ets = state["et"].pop(i)
                pv = state["pv"]
                for s in range(2):
                    pair_h = st8["pair"] * 2 + s
                    if full:
                        # fp8 DoubleRow: both k-tiles in one matmul
                        v8t = v8[kp[0] // 2]
                        lhsT = bass.AP(
                            tensor=v8t.tensor,
                            offset=v8t[:].offset + pair_h * 65,
                            ap=[list(v8t[:].ap[0]), [528, 2], [1, 65]])
                        et = ets[s]
                        rhs = bass.AP(
                            tensor=et.tensor, offset=et[:].offset,
                            ap=[list(et[:].ap[0]), [512, 2], [1, 512]])
                        nc.tensor.matmul(
                            pv[s][0:65, 0:512], lhsT, rhs,
                            start=(kp[0] == 0), stop=False, perf_mode=DR,
                            skip_group_check=True)
                        continue
                    vsl = slice(pair_h * 65, pair_h * 65 + 65)
                    for j in range(2):
                        kt, off = kp[j], offs[j]
                        nc.tensor.matmul(
                            pv[s][0:65, off:512], v16[kt][:, vsl],
                            ets[s][:, j * 512 + off:(j + 1) * 512],
                            start=(kt == 0), stop=(kt == nkt - 1),
                            skip_group_check=True)

            def finish(qb=qb, state=state):
                pv = state["pv"]
                qsl = slice(qb * 512, (qb + 1) * 512)
                for s in range(2):
                    # evacuate pv early so the bank frees for the next qb
                    ysb = po_ysb.tile([128, 512], F32, tag="ysb",
                                      name="ysb")
                    nc.vector.tensor_copy(ysb[0:65, :], pv[s][0:65, :])
                    dscr = po_dram.tile([1, 512], F32, tag="dscr",
                                        name="dscr")
                    (nc.gpsimd if USE_GPS_DMA else nc.sync).dma_start(out=dscr[:], in_=ysb[64:65, :])
                    rec = po_rec.tile([128, 512], F32, tag="recip",
                                      name="rec")
                    (nc.gpsimd if USE_GPS_DMA else nc.sync).dma_start(out=rec[0:64, :], in_=bass.AP(
                        tensor=dscr.tensor, offset=dscr[:].offset,
                        ap=[[0, 64]] + [list(a) for a in dscr[:].ap[1:]]))
                    nc.vector.reciprocal_approx_fast(rec[0:64, :],
                                                     rec[0:64, :])
                    if s == 0:
                        nc.vector.tensor_mul(yt[0:64, qsl], ysb[0:64, :],
                                             rec[0:64, :])
                    else:
                        # engines can't cross partitions; bounce via DMA
                        ytmp = po_ytmp.tile([128, 512], F16, tag="ytmp",
                                            name="ytmp")
                        nc.vector.tensor_mul(ytmp[0:64, :], ysb[0:64, :],
                                             rec[0:64, :])
                        (nc.gpsimd if USE_GPS_DMA else nc.sync).dma_start(out=yt[64:128, qsl],
                                            in_=ytmp[0:64, :])

            def unit(i, L=L, state=state, emit_exp=emit_exp,
                     emit_pv=emit_pv, emit_scores=emit_scores,
                     finish=finish):
                # pipeline: exp(i-1) first, then pv(i-2), then scores(i)
                if i == 0:
                    state["pv"] = [
                        pp_pv.tile([128, 512], F32, tag="pv", name=f"pv{s}")
                        for s in range(2)]
                if 1 <= i <= L:
                    emit_exp(i - 1)
                if 2 <= i <= L + 1:
                    emit_pv(i - 2)
                if i < L:
                    emit_scores(i)
                if i == L + 1:
                    finish()

            for i in range(L + 2):
                units.append(lambda i=i, u=unit: u(i))
        return units

    def prep_proj(couple):
        wp_sb = []
        for pq in range(2):
            for cb in range(2):
                prow = (couple * 2 + pq) * 128
                wt = po_wp.tile([128, 512], F16, tag="wp", name="wpt")
                nc.sync.dma_start(
                    out=wt[:],
                    in_=wp[prow:prow + 128, cb * 512:(cb + 1) * 512])
                wp_sb.append(wt)
        return wp_sb

    def proj_units(couple, wp_sb, yts):
        out_p = out_ab[couple]
        units = []
        for tt0 in range(0, NTT, 2):
            def unit(tt0=tt0):
                for tt in (tt0, tt0 + 1):
                    ot = po_ot.tile([128, C], F16, tag="ot", name="ot")
                    ps2 = pp_main.tile([128, 1024], F32, tag="main",
                                       name="pproj")
                    # pq outer: stationary (y tile) reused across cb
                    for pq in range(2):
                        for cb in range(2):
                            nc.tensor.matmul(
                                ps2[:, cb * 512:(cb + 1) * 512],
                                yts[pq][:, tt * 128:(tt + 1) * 128],
                                wp_sb[pq * 2 + cb][:],
                                start=(pq == 0), stop=(pq == 1))
                    for cb in range(2):
                        nc.vector.tensor_copy(
                            ot[:, cb * 512:(cb + 1) * 512],
                            ps2[:, cb * 512:(cb + 1) * 512])
                    nc.sync.dma_start(
                        out=out_p[tt * 128:(tt + 1) * 128, :], in_=ot[:])
            units.append(unit)
        return units

    def round_robin(*streams):
        streams = [list(s) for s in streams if s]
        while any(streams):
            for s in streams:
                if s:
                    s.pop(0)()

    for u in v_units():
        u()
    pair_state = []
    yts = []
    st0 = prep_qkv(0)
    st0["pair"] = 0
    pair_state.append(st0)
    for u in qkv_units(st0):
        u()
    for p in range(NPAIR):
        yt = po_yt.tile([128, T], F16, tag="yT", name="yt")
        yts.append(yt)
        streams = []
        if p + 1 < NPAIR:
            stn = prep_qkv(p + 1)
            stn["pair"] = p + 1
            pair_state.append(stn)
            streams.append(qkv_units(stn))
        if p == 2:
            wp_sb = prep_proj(0)
            streams.append(proj_units(0, wp_sb, yts[0:2]))
        streams.append(attn_units(pair_state[p], yt))
        round_robin(*streams)
    wp_sb = prep_proj(1)
    for u in proj_units(1, wp_sb, yts[2:4]):
        u()

    ctx.close()


_CACHE = {}


def _build():
    if "nc" in _CACHE:
        return _CACHE["nc"]
    nc = bacc.Bacc("TRN2", target_bir_lowering=False, debug=False,
                   enable_asserts=True, num_devices=N_CORES)
    aps = {
        "xt": nc.dram_tensor("xt", [C, T], F16, kind="ExternalInput").ap(),
        "wq": nc.dram_tensor("wq", [C, F], F16, kind="ExternalInput").ap(),
        "wk": nc.dram_tensor("wk", [C, F], F16, kind="ExternalInput").ap(),
        "wva": nc.dram_tensor("wva", [C, VW], F16, kind="ExternalInput").ap(),
        "bq": nc.dram_tensor("bq", [F, 1], F32, kind="ExternalInput").ap(),
        "bk": nc.dram_tensor("bk", [F, 1], F32, kind="ExternalInput").ap(),
        "bva2": nc.dram_tensor("bva2", [1, VW], F32,
                               kind="ExternalInput").ap(),
        "wp": nc.dram_tensor("wp", [F, C], F16, kind="ExternalInput").ap(),
        "cmask": nc.dram_tensor("cmask", [128, 128], F32,
                                kind="ExternalInput").ap(),
        "out_pa": nc.dram_tensor("out_pa", [T, C], F16,
                                 kind="ExternalOutput").ap(),
        "out_pb": nc.dram_tensor("out_pb", [T, C], F16,
                                 kind="ExternalOutput").ap(),
    }
    with tile.TileContext(nc) as tc:
        _emit(tc, aps)
    nc.compile()
    _CACHE["nc"] = nc
    return nc


def _make_in_maps(x, Wqkv, bqkv, Wproj):
    x = np.asarray(x, dtype=np.float32)
    Wqkv = np.asarray(Wqkv, dtype=np.float32)
    bqkv = np.asarray(bqkv, dtype=np.float32)
    Wproj = np.asarray(Wproj, dtype=np.float32)

    # triangular causal mask: M[p, f] = 0 if f >= p else -1e9
    p_idx = np.arange(128)[:, None]
    u_idx = np.arange(128)[None, :]
    cmask = np.where(u_idx >= p_idx, 0.0, -1e9).astype(np.float32)

    in_maps = []
    for core in range(N_CORES):
        b, g = divmod(core, 2)
        q0, k0, v0 = 512 * g, C + 512 * g, 2 * C + 512 * g
        wva = np.zeros((C, VW), dtype=np.float32)
        bva = np.zeros((1, VW), dtype=np.float32)
        for h in range(NH):
            src = v0 + D * h
            dst = 65 * h
            # per-head layout [v(64), one]
            wva[:, dst:dst + 64] = Wqkv[:, src:src + 64]
            bva[0, dst:dst + 64] = bqkv[src:src + 64]
            bva[0, dst + 64] = 1.0
        in_maps.append({
            "xt": np.ascontiguousarray(x[b].T).astype(np.float16),
            "wq": np.ascontiguousarray(Wqkv[:, q0:q0 + F]).astype(np.float16),
            "wk": np.ascontiguousarray(Wqkv[:, k0:k0 + F]).astype(np.float16),
            "wva": np.ascontiguousarray(wva).astype(np.float16),
            "bq": np.ascontiguousarray(bqkv[q0:q0 + F].reshape(F, 1) * 0.125),
            "bk": np.ascontiguousarray(bqkv[k0:k0 + F].reshape(F, 1)),
            "bva2": np.ascontiguousarray(bva),
            "wp": np.ascontiguousarray(
                Wproj[512 * g:512 * g + F, :]).astype(np.float16),
            "cmask": cmask,
        })
    return in_maps


def run_sharded(x, Wqkv, bqkv, Wproj, bproj, trace=False):
    nc = _build()
    in_maps = _make_in_maps(x, Wqkv, bqkv, Wproj)
    res = run_bass_kernel_spmd(nc, in_maps, core_ids=list(range(N_CORES)),
                               trace=trace)
    bproj = np.asarray(bproj, dtype=np.float32)
    out = np.empty((B, T, C), dtype=np.float32)
    for b in range(B):
        acc = bproj[None, :].astype(np.float32).repeat(T, axis=0)
        for core in (2 * b, 2 * b + 1):
            acc = acc + res.results[core]["out_pa"].astype(np.float32) \
                + res.results[core]["out_pb"].astype(np.float32)
        out[b] = acc
    return out, res


def kernel(x, Wqkv, bqkv, Wproj, bproj):
    out, _ = run_sharded(x, Wqkv, bqkv, Wproj, bproj, trace=False)
    return out


# revision 24
# speedup vs baseline: 1.2869x; 1.0504x over previous
"""Causal self-attention (B=4, T=2048, C=1024, H=16, Dh=64) on 8 trn2 NeuronCores.

Sharding: core i <-> (batch b = i//2, head-group g = i%2). Each core computes
8 heads of one batch end-to-end (qkv slice, causal attention, partial output
projection); the host sums the head-group partials per batch and adds bproj.
No device collectives.

v2 layout: x arrives pre-transposed from the host ([C, T]) so no PE
transposes are needed. Scores use the transposed layout sT[tk, tq]; the two
heads of a pair occupy PE row-groups 0-63 / 64-127 and their score matmuls
are emitted adjacently so the 16x 32x32 sub-arrays run them concurrently.
Softmax denominators come from an extra ones column interleaved into v
(M=65 PV matmuls) and are broadcast across partitions with a DRAM-bounce
DMA. Causal masking only touches the 128 diagonal columns of each k-tile.
"""

import ml_dtypes
import numpy as np

import concourse.bass as bass
import concourse.tile as tile
from concourse import bacc, mybir
from concourse.bass_utils import run_bass_kernel_spmd

F32 = mybir.dt.float32
F16 = mybir.dt.float16
F8 = mybir.dt.float8e4
DR = mybir.MatmulPerfMode.DoubleRow
ADD = mybir.AluOpType.add
MULT = mybir.AluOpType.mult
USE_DR = True        # fp8 DoubleRow PV for off-diagonal k-pairs
USE_GPS_DMA = False  # issue bounce DMAs from the GpSimd queue

N_CORES = 8
B, T, C = 4, 2048, 1024
NH_TOT, D = 16, 64
F = 512            # features per core (8 heads)
NH = 8             # local heads
NPAIR = 4          # head pairs (128 feats each)
CCH = C // 128     # 8 contraction chunks
NTT = T // 128     # 16 t tiles
NTB = T // 512     # 4 t blocks (qkv production)
NQB = T // 512     # 4 q blocks (attention)
VW = NH * (D + 1)  # 520: augmented v width (per-head [v(64), one])


def _emit(tc, aps):
    from contextlib import ExitStack
    nc = tc.nc
    xt, wva, bq, bk, wp = (
        aps["xt"], aps["wva"], aps["bq"], aps["bk"], aps["wp"])
    cmask = aps["cmask"]
    out_ab = [aps["out_pa"], aps["out_pb"]]

    ctx = ExitStack()
    # PSUM: pp_main 3x[128,1024] = 6 banks, pp_pv 2x[128,512] = 2 banks
    pp_main = ctx.enter_context(tc.tile_pool(name="ps_main", bufs=3,
                                             space="PSUM"))
    pp_pv = ctx.enter_context(tc.tile_pool(name="ps_pv", bufs=2, space="PSUM"))
    po_xt = ctx.enter_context(tc.tile_pool(name="xt", bufs=1))
    po_xt8 = ctx.enter_context(tc.tile_pool(name="xt8", bufs=1))
    po_v = ctx.enter_context(tc.tile_pool(name="v_all", bufs=1))
    po_v8 = ctx.enter_context(tc.tile_pool(name="v8", bufs=1))
    po_ysb = ctx.enter_context(tc.tile_pool(name="ysb", bufs=2))
    po_mask = ctx.enter_context(tc.tile_pool(name="mask", bufs=1))
    po_wva = ctx.enter_context(tc.tile_pool(name="wva", bufs=8))
    po_qkt = ctx.enter_context(tc.tile_pool(name="qkT", bufs=4))
    po_bias = ctx.enter_context(tc.tile_pool(name="bias", bufs=1))
    po_wqk = ctx.enter_context(tc.tile_pool(name="wqk", bufs=8))
    po_yt = ctx.enter_context(tc.tile_pool(name="yT", bufs=4))
    po_et = ctx.enter_context(tc.tile_pool(name="expT", bufs=4))
    po_rec = ctx.enter_context(tc.tile_pool(name="recip", bufs=3))
    po_den = ctx.enter_context(tc.tile_pool(name="den", bufs=2))
    po_ytmp = ctx.enter_context(tc.tile_pool(name="ytmp", bufs=2))
    po_ot = ctx.enter_context(tc.tile_pool(name="ot", bufs=3))
    po_wp = ctx.enter_context(tc.tile_pool(name="wp", bufs=4))
    po_dram = ctx.enter_context(tc.tile_pool(name="dram_scr", bufs=4,
                                             space="DRAM"))

    mask_sb = po_mask.tile([128, 128], F32, tag="mask")
    nc.sync.dma_start(out=mask_sb[:], in_=cmask[:])
    # bva broadcast to all 128 partitions straight from DRAM ([1,VW] src)
    bva_bc = po_bias.tile([128, VW], F32, tag="bva_bc")
    bva2 = aps["bva2"]
    nc.sync.dma_start(out=bva_bc[:], in_=bass.AP(
        tensor=bva2.tensor, offset=bva2.offset,
        ap=[[0, 128]] + [list(a) for a in bva2.ap[1:]]))

    # ---- phase 0: xT straight from DRAM (host pre-transposed).
    # Startup DMAs rotate across idle engine queues for parallel dispatch.
    dqs = [nc.sync, nc.gpsimd, nc.scalar]
    xT = [po_xt.tile([128, T], F16, tag=f"xT{c}", name=f"xT{c}")
          for c in range(CCH)]
    for c in range(CCH):
        dqs[c % 3].dma_start(out=xT[c][:], in_=xt[c * 128:(c + 1) * 128, :])
    xt8 = aps["xt8"]
    xT8 = [po_xt8.tile([128, 2 * T], F8, tag=f"xT8_{cp}", name=f"xT8_{cp}")
           for cp in range(CCH // 2)]
    for cp in range(CCH // 2):
        dqs[(cp + 2) % 3].dma_start(
            out=xT8[cp][:], in_=xt8[cp * 128:(cp + 1) * 128, :])

    # ---- phase 0b: v tiles [128, 520] = 8 heads x [v(64) | one],
    # wva host-interleaved with ones columns.  v8: fp8 copy, k-tile pairs
    # packed for DoubleRow ([0:520] = even kt, [528:1048] = odd kt). ----
    v16 = [po_v.tile([128, VW], F16, tag=f"v{tt}", name=f"v{tt}")
           for tt in range(NTT)]
    v8 = [po_v8.tile([128, 1056], F8, tag=f"v8_{kp}", name=f"v8_{kp}")
          for kp in range(NTT // 2)]
    wva_sb = []
    for c in range(CCH):
        wt = po_wva.tile([128, VW], F16, tag="wva")
        dqs[(c + 1) % 3].dma_start(out=wt[:],
                                   in_=wva[c * 128:(c + 1) * 128, :])
        wva_sb.append(wt)

    def v_units():
        units = []
        for tt in range(NTT):
            def unit(tt=tt):
                ps2 = pp_main.tile([128, 1024], F32, tag="main", name="psv")
                for half in range(2):
                    cs = slice(half * 260, half * 260 + 260)
                    ps = ps2[:, half * 512:half * 512 + 260]
                    for c in range(CCH):
                        nc.tensor.matmul(
                            ps, xT[c][:, tt * 128:(tt + 1) * 128],
                            wva_sb[c][:, cs], start=(c == 0),
                            stop=(c == CCH - 1))
                    nc.vector.tensor_add(v16[tt][:, cs], ps, bva_bc[:, cs])
                if USE_DR:
                    # fp8 copy of the finished v16 tile (SBUF->SBUF)
                    o = (tt % 2) * 528
                    nc.vector.tensor_copy(v8[tt // 2][:, o:o + VW],
                                          v16[tt][:, 0:VW])
            units.append(unit)
        return units

    # ---- per head pair: qkv -> attention -> partial proj ----
    # wqk8 row-block cp, col-block pair: [q_even|q_odd|k_even|k_odd] fp8,
    # weights pre-scaled x512 on host (e4m3 subnormal avoidance).
    wqk8 = aps["wqk8"]

    def prep_qkv(pair):
        psl = slice(pair * 128, (pair + 1) * 128)
        wqk_c = []
        for cp in range(CCH // 2):
            wt = po_wqk.tile([128, 512], F8, tag="wqk", name="wt")
            nc.sync.dma_start(
                out=wt[:],
                in_=wqk8[cp * 128:(cp + 1) * 128,
                         pair * 512:(pair + 1) * 512])
            wqk_c.append(wt)
        bq_sb = po_bias.tile([128, 1], F32, tag=f"bq{pair}", name=f"bq{pair}")
        nc.sync.dma_start(out=bq_sb[:], in_=bq[psl, :])
        bk_sb = po_bias.tile([128, 1], F32, tag=f"bk{pair}", name=f"bk{pair}")
        nc.sync.dma_start(out=bk_sb[:], in_=bk[psl, :])
        qT = po_qkt.tile([128, T], F16, tag="qT", name="qT")
        kT = po_qkt.tile([128, T], F16, tag="kT", name="kT")
        return dict(wqk=wqk_c, bq=bq_sb, bk=bk_sb, qT=qT, kT=kT)

    def qkv_units(st8):
        units = []
        for tb in range(NTB):
            def unit(tb=tb):
                ncp = CCH // 2
                ps2 = pp_main.tile([128, 1024], F32, tag="main", name="psqk")
                psq, psk = ps2[:, 0:512], ps2[:, 512:1024]
                for qk in range(2):
                    dst = psq if qk == 0 else psk
                    for cp in range(ncp):
                        wt = st8["wqk"][cp]
                        lhsT = bass.AP(
                            tensor=wt.tensor,
                            offset=wt[:].offset + qk * 256,
                            ap=[list(wt[:].ap[0]), [128, 2], [1, 128]])
                        x8 = xT8[cp]
                        rhs = bass.AP(
                            tensor=x8.tensor,
                            offset=x8[:].offset + tb * 512,
                            ap=[list(x8[:].ap[0]), [T, 2], [1, 512]])
                        nc.tensor.matmul(dst, lhsT, rhs, start=(cp == 0),
                                         stop=(cp == ncp - 1), perf_mode=DR,
                                         skip_group_check=True)
                tsl = slice(tb * 512, (tb + 1) * 512)
                # psum*(1/(sqrt(D)*512)) + bq/sqrt(D)  (bq pre-scaled)
                nc.vector.tensor_scalar(
                    out=st8["qT"][:, tsl], in0=psq, scalar1=0.125 / 512.0,
                    scalar2=st8["bq"][:], op0=MULT, op1=ADD)
                nc.vector.tensor_scalar(
                    out=st8["kT"][:, tsl], in0=psk, scalar1=1.0 / 512.0,
                    scalar2=st8["bk"][:], op0=MULT, op1=ADD)
            units.append(unit)
        return units

    def attn_units(st8, yt):
        qT, kT = st8["qT"], st8["kT"]
        units = []
        for qb in range(NQB):
            nkt = 4 * qb + 4
            kpairs = [(2 * i, 2 * i + 1) for i in range(nkt // 2)]
            L = len(kpairs)
            state = {"st": {}, "et": {}, "pv": None}

            def offs_of(kp, qb=qb):
                return tuple(max(0, 128 * (kt - 4 * qb)) for kt in kp)

            def emit_scores(i, qb=qb, kpairs=kpairs, state=state,
                            offs_of=offs_of):
                kp = kpairs[i]
                offs = offs_of(kp)
                sts = []
                for s in range(2):
                    sts.append(pp_main.tile([128, 1024], F32, tag="main",
                                            name=f"st{s}"))
                for j in range(2):   # k-tile within pair, outer for adjacency
                    for s in range(2):  # head A/B adjacent -> concurrent
                        rq = slice(s * 64, s * 64 + 64)
                        kt, off = kp[j], offs[j]
                        nc.tensor.matmul(
                            sts[s][:, j * 512 + off:(j + 1) * 512],
                            kT[rq, kt * 128:(kt + 1) * 128],
                            qT[rq, qb * 512 + off:(qb + 1) * 512],
                            start=True, stop=True)
                for s in range(2):
                    for j in range(2):
                        kt, off = kp[j], offs[j]
                        if kt >= 4 * qb:  # diagonal tile: triangular mask
                            nc.vector.tensor_add(
                                sts[s][:, j * 512 + off:j * 512 + off + 128],
                                sts[s][:, j * 512 + off:j * 512 + off + 128],
                                mask_sb[:])
                state["st"][i] = sts

            def emit_exp(i, qb=qb, kpairs=kpairs, state=state,
                         offs_of=offs_of):
                kp = kpairs[i]
                offs = offs_of(kp)
                lo = offs[0]  # 0 or 256; single op over [lo:1024]
                full = USE_DR and qb >= 1
                sts = state["st"].pop(i)
                ets = []
                for s in range(2):
                    et = po_et.tile([128, 1024], F8 if full else F16,
                                    tag="expT8" if full else "expT",
                                    name="et")
                    nc.scalar.activation(
                        et[:, lo:1024], sts[s][:, lo:1024],
                        mybir.ActivationFunctionType.Exp)
                    if full and kp[0] >= 4 * qb:
                        # DoubleRow reads full strips: zero the regions the
                        # narrowed score matmuls never computed.
                        if lo > 0:
                            nc.vector.memset(et[:, 0:lo], 0.0)
                        if offs[1] > 0:
                            nc.vector.memset(et[:, 512:512 + offs[1]], 0.0)
                    ets.append(et)
                state["et"][i] = ets

            def emit_pv(i, qb=qb, kpairs=kpairs, nkt=nkt, state=state,
                        offs_of=offs_of):
                kp = kpairs[i]
                offs = offs_of(kp)
                full = USE_DR and qb >= 1
                ets = state["et"].pop(i)
                pv = state["pv"]
                for s in range(2):
                    pair_h = st8["pair"] * 2 + s
                    if full:
                        # fp8 DoubleRow: both k-tiles in one matmul
                        v8t = v8[kp[0] // 2]
                        lhsT = bass.AP(
                            tensor=v8t.tensor,
                            offset=v8t[:].offset + pair_h * 65,
                            ap=[list(v8t[:].ap[0]), [528, 2], [1, 65]])
                        et = ets[s]
                        rhs = bass.AP(
                            tensor=et.tensor, offset=et[:].offset,
                            ap=[list(et[:].ap[0]), [512, 2], [1, 512]])
                        nc.tensor.matmul(
                            pv[s][0:65, 0:512], lhsT, rhs,
                            start=(kp[0] == 0), stop=(kp[1] == nkt - 1),
                            perf_mode=DR, skip_group_check=True)
                        continue
                    vsl = slice(pair_h * 65, pair_h * 65 + 65)
                    for j in range(2):
                        kt, off = kp[j], offs[j]
                        nc.tensor.matmul(
                            pv[s][0:65, off:512], v16[kt][:, vsl],
                            ets[s][:, j * 512 + off:(j + 1) * 512],
                            start=(kt == 0), stop=(kt == nkt - 1),
                            skip_group_check=True)

            def finish(qb=qb, state=state):
                pv = state["pv"]
                qsl = slice(qb * 512, (qb + 1) * 512)
                for s in range(2):
                    # evacuate pv early so the bank frees for the next qb
                    ysb = po_ysb.tile([128, 512], F32, tag="ysb",
                                      name="ysb")
                    nc.vector.tensor_copy(ysb[0:65, :], pv[s][0:65, :])
                    dscr = po_dram.tile([1, 512], F32, tag="dscr",
                                        name="dscr")
                    (nc.gpsimd if USE_GPS_DMA else nc.sync).dma_start(out=dscr[:], in_=ysb[64:65, :])
                    rec = po_rec.tile([128, 512], F32, tag="recip",
                                      name="rec")
                    (nc.gpsimd if USE_GPS_DMA else nc.sync).dma_start(out=rec[0:64, :], in_=bass.AP(
                        tensor=dscr.tensor, offset=dscr[:].offset,
                        ap=[[0, 64]] + [list(a) for a in dscr[:].ap[1:]]))
                    nc.vector.reciprocal_approx_fast(rec[0:64, :],
                                                     rec[0:64, :])
                    if s == 0:
                        nc.vector.tensor_mul(yt[0:64, qsl], ysb[0:64, :],
                                             rec[0:64, :])
                    else:
                        # engines can't cross partitions; bounce via DMA
                        ytmp = po_ytmp.tile([128, 512], F16, tag="ytmp",
                                            name="ytmp")
                        nc.vector.tensor_mul(ytmp[0:64, :], ysb[0:64, :],
                                             rec[0:64, :])
                        (nc.gpsimd if USE_GPS_DMA else nc.sync).dma_start(out=yt[64:128, qsl],
                                            in_=ytmp[0:64, :])

            def unit(i, L=L, state=state, emit_exp=emit_exp,
                     emit_pv=emit_pv, emit_scores=emit_scores,
                     finish=finish):
                # pipeline: exp(i-1) first, then pv(i-2), then scores(i)
                if i == 0:
                    state["pv"] = [
                        pp_pv.tile([128, 512], F32, tag="pv", name=f"pv{s}")
                        for s in range(2)]
                if 1 <= i <= L:
                    emit_exp(i - 1)
                if 2 <= i <= L + 1:
                    emit_pv(i - 2)
                if i < L:
                    emit_scores(i)
                if i == L + 1:
                    finish()

            for i in range(L + 2):
                units.append(lambda i=i, u=unit: u(i))
        return units

    def prep_proj(couple):
        wp_sb = []
        for pq in range(2):
            for cb in range(2):
                prow = (couple * 2 + pq) * 128
                wt = po_wp.tile([128, 512], F16, tag="wp", name="wpt")
                nc.sync.dma_start(
                    out=wt[:],
                    in_=wp[prow:prow + 128, cb * 512:(cb + 1) * 512])
                wp_sb.append(wt)
        return wp_sb

    def proj_units(couple, wp_sb, yts):
        out_p = out_ab[couple]
        units = []
        for tt0 in range(0, NTT, 2):
            def unit(tt0=tt0):
                for tt in (tt0, tt0 + 1):
                    ot = po_ot.tile([128, C], F16, tag="ot", name="ot")
                    ps2 = pp_main.tile([128, 1024], F32, tag="main",
                                       name="pproj")
                    # pq outer: stationary (y tile) reused across cb
                    for pq in range(2):
                        for cb in range(2):
                            nc.tensor.matmul(
                                ps2[:, cb * 512:(cb + 1) * 512],
                                yts[pq][:, tt * 128:(tt + 1) * 128],
                                wp_sb[pq * 2 + cb][:],
                                start=(pq == 0), stop=(pq == 1))
                    for cb in range(2):
                        nc.vector.tensor_copy(
                            ot[:, cb * 512:(cb + 1) * 512],
                            ps2[:, cb * 512:(cb + 1) * 512])
                    nc.sync.dma_start(
                        out=out_p[tt * 128:(tt + 1) * 128, :], in_=ot[:])
            units.append(unit)
        return units

    def round_robin(*streams):
        streams = [list(s) for s in streams if s]
        while any(streams):
            for s in streams:
                if s:
                    s.pop(0)()

    for u in v_units():
        u()
    pair_state = []
    yts = []
    st0 = prep_qkv(0)
    st0["pair"] = 0
    pair_state.append(st0)
    for u in qkv_units(st0):
        u()
    for p in range(NPAIR):
        yt = po_yt.tile([128, T], F16, tag="yT", name="yt")
        yts.append(yt)
        streams = []
        if p + 1 < NPAIR:
            stn = prep_qkv(p + 1)
            stn["pair"] = p + 1
            pair_state.append(stn)
            streams.append(qkv_units(stn))
        if p == 2:
            wp_sb = prep_proj(0)
            streams.append(proj_units(0, wp_sb, yts[0:2]))
        streams.append(attn_units(pair_state[p], yt))
        round_robin(*streams)
    wp_sb = prep_proj(1)
    for u in proj_units(1, wp_sb, yts[2:4]):
        u()

    ctx.close()


_CACHE = {}


def _build():
    if "nc" in _CACHE:
        return _CACHE["nc"]
    nc = bacc.Bacc("TRN2", target_bir_lowering=False, debug=False,
                   enable_asserts=True, num_devices=N_CORES)
    aps = {
        "xt": nc.dram_tensor("xt", [C, T], F16, kind="ExternalInput").ap(),
        "xt8": nc.dram_tensor("xt8", [C // 2, 2 * T], F8,
                              kind="ExternalInput").ap(),
        "wqk8": nc.dram_tensor("wqk8", [C // 2, 4 * F], F8,
                               kind="ExternalInput").ap(),
        "wva": nc.dram_tensor("wva", [C, VW], F16, kind="ExternalInput").ap(),
        "bq": nc.dram_tensor("bq", [F, 1], F32, kind="ExternalInput").ap(),
        "bk": nc.dram_tensor("bk", [F, 1], F32, kind="ExternalInput").ap(),
        "bva2": nc.dram_tensor("bva2", [1, VW], F32,
                               kind="ExternalInput").ap(),
        "wp": nc.dram_tensor("wp", [F, C], F16, kind="ExternalInput").ap(),
        "cmask": nc.dram_tensor("cmask", [128, 128], F32,
                                kind="ExternalInput").ap(),
        "out_pa": nc.dram_tensor("out_pa", [T, C], F16,
                                 kind="ExternalOutput").ap(),
        "out_pb": nc.dram_tensor("out_pb", [T, C], F16,
                                 kind="ExternalOutput").ap(),
    }
    with tile.TileContext(nc) as tc:
        _emit(tc, aps)
    nc.compile()
    _CACHE["nc"] = nc
    return nc


def _make_in_maps(x, Wqkv, bqkv, Wproj):
    x = np.asarray(x, dtype=np.float32)
    Wqkv = np.asarray(Wqkv, dtype=np.float32)
    bqkv = np.asarray(bqkv, dtype=np.float32)
    Wproj = np.asarray(Wproj, dtype=np.float32)

    # triangular causal mask: M[p, f] = 0 if f >= p else -1e9
    p_idx = np.arange(128)[:, None]
    u_idx = np.arange(128)[None, :]
    cmask = np.where(u_idx >= p_idx, 0.0, -1e9).astype(np.float32)

    in_maps = []
    for core in range(N_CORES):
        b, g = divmod(core, 2)
        q0, k0, v0 = 512 * g, C + 512 * g, 2 * C + 512 * g
        wva = np.zeros((C, VW), dtype=np.float32)
        bva = np.zeros((1, VW), dtype=np.float32)
        for h in range(NH):
            src = v0 + D * h
            dst = 65 * h
            # per-head layout [v(64), one]
            wva[:, dst:dst + 64] = Wqkv[:, src:src + 64]
            bva[0, dst:dst + 64] = bqkv[src:src + 64]
            bva[0, dst + 64] = 1.0
        xTf8 = np.ascontiguousarray(x[b].T).astype(ml_dtypes.float8_e4m3fn)
        xr = xTf8.reshape(C // 256, 2, 128, T)
        xt8 = np.concatenate([xr[:, 0], xr[:, 1]], axis=2).reshape(
            C // 2, 2 * T)
        wq_s = (512.0 * Wqkv[:, q0:q0 + F]).astype(ml_dtypes.float8_e4m3fn)
        wk_s = (512.0 * Wqkv[:, k0:k0 + F]).astype(ml_dtypes.float8_e4m3fn)
        wqr = wq_s.reshape(C // 256, 2, 128, F)
        wkr = wk_s.reshape(C // 256, 2, 128, F)
        wqk8 = np.zeros((C // 2, 4 * F), dtype=ml_dtypes.float8_e4m3fn)
        for cp in range(C // 256):
            for pair in range(NPAIR):
                csl = slice(pair * 128, (pair + 1) * 128)
                blk = np.concatenate(
                    [wqr[cp, 0][:, csl], wqr[cp, 1][:, csl],
                     wkr[cp, 0][:, csl], wkr[cp, 1][:, csl]], axis=1)
                wqk8[cp * 128:(cp + 1) * 128,
                     pair * 512:(pair + 1) * 512] = blk
        in_maps.append({
            "xt": np.ascontiguousarray(x[b].T).astype(np.float16),
            "xt8": xt8,
            "wqk8": wqk8,
            "wva": np.ascontiguousarray(wva).astype(np.float16),
            "bq": np.ascontiguousarray(bqkv[q0:q0 + F].reshape(F, 1) * 0.125),
            "bk": np.ascontiguousarray(bqkv[k0:k0 + F].reshape(F, 1)),
            "bva2": np.ascontiguousarray(bva),
            "wp": np.ascontiguousarray(
                Wproj[512 * g:512 * g + F, :]).astype(np.float16),
            "cmask": cmask,
        })
    return in_maps


def run_sharded(x, Wqkv, bqkv, Wproj, bproj, trace=False):
    nc = _build()
    in_maps = _make_in_maps(x, Wqkv, bqkv, Wproj)
    res = run_bass_kernel_spmd(nc, in_maps, core_ids=list(range(N_CORES)),
                               trace=trace)
    bproj = np.asarray(bproj, dtype=np.float32)
    out = np.empty((B, T, C), dtype=np.float32)
    for b in range(B):
        acc = bproj[None, :].astype(np.float32).repeat(T, axis=0)
        for core in (2 * b, 2 * b + 1):
            acc = acc + res.results[core]["out_pa"].astype(np.float32) \
                + res.results[core]["out_pb"].astype(np.float32)
        out[b] = acc
    return out, res


def kernel(x, Wqkv, bqkv, Wproj, bproj):
    out, _ = run_sharded(x, Wqkv, bqkv, Wproj, bproj, trace=False)
    return out


# revision 25
# speedup vs baseline: 1.3250x; 1.0296x over previous
"""Causal self-attention (B=4, T=2048, C=1024, H=16, Dh=64) on 8 trn2 NeuronCores.

Sharding: core i <-> (batch b = i//2, head-group g = i%2). Each core computes
8 heads of one batch end-to-end (qkv slice, causal attention, partial output
projection); the host sums the head-group partials per batch and adds bproj.
No device collectives.

v2 layout: x arrives pre-transposed from the host ([C, T]) so no PE
transposes are needed. Scores use the transposed layout sT[tk, tq]; the two
heads of a pair occupy PE row-groups 0-63 / 64-127 and their score matmuls
are emitted adjacently so the 16x 32x32 sub-arrays run them concurrently.
Softmax denominators come from an extra ones column interleaved into v
(M=65 PV matmuls) and are broadcast across partitions with a DRAM-bounce
DMA. Causal masking only touches the 128 diagonal columns of each k-tile.
"""

import ml_dtypes
import numpy as np

import concourse.bass as bass
import concourse.tile as tile
from concourse import bacc, mybir
from concourse.bass_utils import run_bass_kernel_spmd

F32 = mybir.dt.float32
F16 = mybir.dt.float16
F8 = mybir.dt.float8e4
DR = mybir.MatmulPerfMode.DoubleRow
ADD = mybir.AluOpType.add
MULT = mybir.AluOpType.mult
USE_DR = True        # fp8 DoubleRow PV for off-diagonal k-pairs
USE_GPS_DMA = False  # issue bounce DMAs from the GpSimd queue

N_CORES = 8
B, T, C = 4, 2048, 1024
NH_TOT, D = 16, 64
F = 512            # features per core (8 heads)
NH = 8             # local heads
NPAIR = 4          # head pairs (128 feats each)
CCH = C // 128     # 8 contraction chunks
NTT = T // 128     # 16 t tiles
NTB = T // 512     # 4 t blocks (qkv production)
NQB = T // 512     # 4 q blocks (attention)
VW = NH * (D + 1)  # 520: augmented v width (per-head [v(64), one])


def _emit(tc, aps):
    from contextlib import ExitStack
    nc = tc.nc
    xt, wva, bq, bk, wp = (
        aps["xt"], aps["wva"], aps["bq"], aps["bk"], aps["wp"])
    cmask = aps["cmask"]
    out_ab = [aps["out_pa"], aps["out_pb"]]

    ctx = ExitStack()
    # PSUM: pp_main 3x[128,1024] = 6 banks, pp_pv 2x[128,512] = 2 banks
    pp_main = ctx.enter_context(tc.tile_pool(name="ps_main", bufs=3,
                                             space="PSUM"))
    pp_pv = ctx.enter_context(tc.tile_pool(name="ps_pv", bufs=2, space="PSUM"))
    po_xt = ctx.enter_context(tc.tile_pool(name="xt", bufs=1))
    po_xt8 = ctx.enter_context(tc.tile_pool(name="xt8", bufs=1))
    po_v = ctx.enter_context(tc.tile_pool(name="v_all", bufs=1))
    po_v8 = ctx.enter_context(tc.tile_pool(name="v8", bufs=1))
    po_ysb = ctx.enter_context(tc.tile_pool(name="ysb", bufs=3))
    po_mask = ctx.enter_context(tc.tile_pool(name="mask", bufs=1))
    po_wva = ctx.enter_context(tc.tile_pool(name="wva", bufs=8))
    po_qkt = ctx.enter_context(tc.tile_pool(name="qkT", bufs=4))
    po_bias = ctx.enter_context(tc.tile_pool(name="bias", bufs=1))
    po_wqk = ctx.enter_context(tc.tile_pool(name="wqk", bufs=8))
    po_yt = ctx.enter_context(tc.tile_pool(name="yT", bufs=4))
    po_et = ctx.enter_context(tc.tile_pool(name="expT", bufs=6))
    po_rec = ctx.enter_context(tc.tile_pool(name="recip", bufs=3))
    po_den = ctx.enter_context(tc.tile_pool(name="den", bufs=2))
    po_ytmp = ctx.enter_context(tc.tile_pool(name="ytmp", bufs=2))
    po_ot = ctx.enter_context(tc.tile_pool(name="ot", bufs=3))
    po_wp = ctx.enter_context(tc.tile_pool(name="wp", bufs=4))
    po_dram = ctx.enter_context(tc.tile_pool(name="dram_scr", bufs=4,
                                             space="DRAM"))

    mask_sb = po_mask.tile([128, 128], F32, tag="mask")
    nc.sync.dma_start(out=mask_sb[:], in_=cmask[:])
    # bva broadcast to all 128 partitions straight from DRAM ([1,VW] src)
    bva_bc = po_bias.tile([128, VW], F32, tag="bva_bc")
    bva2 = aps["bva2"]
    nc.sync.dma_start(out=bva_bc[:], in_=bass.AP(
        tensor=bva2.tensor, offset=bva2.offset,
        ap=[[0, 128]] + [list(a) for a in bva2.ap[1:]]))

    # ---- phase 0: xT straight from DRAM (host pre-transposed).
    # Startup DMAs rotate across idle engine queues for parallel dispatch.
    dqs = [nc.sync, nc.gpsimd]
    xT = [po_xt.tile([128, T], F16, tag=f"xT{c}", name=f"xT{c}")
          for c in range(CCH)]
    for ch in range(2):
        csl = slice(ch * (T // 2), (ch + 1) * (T // 2))
        for c in range(CCH):
            dqs[c % 2].dma_start(out=xT[c][:, csl],
                                 in_=xt[c * 128:(c + 1) * 128, csl])
    xt8 = aps["xt8"]
    xT8 = [po_xt8.tile([128, 2 * T], F8, tag=f"xT8_{cp}", name=f"xT8_{cp}")
           for cp in range(CCH // 2)]
    for cp in range(CCH // 2):
        dqs[(cp + 2) % 2].dma_start(
            out=xT8[cp][:], in_=xt8[cp * 128:(cp + 1) * 128, :])

    # ---- phase 0b: v tiles [128, 520] = 8 heads x [v(64) | one],
    # wva host-interleaved with ones columns.  v8: fp8 copy, k-tile pairs
    # packed for DoubleRow ([0:520] = even kt, [528:1048] = odd kt). ----
    v16 = [po_v.tile([128, VW], F16, tag=f"v{tt}", name=f"v{tt}")
           for tt in range(NTT)]
    v8 = [po_v8.tile([128, 1056], F8, tag=f"v8_{kp}", name=f"v8_{kp}")
          for kp in range(NTT // 2)]
    wva_sb = []
    for c in range(CCH):
        wt = po_wva.tile([128, VW], F16, tag="wva")
        dqs[(c + 1) % 2].dma_start(out=wt[:],
                                   in_=wva[c * 128:(c + 1) * 128, :])
        wva_sb.append(wt)

    def v_units():
        units = []
        for tt in range(NTT):
            def unit(tt=tt):
                ps2 = pp_main.tile([128, 1024], F32, tag="main", name="psv")
                for half in range(2):
                    cs = slice(half * 260, half * 260 + 260)
                    ps = ps2[:, half * 512:half * 512 + 260]
                    for c in range(CCH):
                        nc.tensor.matmul(
                            ps, xT[c][:, tt * 128:(tt + 1) * 128],
                            wva_sb[c][:, cs], start=(c == 0),
                            stop=(c == CCH - 1))
                    nc.vector.tensor_add(v16[tt][:, cs], ps, bva_bc[:, cs])
                if USE_DR:
                    # fp8 copy of the finished v16 tile (SBUF->SBUF)
                    o = (tt % 2) * 528
                    nc.vector.tensor_copy(v8[tt // 2][:, o:o + VW],
                                          v16[tt][:, 0:VW])
            units.append(unit)
        return units

    # ---- per head pair: qkv -> attention -> partial proj ----
    # wqk8 row-block cp, col-block pair: [q_even|q_odd|k_even|k_odd] fp8,
    # weights pre-scaled x512 on host (e4m3 subnormal avoidance).
    wqk8 = aps["wqk8"]

    def prep_qkv(pair):
        psl = slice(pair * 128, (pair + 1) * 128)
        wqk_c = []
        for cp in range(CCH // 2):
            wt = po_wqk.tile([128, 512], F8, tag="wqk", name="wt")
            nc.sync.dma_start(
                out=wt[:],
                in_=wqk8[cp * 128:(cp + 1) * 128,
                         pair * 512:(pair + 1) * 512])
            wqk_c.append(wt)
        bq_sb = po_bias.tile([128, 1], F32, tag=f"bq{pair}", name=f"bq{pair}")
        nc.sync.dma_start(out=bq_sb[:], in_=bq[psl, :])
        bk_sb = po_bias.tile([128, 1], F32, tag=f"bk{pair}", name=f"bk{pair}")
        nc.sync.dma_start(out=bk_sb[:], in_=bk[psl, :])
        qT = po_qkt.tile([128, T], F16, tag="qT", name="qT")
        kT = po_qkt.tile([128, T], F16, tag="kT", name="kT")
        return dict(wqk=wqk_c, bq=bq_sb, bk=bk_sb, qT=qT, kT=kT)

    def qkv_units(st8):
        units = []
        for tb in range(NTB):
            def unit(tb=tb):
                ncp = CCH // 2
                ps2 = pp_main.tile([128, 1024], F32, tag="main", name="psqk")
                psq, psk = ps2[:, 0:512], ps2[:, 512:1024]
                for qk in range(2):
                    dst = psq if qk == 0 else psk
                    for cp in range(ncp):
                        wt = st8["wqk"][cp]
                        lhsT = bass.AP(
                            tensor=wt.tensor,
                            offset=wt[:].offset + qk * 256,
                            ap=[list(wt[:].ap[0]), [128, 2], [1, 128]])
                        x8 = xT8[cp]
                        rhs = bass.AP(
                            tensor=x8.tensor,
                            offset=x8[:].offset + tb * 512,
                            ap=[list(x8[:].ap[0]), [T, 2], [1, 512]])
                        nc.tensor.matmul(dst, lhsT, rhs, start=(cp == 0),
                                         stop=(cp == ncp - 1), perf_mode=DR,
                                         skip_group_check=True)
                tsl = slice(tb * 512, (tb + 1) * 512)
                # psum*(1/(sqrt(D)*512)) + bq/sqrt(D)  (bq pre-scaled)
                nc.vector.tensor_scalar(
                    out=st8["qT"][:, tsl], in0=psq, scalar1=0.125 / 512.0,
                    scalar2=st8["bq"][:], op0=MULT, op1=ADD)
                nc.vector.tensor_scalar(
                    out=st8["kT"][:, tsl], in0=psk, scalar1=1.0 / 512.0,
                    scalar2=st8["bk"][:], op0=MULT, op1=ADD)
            units.append(unit)
        return units

    def attn_units(st8, yt):
        qT, kT = st8["qT"], st8["kT"]
        units = []
        for qb in range(NQB):
            nkt = 4 * qb + 4
            kpairs = [(2 * i, 2 * i + 1) for i in range(nkt // 2)]
            L = len(kpairs)
            state = {"st": {}, "et": {}, "pv": None}

            def offs_of(kp, qb=qb):
                return tuple(max(0, 128 * (kt - 4 * qb)) for kt in kp)

            def emit_scores(i, qb=qb, kpairs=kpairs, state=state,
                            offs_of=offs_of):
                kp = kpairs[i]
                offs = offs_of(kp)
                sts = []
                for s in range(2):
                    sts.append(pp_main.tile([128, 1024], F32, tag="main",
                                            name=f"st{s}"))
                for j in range(2):   # k-tile within pair, outer for adjacency
                    for s in range(2):  # head A/B adjacent -> concurrent
                        rq = slice(s * 64, s * 64 + 64)
                        kt, off = kp[j], offs[j]
                        nc.tensor.matmul(
                            sts[s][:, j * 512 + off:(j + 1) * 512],
                            kT[rq, kt * 128:(kt + 1) * 128],
                            qT[rq, qb * 512 + off:(qb + 1) * 512],
                            start=True, stop=True)
                for s in range(2):
                    for j in range(2):
                        kt, off = kp[j], offs[j]
                        if kt >= 4 * qb:  # diagonal tile: triangular mask
                            nc.vector.tensor_add(
                                sts[s][:, j * 512 + off:j * 512 + off + 128],
                                sts[s][:, j * 512 + off:j * 512 + off + 128],
                                mask_sb[:])
                state["st"][i] = sts

            def emit_exp(i, qb=qb, kpairs=kpairs, state=state,
                         offs_of=offs_of):
                kp = kpairs[i]
                offs = offs_of(kp)
                lo = offs[0]  # 0 or 256; single op over [lo:1024]
                full = USE_DR and qb >= 1
                sts = state["st"].pop(i)
                ets = []
                for s in range(2):
                    et = po_et.tile([128, 1024], F8 if full else F16,
                                    tag="expT8" if full else "expT",
                                    name="et")
                    nc.scalar.activation(
                        et[:, lo:1024], sts[s][:, lo:1024],
                        mybir.ActivationFunctionType.Exp)
                    if full and kp[0] >= 4 * qb:
                        # DoubleRow reads full strips: zero the regions the
                        # narrowed score matmuls never computed.
                        if lo > 0:
                            nc.vector.memset(et[:, 0:lo], 0.0)
                        if offs[1] > 0:
                            nc.vector.memset(et[:, 512:512 + offs[1]], 0.0)
                    ets.append(et)
                state["et"][i] = ets

            def emit_pv(i, qb=qb, kpairs=kpairs, nkt=nkt, state=state,
                        offs_of=offs_of):
                kp = kpairs[i]
                offs = offs_of(kp)
                full = USE_DR and qb >= 1
                ets = state["et"].pop(i)
                pv = state["pv"]
                for s in range(2):
                    pair_h = st8["pair"] * 2 + s
                    if full:
                        # fp8 DoubleRow: both k-tiles in one matmul
                        v8t = v8[kp[0] // 2]
                        lhsT = bass.AP(
                            tensor=v8t.tensor,
                            offset=v8t[:].offset + pair_h * 65,
                            ap=[list(v8t[:].ap[0]), [528, 2], [1, 65]])
                        et = ets[s]
                        rhs = bass.AP(
                            tensor=et.tensor, offset=et[:].offset,
                            ap=[list(et[:].ap[0]), [512, 2], [1, 512]])
                        nc.tensor.matmul(
                            pv[s][0:65, 0:512], lhsT, rhs,
                            start=(kp[0] == 0), stop=(kp[1] == nkt - 1),
                            perf_mode=DR, skip_group_check=True)
                        continue
                    vsl = slice(pair_h * 65, pair_h * 65 + 65)
                    for j in range(2):
                        kt, off = kp[j], offs[j]
                        nc.tensor.matmul(
                            pv[s][0:65, off:512], v16[kt][:, vsl],
                            ets[s][:, j * 512 + off:(j + 1) * 512],
                            start=(kt == 0), stop=(kt == nkt - 1),
                            skip_group_check=True)

            def finish(qb=qb, state=state):
                pv = state["pv"]
                qsl = slice(qb * 512, (qb + 1) * 512)
                for s in range(2):
                    # evacuate pv early so the bank frees for the next qb
                    ysb = po_ysb.tile([128, 512], F32, tag="ysb",
                                      name="ysb")
                    nc.vector.tensor_copy(ysb[0:65, :], pv[s][0:65, :])
                    dscr = po_dram.tile([1, 512], F32, tag="dscr",
                                        name="dscr")
                    (nc.gpsimd if USE_GPS_DMA else nc.sync).dma_start(out=dscr[:], in_=ysb[64:65, :])
                    rec = po_rec.tile([128, 512], F32, tag="recip",
                                      name="rec")
                    (nc.gpsimd if USE_GPS_DMA else nc.sync).dma_start(out=rec[0:64, :], in_=bass.AP(
                        tensor=dscr.tensor, offset=dscr[:].offset,
                        ap=[[0, 64]] + [list(a) for a in dscr[:].ap[1:]]))
                    nc.vector.reciprocal_approx_fast(rec[0:64, :],
                                                     rec[0:64, :])
                    if s == 0:
                        nc.vector.tensor_mul(yt[0:64, qsl], ysb[0:64, :],
                                             rec[0:64, :])
                    else:
                        # engines can't cross partitions; bounce via DMA
                        ytmp = po_ytmp.tile([128, 512], F16, tag="ytmp",
                                            name="ytmp")
                        nc.vector.tensor_mul(ytmp[0:64, :], ysb[0:64, :],
                                             rec[0:64, :])
                        (nc.gpsimd if USE_GPS_DMA else nc.sync).dma_start(out=yt[64:128, qsl],
                                            in_=ytmp[0:64, :])

            def unit(i, L=L, state=state, emit_exp=emit_exp,
                     emit_pv=emit_pv, emit_scores=emit_scores,
                     finish=finish):
                # pipeline: exp(i-1) first, then pv(i-2), then scores(i)
                if i == 0:
                    state["pv"] = [
                        pp_pv.tile([128, 512], F32, tag="pv", name=f"pv{s}")
                        for s in range(2)]
                if 1 <= i <= L:
                    emit_exp(i - 1)
                if 2 <= i <= L + 1:
                    emit_pv(i - 2)
                if i < L:
                    emit_scores(i)
                if i == L + 1:
                    finish()

            for i in range(L + 2):
                units.append(lambda i=i, u=unit: u(i))
        return units

    def prep_proj(couple):
        wp_sb = []
        for pq in range(2):
            for cb in range(2):
                prow = (couple * 2 + pq) * 128
                wt = po_wp.tile([128, 512], F16, tag="wp", name="wpt")
                nc.sync.dma_start(
                    out=wt[:],
                    in_=wp[prow:prow + 128, cb * 512:(cb + 1) * 512])
                wp_sb.append(wt)
        return wp_sb

    def proj_units(couple, wp_sb, yts):
        out_p = out_ab[couple]
        units = []
        for tt0 in range(0, NTT, 2):
            def unit(tt0=tt0):
                for tt in (tt0, tt0 + 1):
                    ot = po_ot.tile([128, C], F16, tag="ot", name="ot")
                    ps2 = pp_main.tile([128, 1024], F32, tag="main",
                                       name="pproj")
                    # pq outer: stationary (y tile) reused across cb
                    for pq in range(2):
                        for cb in range(2):
                            nc.tensor.matmul(
                                ps2[:, cb * 512:(cb + 1) * 512],
                                yts[pq][:, tt * 128:(tt + 1) * 128],
                                wp_sb[pq * 2 + cb][:],
                                start=(pq == 0), stop=(pq == 1))
                    for cb in range(2):
                        nc.vector.tensor_copy(
                            ot[:, cb * 512:(cb + 1) * 512],
                            ps2[:, cb * 512:(cb + 1) * 512])
                    nc.sync.dma_start(
                        out=out_p[tt * 128:(tt + 1) * 128, :], in_=ot[:])
            units.append(unit)
        return units

    def round_robin(*streams):
        # proportional interleave: each pop advances the stream whose
        # remaining fraction is largest, spreading short fill streams
        # evenly across the long attention stream
        streams = [list(s) for s in streams if s]
        totals = [len(s) for s in streams]
        while any(streams):
            frac = [(len(s) / t, i) for i, (s, t) in
                    enumerate(zip(streams, totals)) if s]
            _, i = max(frac)
            streams[i].pop(0)()

    for u in v_units():
        u()
    pair_state = []
    yts = []
    st0 = prep_qkv(0)
    st0["pair"] = 0
    pair_state.append(st0)
    for u in qkv_units(st0):
        u()
    for p in range(NPAIR):
        yt = po_yt.tile([128, T], F16, tag="yT", name="yt")
        yts.append(yt)
        streams = []
        if p + 1 < NPAIR:
            stn = prep_qkv(p + 1)
            stn["pair"] = p + 1
            pair_state.append(stn)
            streams.append(qkv_units(stn))
        if p == 2:
            wp_sb = prep_proj(0)
            streams.append(proj_units(0, wp_sb, yts[0:2]))
        streams.append(attn_units(pair_state[p], yt))
        round_robin(*streams)
    wp_sb = prep_proj(1)
    for u in proj_units(1, wp_sb, yts[2:4]):
        u()

    ctx.close()


_CACHE = {}


def _build():
    if "nc" in _CACHE:
        return _CACHE["nc"]
    nc = bacc.Bacc("TRN2", target_bir_lowering=False, debug=False,
                   enable_asserts=True, num_devices=N_CORES)
    aps = {
        "xt": nc.dram_tensor("xt", [C, T], F16, kind="ExternalInput").ap(),
        "xt8": nc.dram_tensor("xt8", [C // 2, 2 * T], F8,
                              kind="ExternalInput").ap(),
        "wqk8": nc.dram_tensor("wqk8", [C // 2, 4 * F], F8,
                               kind="ExternalInput").ap(),
        "wva": nc.dram_tensor("wva", [C, VW], F16, kind="ExternalInput").ap(),
        "bq": nc.dram_tensor("bq", [F, 1], F32, kind="ExternalInput").ap(),
        "bk": nc.dram_tensor("bk", [F, 1], F32, kind="ExternalInput").ap(),
        "bva2": nc.dram_tensor("bva2", [1, VW], F32,
                               kind="ExternalInput").ap(),
        "wp": nc.dram_tensor("wp", [F, C], F16, kind="ExternalInput").ap(),
        "cmask": nc.dram_tensor("cmask", [128, 128], F32,
                                kind="ExternalInput").ap(),
        "out_pa": nc.dram_tensor("out_pa", [T, C], F16,
                                 kind="ExternalOutput").ap(),
        "out_pb": nc.dram_tensor("out_pb", [T, C], F16,
                                 kind="ExternalOutput").ap(),
    }
    with tile.TileContext(nc) as tc:
        _emit(tc, aps)
    nc.compile()
    _CACHE["nc"] = nc
    return nc


def _make_in_maps(x, Wqkv, bqkv, Wproj):
    x = np.asarray(x, dtype=np.float32)
    Wqkv = np.asarray(Wqkv, dtype=np.float32)
    bqkv = np.asarray(bqkv, dtype=np.float32)
    Wproj = np.asarray(Wproj, dtype=np.float32)

    # triangular causal mask: M[p, f] = 0 if f >= p else -1e9
    p_idx = np.arange(128)[:, None]
    u_idx = np.arange(128)[None, :]
    cmask = np.where(u_idx >= p_idx, 0.0, -1e9).astype(np.float32)

    in_maps = []
    for core in range(N_CORES):
        b, g = divmod(core, 2)
        q0, k0, v0 = 512 * g, C + 512 * g, 2 * C + 512 * g
        wva = np.zeros((C, VW), dtype=np.float32)
        bva = np.zeros((1, VW), dtype=np.float32)
        for h in range(NH):
            src = v0 + D * h
            dst = 65 * h
            # per-head layout [v(64), one]
            wva[:, dst:dst + 64] = Wqkv[:, src:src + 64]
            bva[0, dst:dst + 64] = bqkv[src:src + 64]
            bva[0, dst + 64] = 1.0
        xTf8 = np.ascontiguousarray(x[b].T).astype(ml_dtypes.float8_e4m3fn)
        xr = xTf8.reshape(C // 256, 2, 128, T)
        xt8 = np.concatenate([xr[:, 0], xr[:, 1]], axis=2).reshape(
            C // 2, 2 * T)
        wq_s = (512.0 * Wqkv[:, q0:q0 + F]).astype(ml_dtypes.float8_e4m3fn)
        wk_s = (512.0 * Wqkv[:, k0:k0 + F]).astype(ml_dtypes.float8_e4m3fn)
        wqr = wq_s.reshape(C // 256, 2, 128, F)
        wkr = wk_s.reshape(C // 256, 2, 128, F)
        wqk8 = np.zeros((C // 2, 4 * F), dtype=ml_dtypes.float8_e4m3fn)
        for cp in range(C // 256):
            for pair in range(NPAIR):
                csl = slice(pair * 128, (pair + 1) * 128)
                blk = np.concatenate(
                    [wqr[cp, 0][:, csl], wqr[cp, 1][:, csl],
                     wkr[cp, 0][:, csl], wkr[cp, 1][:, csl]], axis=1)
                wqk8[cp * 128:(cp + 1) * 128,
                     pair * 512:(pair + 1) * 512] = blk
        in_maps.append({
            "xt": np.ascontiguousarray(x[b].T).astype(np.float16),
            "xt8": xt8,
            "wqk8": wqk8,
            "wva": np.ascontiguousarray(wva).astype(np.float16),
            "bq": np.ascontiguousarray(bqkv[q0:q0 + F].reshape(F, 1) * 0.125),
            "bk": np.ascontiguousarray(bqkv[k0:k0 + F].reshape(F, 1)),
            "bva2": np.ascontiguousarray(bva),
            "wp": np.ascontiguousarray(
                Wproj[512 * g:512 * g + F, :]).astype(np.float16),
            "cmask": cmask,
        })
    return in_maps


def run_sharded(x, Wqkv, bqkv, Wproj, bproj, trace=False):
    nc = _build()
    in_maps = _make_in_maps(x, Wqkv, bqkv, Wproj)
    res = run_bass_kernel_spmd(nc, in_maps, core_ids=list(range(N_CORES)),
                               trace=trace)
    bproj = np.asarray(bproj, dtype=np.float32)
    out = np.empty((B, T, C), dtype=np.float32)
    for b in range(B):
        acc = bproj[None, :].astype(np.float32).repeat(T, axis=0)
        for core in (2 * b, 2 * b + 1):
            acc = acc + res.results[core]["out_pa"].astype(np.float32) \
                + res.results[core]["out_pb"].astype(np.float32)
        out[b] = acc
    return out, res


def kernel(x, Wqkv, bqkv, Wproj, bproj):
    out, _ = run_sharded(x, Wqkv, bqkv, Wproj, bproj, trace=False)
    return out


# revision 27
# speedup vs baseline: 1.3361x; 1.0084x over previous
"""Causal self-attention (B=4, T=2048, C=1024, H=16, Dh=64) on 8 trn2 NeuronCores.

Sharding: core i <-> (batch b = i//2, head-group g = i%2). Each core computes
8 heads of one batch end-to-end (qkv slice, causal attention, partial output
projection); the host sums the head-group partials per batch and adds bproj.
No device collectives.

v2 layout: x arrives pre-transposed from the host ([C, T]) so no PE
transposes are needed. Scores use the transposed layout sT[tk, tq]; the two
heads of a pair occupy PE row-groups 0-63 / 64-127 and their score matmuls
are emitted adjacently so the 16x 32x32 sub-arrays run them concurrently.
Softmax denominators come from an extra ones column interleaved into v
(M=65 PV matmuls) and are broadcast across partitions with a DRAM-bounce
DMA. Causal masking only touches the 128 diagonal columns of each k-tile.
"""

import ml_dtypes
import numpy as np

import concourse.bass as bass
import concourse.tile as tile
from concourse import bacc, mybir
from concourse.bass_utils import run_bass_kernel_spmd

F32 = mybir.dt.float32
F16 = mybir.dt.float16
F8 = mybir.dt.float8e4
DR = mybir.MatmulPerfMode.DoubleRow
ADD = mybir.AluOpType.add
MULT = mybir.AluOpType.mult
USE_DR = True        # fp8 DoubleRow PV for off-diagonal k-pairs
USE_GPS_DMA = False  # issue bounce DMAs from the GpSimd queue

N_CORES = 8
B, T, C = 4, 2048, 1024
NH_TOT, D = 16, 64
F = 512            # features per core (8 heads)
NH = 8             # local heads
NPAIR = 4          # head pairs (128 feats each)
CCH = C // 128     # 8 contraction chunks
NTT = T // 128     # 16 t tiles
NTB = T // 512     # 4 t blocks (qkv production)
NQB = T // 512     # 4 q blocks (attention)
VW = NH * (D + 1)  # 520: augmented v width (per-head [v(64), one])


def _emit(tc, aps):
    from contextlib import ExitStack
    nc = tc.nc
    xt, wva, bq, bk, wp = (
        aps["xt"], aps["wva"], aps["bq"], aps["bk"], aps["wp"])
    cmask = aps["cmask"]
    out_ab = [aps["out_pa"], aps["out_pb"]]

    ctx = ExitStack()
    # PSUM: pp_main 3x[128,1024] = 6 banks, pp_pv 2x[128,512] = 2 banks
    pp_main = ctx.enter_context(tc.tile_pool(name="ps_main", bufs=3,
                                             space="PSUM"))
    pp_pv = ctx.enter_context(tc.tile_pool(name="ps_pv", bufs=2, space="PSUM"))
    po_xt = ctx.enter_context(tc.tile_pool(name="xt", bufs=1))
    po_xt8 = ctx.enter_context(tc.tile_pool(name="xt8", bufs=1))
    po_v = ctx.enter_context(tc.tile_pool(name="v_all", bufs=1))
    po_v8 = ctx.enter_context(tc.tile_pool(name="v8", bufs=1))
    po_ysb = ctx.enter_context(tc.tile_pool(name="ysb", bufs=3))
    po_mask = ctx.enter_context(tc.tile_pool(name="mask", bufs=1))
    po_wva = ctx.enter_context(tc.tile_pool(name="wva", bufs=8))
    po_qkt = ctx.enter_context(tc.tile_pool(name="qkT", bufs=4))
    po_bias = ctx.enter_context(tc.tile_pool(name="bias", bufs=1))
    po_wqk = ctx.enter_context(tc.tile_pool(name="wqk", bufs=8))
    po_yt = ctx.enter_context(tc.tile_pool(name="yT", bufs=4))
    po_et = ctx.enter_context(tc.tile_pool(name="expT", bufs=6))
    po_rec = ctx.enter_context(tc.tile_pool(name="recip", bufs=3))
    po_den = ctx.enter_context(tc.tile_pool(name="den", bufs=2))
    po_ytmp = ctx.enter_context(tc.tile_pool(name="ytmp", bufs=2))
    po_ot = ctx.enter_context(tc.tile_pool(name="ot", bufs=3))
    po_wp = ctx.enter_context(tc.tile_pool(name="wp", bufs=4))
    po_dram = ctx.enter_context(tc.tile_pool(name="dram_scr", bufs=4,
                                             space="DRAM"))

    mask_sb = po_mask.tile([128, 128], F32, tag="mask")
    nc.sync.dma_start(out=mask_sb[:], in_=cmask[:])
    # bva broadcast to all 128 partitions straight from DRAM ([1,VW] src)
    bva_bc = po_bias.tile([128, VW], F32, tag="bva_bc")
    bva2 = aps["bva2"]
    nc.sync.dma_start(out=bva_bc[:], in_=bass.AP(
        tensor=bva2.tensor, offset=bva2.offset,
        ap=[[0, 128]] + [list(a) for a in bva2.ap[1:]]))

    # ---- phase 0: xT straight from DRAM (host pre-transposed).
    # Startup DMAs rotate across idle engine queues for parallel dispatch.
    dqs = [nc.sync, nc.gpsimd]
    xT = [po_xt.tile([128, T], F16, tag=f"xT{c}", name=f"xT{c}")
          for c in range(CCH)]
    for ch in range(2):
        csl = slice(ch * (T // 2), (ch + 1) * (T // 2))
        for c in range(CCH):
            dqs[c % 2].dma_start(out=xT[c][:, csl],
                                 in_=xt[c * 128:(c + 1) * 128, csl])
    xt8 = aps["xt8"]
    xT8 = [po_xt8.tile([128, 2 * T], F8, tag=f"xT8_{cp}", name=f"xT8_{cp}")
           for cp in range(CCH // 2)]
    for cp in range(CCH // 2):
        dqs[(cp + 2) % 2].dma_start(
            out=xT8[cp][:], in_=xt8[cp * 128:(cp + 1) * 128, :])

    # ---- phase 0b: v tiles [128, 520] = 8 heads x [v(64) | one],
    # wva host-interleaved with ones columns.  v8: fp8 copy, k-tile pairs
    # packed for DoubleRow ([0:520] = even kt, [528:1048] = odd kt). ----
    v16 = [po_v.tile([128, VW], F16, tag=f"v{tt}", name=f"v{tt}")
           for tt in range(NTT)]
    v8 = [po_v8.tile([128, 1056], F8, tag=f"v8_{kp}", name=f"v8_{kp}")
          for kp in range(NTT // 2)]
    wva_sb = []
    for c in range(CCH):
        wt = po_wva.tile([128, VW], F16, tag="wva")
        dqs[(c + 1) % 2].dma_start(out=wt[:],
                                   in_=wva[c * 128:(c + 1) * 128, :])
        wva_sb.append(wt)

    def v_units():
        units = []
        for tt in range(NTT):
            def unit(tt=tt):
                ps2 = pp_main.tile([128, 1024], F32, tag="main", name="psv")
                for half in range(2):
                    cs = slice(half * 260, half * 260 + 260)
                    ps = ps2[:, half * 512:half * 512 + 260]
                    for c in range(CCH):
                        nc.tensor.matmul(
                            ps, xT[c][:, tt * 128:(tt + 1) * 128],
                            wva_sb[c][:, cs], start=(c == 0),
                            stop=(c == CCH - 1))
                    nc.vector.tensor_add(v16[tt][:, cs], ps, bva_bc[:, cs])
                if USE_DR:
                    # fp8 copy of the finished v16 tile (SBUF->SBUF)
                    o = (tt % 2) * 528
                    nc.vector.tensor_copy(v8[tt // 2][:, o:o + VW],
                                          v16[tt][:, 0:VW])
            units.append(unit)
        return units

    # ---- per head pair: qkv -> attention -> partial proj ----
    # wqk8 row-block cp, col-block pair: [q_even|q_odd|k_even|k_odd] fp8,
    # weights pre-scaled x512 on host (e4m3 subnormal avoidance).
    wqk8 = aps["wqk8"]

    def prep_qkv(pair):
        psl = slice(pair * 128, (pair + 1) * 128)
        wqk_c = []
        for cp in range(CCH // 2):
            wt = po_wqk.tile([128, 512], F8, tag="wqk", name="wt")
            nc.sync.dma_start(
                out=wt[:],
                in_=wqk8[cp * 128:(cp + 1) * 128,
                         pair * 512:(pair + 1) * 512])
            wqk_c.append(wt)
        bq_sb = po_bias.tile([128, 1], F32, tag=f"bq{pair}", name=f"bq{pair}")
        nc.sync.dma_start(out=bq_sb[:], in_=bq[psl, :])
        bk_sb = po_bias.tile([128, 1], F32, tag=f"bk{pair}", name=f"bk{pair}")
        nc.sync.dma_start(out=bk_sb[:], in_=bk[psl, :])
        qT = po_qkt.tile([128, T], F16, tag="qT", name="qT")
        kT = po_qkt.tile([128, T], F16, tag="kT", name="kT")
        return dict(wqk=wqk_c, bq=bq_sb, bk=bk_sb, qT=qT, kT=kT)

    def qkv_units(st8):
        units = []
        for tb in range(NTB):
            def unit(tb=tb):
                ncp = CCH // 2
                ps2 = pp_main.tile([128, 1024], F32, tag="main", name="psqk")
                psq, psk = ps2[:, 0:512], ps2[:, 512:1024]
                for qk in range(2):
                    dst = psq if qk == 0 else psk
                    for cp in range(ncp):
                        wt = st8["wqk"][cp]
                        lhsT = bass.AP(
                            tensor=wt.tensor,
                            offset=wt[:].offset + qk * 256,
                            ap=[list(wt[:].ap[0]), [128, 2], [1, 128]])
                        x8 = xT8[cp]
                        rhs = bass.AP(
                            tensor=x8.tensor,
                            offset=x8[:].offset + tb * 512,
                            ap=[list(x8[:].ap[0]), [T, 2], [1, 512]])
                        nc.tensor.matmul(dst, lhsT, rhs, start=(cp == 0),
                                         stop=(cp == ncp - 1), perf_mode=DR,
                                         skip_group_check=True)
                tsl = slice(tb * 512, (tb + 1) * 512)
                # psum*(1/(sqrt(D)*512)) + bq/sqrt(D)  (bq pre-scaled)
                nc.vector.tensor_scalar(
                    out=st8["qT"][:, tsl], in0=psq, scalar1=0.125 / 512.0,
                    scalar2=st8["bq"][:], op0=MULT, op1=ADD)
                nc.vector.tensor_scalar(
                    out=st8["kT"][:, tsl], in0=psk, scalar1=1.0 / 512.0,
                    scalar2=st8["bk"][:], op0=MULT, op1=ADD)
            units.append(unit)
        return units

    def attn_units(st8, yt):
        qT, kT = st8["qT"], st8["kT"]
        units = []
        for qb in range(NQB):
            nkt = 4 * qb + 4
            kpairs = [(2 * i, 2 * i + 1) for i in range(nkt // 2)]
            L = len(kpairs)
            state = {"st": {}, "et": {}, "pv": None}

            def offs_of(kp, qb=qb):
                return tuple(max(0, 128 * (kt - 4 * qb)) for kt in kp)

            def emit_scores(i, qb=qb, kpairs=kpairs, state=state,
                            offs_of=offs_of):
                kp = kpairs[i]
                offs = offs_of(kp)
                sts = []
                for s in range(2):
                    sts.append(pp_main.tile([128, 1024], F32, tag="main",
                                            name=f"st{s}"))
                for j in range(2):   # k-tile within pair, outer for adjacency
                    for s in range(2):  # head A/B adjacent -> concurrent
                        rq = slice(s * 64, s * 64 + 64)
                        kt, off = kp[j], offs[j]
                        nc.tensor.matmul(
                            sts[s][:, j * 512 + off:(j + 1) * 512],
                            kT[rq, kt * 128:(kt + 1) * 128],
                            qT[rq, qb * 512 + off:(qb + 1) * 512],
                            start=True, stop=True)
                for s in range(2):
                    if kp[0] >= 4 * qb:
                        # both diag tiles in one op: starts off0 and
                        # 512+off1 differ by 640 for either diag pair
                        st = sts[s]
                        m2 = bass.AP(
                            tensor=st.tensor,
                            offset=st[:].offset + offs[0],
                            ap=[list(st[:].ap[0]), [640, 2], [1, 128]])
                        mb = bass.AP(
                            tensor=mask_sb.tensor,
                            offset=mask_sb[:].offset,
                            ap=[list(mask_sb[:].ap[0]), [0, 2], [1, 128]])
                        nc.vector.tensor_add(m2, m2, mb)
                    elif kp[1] >= 4 * qb:
                        j, off = 1, offs[1]
                        nc.vector.tensor_add(
                            sts[s][:, j * 512 + off:j * 512 + off + 128],
                            sts[s][:, j * 512 + off:j * 512 + off + 128],
                            mask_sb[:])
                state["st"][i] = sts

            def emit_exp(i, qb=qb, kpairs=kpairs, state=state,
                         offs_of=offs_of):
                kp = kpairs[i]
                offs = offs_of(kp)
                lo = offs[0]  # 0 or 256; single op over [lo:1024]
                full = USE_DR and qb >= 1
                sts = state["st"].pop(i)
                ets = []
                for s in range(2):
                    et = po_et.tile([128, 1024], F8 if full else F16,
                                    tag="expT8" if full else "expT",
                                    name="et")
                    nc.scalar.activation(
                        et[:, lo:1024], sts[s][:, lo:1024],
                        mybir.ActivationFunctionType.Exp)
                    if full and kp[0] >= 4 * qb:
                        # DoubleRow reads full strips: zero the regions the
                        # narrowed score matmuls never computed.
                        if lo > 0:
                            nc.vector.memset(et[:, 0:lo], 0.0)
                        if offs[1] > 0:
                            nc.vector.memset(et[:, 512:512 + offs[1]], 0.0)
                    ets.append(et)
                state["et"][i] = ets

            def emit_pv(i, qb=qb, kpairs=kpairs, nkt=nkt, state=state,
                        offs_of=offs_of):
                kp = kpairs[i]
                offs = offs_of(kp)
                full = USE_DR and qb >= 1
                ets = state["et"].pop(i)
                pv = state["pv"]
                for s in range(2):
                    pair_h = st8["pair"] * 2 + s
                    if full:
                        # fp8 DoubleRow: both k-tiles in one matmul
                        v8t = v8[kp[0] // 2]
                        lhsT = bass.AP(
                            tensor=v8t.tensor,
                            offset=v8t[:].offset + pair_h * 65,
                            ap=[list(v8t[:].ap[0]), [528, 2], [1, 65]])
                        et = ets[s]
                        rhs = bass.AP(
                            tensor=et.tensor, offset=et[:].offset,
                            ap=[list(et[:].ap[0]), [512, 2], [1, 512]])
                        nc.tensor.matmul(
                            pv[s][0:65, 0:512], lhsT, rhs,
                            start=(kp[0] == 0), stop=(kp[1] == nkt - 1),
                            perf_mode=DR, skip_group_check=True)
                        continue
                    vsl = slice(pair_h * 65, pair_h * 65 + 65)
                    for j in range(2):
                        kt, off = kp[j], offs[j]
                        nc.tensor.matmul(
                            pv[s][0:65, off:512], v16[kt][:, vsl],
                            ets[s][:, j * 512 + off:(j + 1) * 512],
                            start=(kt == 0), stop=(kt == nkt - 1),
                            skip_group_check=True)

            def finish(qb=qb, state=state):
                pv = state["pv"]
                qsl = slice(qb * 512, (qb + 1) * 512)
                for s in range(2):
                    # evacuate pv early so the bank frees for the next qb
                    ysb = po_ysb.tile([128, 512], F32, tag="ysb",
                                      name="ysb")
                    nc.vector.tensor_copy(ysb[0:65, :], pv[s][0:65, :])
                    dscr = po_dram.tile([1, 512], F32, tag="dscr",
                                        name="dscr")
                    nc.sync.dma_start(out=dscr[:], in_=ysb[64:65, :])
                    rec = po_rec.tile([128, 512], F32, tag="recip",
                                      name="rec")
                    nc.gpsimd.dma_start(out=rec[0:64, :], in_=bass.AP(
                        tensor=dscr.tensor, offset=dscr[:].offset,
                        ap=[[0, 64]] + [list(a) for a in dscr[:].ap[1:]]))
                    nc.vector.reciprocal_approx_fast(rec[0:64, :],
                                                     rec[0:64, :])
                    if s == 0:
                        nc.vector.tensor_mul(yt[0:64, qsl], ysb[0:64, :],
                                             rec[0:64, :])
                    else:
                        # engines can't cross partitions; bounce via DMA
                        ytmp = po_ytmp.tile([128, 512], F16, tag="ytmp",
                                            name="ytmp")
                        nc.vector.tensor_mul(ytmp[0:64, :], ysb[0:64, :],
                                             rec[0:64, :])
                        (nc.gpsimd if USE_GPS_DMA else nc.sync).dma_start(out=yt[64:128, qsl],
                                            in_=ytmp[0:64, :])

            def unit(i, L=L, state=state, emit_exp=emit_exp,
                     emit_pv=emit_pv, emit_scores=emit_scores,
                     finish=finish):
                # pipeline: exp(i-1) first, then pv(i-2), then scores(i)
                if i == 0:
                    state["pv"] = [
                        pp_pv.tile([128, 512], F32, tag="pv", name=f"pv{s}")
                        for s in range(2)]
                if 1 <= i <= L:
                    emit_exp(i - 1)
                if 2 <= i <= L + 1:
                    emit_pv(i - 2)
                if i < L:
                    emit_scores(i)
                if i == L + 1:
                    finish()

            for i in range(L + 2):
                units.append(lambda i=i, u=unit: u(i))
        return units

    def prep_proj(couple):
        wp_sb = []
        for pq in range(2):
            for cb in range(2):
                prow = (couple * 2 + pq) * 128
                wt = po_wp.tile([128, 512], F16, tag="wp", name="wpt")
                nc.sync.dma_start(
                    out=wt[:],
                    in_=wp[prow:prow + 128, cb * 512:(cb + 1) * 512])
                wp_sb.append(wt)
        return wp_sb

    def proj_units(couple, wp_sb, yts):
        out_p = out_ab[couple]
        units = []
        for tt0 in range(0, NTT, 2):
            def unit(tt0=tt0):
                for tt in (tt0, tt0 + 1):
                    ot = po_ot.tile([128, C], F16, tag="ot", name="ot")
                    ps2 = pp_main.tile([128, 1024], F32, tag="main",
                                       name="pproj")
                    # pq outer: stationary (y tile) reused across cb
                    for pq in range(2):
                        for cb in range(2):
                            nc.tensor.matmul(
                                ps2[:, cb * 512:(cb + 1) * 512],
                                yts[pq][:, tt * 128:(tt + 1) * 128],
                                wp_sb[pq * 2 + cb][:],
                                start=(pq == 0), stop=(pq == 1))
                    for cb in range(2):
                        nc.vector.tensor_copy(
                            ot[:, cb * 512:(cb + 1) * 512],
                            ps2[:, cb * 512:(cb + 1) * 512])
                    nc.sync.dma_start(
                        out=out_p[tt * 128:(tt + 1) * 128, :], in_=ot[:])
            units.append(unit)
        return units

    def round_robin(*streams):
        # proportional interleave: each pop advances the stream whose
        # remaining fraction is largest, spreading short fill streams
        # evenly across the long attention stream
        streams = [list(s) for s in streams if s]
        totals = [len(s) for s in streams]
        while any(streams):
            frac = [(len(s) / t, i) for i, (s, t) in
                    enumerate(zip(streams, totals)) if s]
            _, i = max(frac)
            streams[i].pop(0)()

    for u in v_units():
        u()
    pair_state = []
    yts = []
    st0 = prep_qkv(0)
    st0["pair"] = 0
    pair_state.append(st0)
    for u in qkv_units(st0):
        u()
    for p in range(NPAIR):
        yt = po_yt.tile([128, T], F16, tag="yT", name="yt")
        yts.append(yt)
        streams = []
        if p + 1 < NPAIR:
            stn = prep_qkv(p + 1)
            stn["pair"] = p + 1
            pair_state.append(stn)
            streams.append(qkv_units(stn))
        if p == 2:
            wp_sb = prep_proj(0)
            streams.append(proj_units(0, wp_sb, yts[0:2]))
        streams.append(attn_units(pair_state[p], yt))
        round_robin(*streams)
    wp_sb = prep_proj(1)
    for u in proj_units(1, wp_sb, yts[2:4]):
        u()

    ctx.close()


_CACHE = {}


def _build():
    if "nc" in _CACHE:
        return _CACHE["nc"]
    nc = bacc.Bacc("TRN2", target_bir_lowering=False, debug=False,
                   enable_asserts=True, num_devices=N_CORES)
    aps = {
        "xt": nc.dram_tensor("xt", [C, T], F16, kind="ExternalInput").ap(),
        "xt8": nc.dram_tensor("xt8", [C // 2, 2 * T], F8,
                              kind="ExternalInput").ap(),
        "wqk8": nc.dram_tensor("wqk8", [C // 2, 4 * F], F8,
                               kind="ExternalInput").ap(),
        "wva": nc.dram_tensor("wva", [C, VW], F16, kind="ExternalInput").ap(),
        "bq": nc.dram_tensor("bq", [F, 1], F32, kind="ExternalInput").ap(),
        "bk": nc.dram_tensor("bk", [F, 1], F32, kind="ExternalInput").ap(),
        "bva2": nc.dram_tensor("bva2", [1, VW], F32,
                               kind="ExternalInput").ap(),
        "wp": nc.dram_tensor("wp", [F, C], F16, kind="ExternalInput").ap(),
        "cmask": nc.dram_tensor("cmask", [128, 128], F32,
                                kind="ExternalInput").ap(),
        "out_pa": nc.dram_tensor("out_pa", [T, C], F16,
                                 kind="ExternalOutput").ap(),
        "out_pb": nc.dram_tensor("out_pb", [T, C], F16,
                                 kind="ExternalOutput").ap(),
    }
    with tile.TileContext(nc) as tc:
        _emit(tc, aps)
    nc.compile()
    _CACHE["nc"] = nc
    return nc


def _make_in_maps(x, Wqkv, bqkv, Wproj):
    x = np.asarray(x, dtype=np.float32)
    Wqkv = np.asarray(Wqkv, dtype=np.float32)
    bqkv = np.asarray(bqkv, dtype=np.float32)
    Wproj = np.asarray(Wproj, dtype=np.float32)

    # triangular causal mask: M[p, f] = 0 if f >= p else -1e9
    p_idx = np.arange(128)[:, None]
    u_idx = np.arange(128)[None, :]
    cmask = np.where(u_idx >= p_idx, 0.0, -1e9).astype(np.float32)

    in_maps = []
    for core in range(N_CORES):
        b, g = divmod(core, 2)
        q0, k0, v0 = 512 * g, C + 512 * g, 2 * C + 512 * g
        wva = np.zeros((C, VW), dtype=np.float32)
        bva = np.zeros((1, VW), dtype=np.float32)
        for h in range(NH):
            src = v0 + D * h
            dst = 65 * h
            # per-head layout [v(64), one]
            wva[:, dst:dst + 64] = Wqkv[:, src:src + 64]
            bva[0, dst:dst + 64] = bqkv[src:src + 64]
            bva[0, dst + 64] = 1.0
        xTf8 = np.ascontiguousarray(x[b].T).astype(ml_dtypes.float8_e4m3fn)
        xr = xTf8.reshape(C // 256, 2, 128, T)
        xt8 = np.concatenate([xr[:, 0], xr[:, 1]], axis=2).reshape(
            C // 2, 2 * T)
        wq_s = (512.0 * Wqkv[:, q0:q0 + F]).astype(ml_dtypes.float8_e4m3fn)
        wk_s = (512.0 * Wqkv[:, k0:k0 + F]).astype(ml_dtypes.float8_e4m3fn)
        wqr = wq_s.reshape(C // 256, 2, 128, F)
        wkr = wk_s.reshape(C // 256, 2, 128, F)
        wqk8 = np.zeros((C // 2, 4 * F), dtype=ml_dtypes.float8_e4m3fn)
        for cp in range(C // 256):
            for pair in range(NPAIR):
                csl = slice(pair * 128, (pair + 1) * 128)
                blk = np.concatenate(
                    [wqr[cp, 0][:, csl], wqr[cp, 1][:, csl],
                     wkr[cp, 0][:, csl], wkr[cp, 1][:, csl]], axis=1)
                wqk8[cp * 128:(cp + 1) * 128,
                     pair * 512:(pair + 1) * 512] = blk
        in_maps.append({
            "xt": np.ascontiguousarray(x[b].T).astype(np.float16),
            "xt8": xt8,
            "wqk8": wqk8,
            "wva": np.ascontiguousarray(wva).astype(np.float16),
            "bq": np.ascontiguousarray(bqkv[q0:q0 + F].reshape(F, 1) * 0.125),
            "bk": np.ascontiguousarray(bqkv[k0:k0 + F].reshape(F, 1)),
            "bva2": np.ascontiguousarray(bva),
            "wp": np.ascontiguousarray(
                Wproj[512 * g:512 * g + F, :]).astype(np.float16),
            "cmask": cmask,
        })
    return in_maps


def run_sharded(x, Wqkv, bqkv, Wproj, bproj, trace=False):
    nc = _build()
    in_maps = _make_in_maps(x, Wqkv, bqkv, Wproj)
    res = run_bass_kernel_spmd(nc, in_maps, core_ids=list(range(N_CORES)),
                               trace=trace)
    bproj = np.asarray(bproj, dtype=np.float32)
    out = np.empty((B, T, C), dtype=np.float32)
    for b in range(B):
        acc = bproj[None, :].astype(np.float32).repeat(T, axis=0)
        for core in (2 * b, 2 * b + 1):
            acc = acc + res.results[core]["out_pa"].astype(np.float32) \
                + res.results[core]["out_pb"].astype(np.float32)
        out[b] = acc
    return out, res


def kernel(x, Wqkv, bqkv, Wproj, bproj):
    out, _ = run_sharded(x, Wqkv, bqkv, Wproj, bproj, trace=False)
    return out


# revision 28
# speedup vs baseline: 1.3944x; 1.0436x over previous
"""Causal self-attention (B=4, T=2048, C=1024, H=16, Dh=64) on 8 trn2 NeuronCores.

Sharding: core i <-> (batch b = i//2, head-group g = i%2). Each core computes
8 heads of one batch end-to-end (qkv slice, causal attention, partial output
projection); the host sums the head-group partials per batch and adds bproj.
No device collectives.

v2 layout: x arrives pre-transposed from the host ([C, T]) so no PE
transposes are needed. Scores use the transposed layout sT[tk, tq]; the two
heads of a pair occupy PE row-groups 0-63 / 64-127 and their score matmuls
are emitted adjacently so the 16x 32x32 sub-arrays run them concurrently.
Softmax denominators come from an extra ones column interleaved into v
(M=65 PV matmuls) and are broadcast across partitions with a DRAM-bounce
DMA. Causal masking only touches the 128 diagonal columns of each k-tile.
"""

import ml_dtypes
import numpy as np

import concourse.bass as bass
import concourse.tile as tile
from concourse import bacc, mybir
from concourse.bass_utils import run_bass_kernel_spmd

F32 = mybir.dt.float32
F16 = mybir.dt.float16
F8 = mybir.dt.float8e4
DR = mybir.MatmulPerfMode.DoubleRow
ADD = mybir.AluOpType.add
MULT = mybir.AluOpType.mult
USE_DR = True        # fp8 DoubleRow PV for off-diagonal k-pairs
USE_GPS_DMA = False  # issue bounce DMAs from the GpSimd queue

N_CORES = 8
B, T, C = 4, 2048, 1024
NH_TOT, D = 16, 64
F = 512            # features per core (8 heads)
NH = 8             # local heads
NPAIR = 4          # head pairs (128 feats each)
CCH = C // 128     # 8 contraction chunks
NTT = T // 128     # 16 t tiles
NTB = T // 512     # 4 t blocks (qkv production)
NQB = T // 512     # 4 q blocks (attention)
VW = NH * (D + 1)  # 520: augmented v width (per-head [v(64), one])


def _emit(tc, aps):
    from contextlib import ExitStack
    nc = tc.nc
    xt, wva, bq, bk, wp = (
        aps["xt"], aps["wva"], aps["bq"], aps["bk"], aps["wp"])
    cmask = aps["cmask"]
    out_ab = [aps["out_pa"], aps["out_pb"]]

    ctx = ExitStack()
    # PSUM: pp_main 3x[128,1024] = 6 banks, pp_pv 2x[128,512] = 2 banks
    pp_main = ctx.enter_context(tc.tile_pool(name="ps_main", bufs=3,
                                             space="PSUM"))
    pp_pv = ctx.enter_context(tc.tile_pool(name="ps_pv", bufs=2, space="PSUM"))
    po_xt = ctx.enter_context(tc.tile_pool(name="xt", bufs=1))
    po_xt8 = ctx.enter_context(tc.tile_pool(name="xt8", bufs=1))
    po_v = ctx.enter_context(tc.tile_pool(name="v_all", bufs=1))
    po_v8 = ctx.enter_context(tc.tile_pool(name="v8", bufs=1))
    po_ysb = ctx.enter_context(tc.tile_pool(name="ysb", bufs=3))
    po_mask = ctx.enter_context(tc.tile_pool(name="mask", bufs=1))
    po_wva = ctx.enter_context(tc.tile_pool(name="wva", bufs=8))
    po_qkt = ctx.enter_context(tc.tile_pool(name="qkT", bufs=4))
    po_bias = ctx.enter_context(tc.tile_pool(name="bias", bufs=1))
    po_wqk = ctx.enter_context(tc.tile_pool(name="wqk", bufs=8))
    po_yt = ctx.enter_context(tc.tile_pool(name="yT", bufs=4))
    po_et = ctx.enter_context(tc.tile_pool(name="expT", bufs=6))
    po_rec = ctx.enter_context(tc.tile_pool(name="recip", bufs=3))
    po_den = ctx.enter_context(tc.tile_pool(name="den", bufs=2))
    po_ytmp = ctx.enter_context(tc.tile_pool(name="ytmp", bufs=2))
    po_ot = ctx.enter_context(tc.tile_pool(name="ot", bufs=3))
    po_wp = ctx.enter_context(tc.tile_pool(name="wp", bufs=4))
    po_dram = ctx.enter_context(tc.tile_pool(name="dram_scr", bufs=4,
                                             space="DRAM"))

    mask_sb = po_mask.tile([128, 128], F32, tag="mask")
    nc.sync.dma_start(out=mask_sb[:], in_=cmask[:])
    # bva broadcast to all 128 partitions straight from DRAM ([1,VW] src)
    bva_bc = po_bias.tile([128, VW], F32, tag="bva_bc")
    bva2 = aps["bva2"]
    nc.sync.dma_start(out=bva_bc[:], in_=bass.AP(
        tensor=bva2.tensor, offset=bva2.offset,
        ap=[[0, 128]] + [list(a) for a in bva2.ap[1:]]))

    # ---- phase 0: fp8 x (qkv path) loads first on the sync queue; the
    # big fp16 xT (v path) streams on the scalar+gpsimd queues so qkv can
    # start the PE early.
    xt8 = aps["xt8"]
    xT8 = [po_xt8.tile([128, 2 * T], F8, tag=f"xT8_{cp}", name=f"xT8_{cp}")
           for cp in range(CCH // 2)]
    for cp in range(CCH // 2):
        nc.sync.dma_start(
            out=xT8[cp][:], in_=xt8[cp * 128:(cp + 1) * 128, :])
    dqs = [nc.scalar, nc.gpsimd]
    xT = [po_xt.tile([128, T], F16, tag=f"xT{c}", name=f"xT{c}")
          for c in range(CCH)]
    for ch in range(2):
        csl = slice(ch * (T // 2), (ch + 1) * (T // 2))
        for c in range(CCH):
            dqs[c % 2].dma_start(out=xT[c][:, csl],
                                 in_=xt[c * 128:(c + 1) * 128, csl])

    # ---- phase 0b: v tiles [128, 520] = 8 heads x [v(64) | one],
    # wva host-interleaved with ones columns.  v8: fp8 copy, k-tile pairs
    # packed for DoubleRow ([0:520] = even kt, [528:1048] = odd kt). ----
    v16 = [po_v.tile([128, VW], F16, tag=f"v{tt}", name=f"v{tt}")
           for tt in range(NTT)]
    v8 = [po_v8.tile([128, 1056], F8, tag=f"v8_{kp}", name=f"v8_{kp}")
          for kp in range(NTT // 2)]
    wva_sb = []
    for c in range(CCH):
        wt = po_wva.tile([128, VW], F16, tag="wva")
        nc.sync.dma_start(out=wt[:],
                          in_=wva[c * 128:(c + 1) * 128, :])
        wva_sb.append(wt)

    def v_units():
        units = []
        for tt in range(NTT):
            def unit(tt=tt):
                ps2 = pp_main.tile([128, 1024], F32, tag="main", name="psv")
                for half in range(2):
                    cs = slice(half * 260, half * 260 + 260)
                    ps = ps2[:, half * 512:half * 512 + 260]
                    for c in range(CCH):
                        nc.tensor.matmul(
                            ps, xT[c][:, tt * 128:(tt + 1) * 128],
                            wva_sb[c][:, cs], start=(c == 0),
                            stop=(c == CCH - 1))
                    nc.vector.tensor_add(v16[tt][:, cs], ps, bva_bc[:, cs])
                if USE_DR:
                    # fp8 copy of the finished v16 tile (SBUF->SBUF)
                    o = (tt % 2) * 528
                    nc.vector.tensor_copy(v8[tt // 2][:, o:o + VW],
                                          v16[tt][:, 0:VW])
            units.append(unit)
        return units

    # ---- per head pair: qkv -> attention -> partial proj ----
    # wqk8 row-block cp, col-block pair: [q_even|q_odd|k_even|k_odd] fp8,
    # weights pre-scaled x512 on host (e4m3 subnormal avoidance).
    wqk8 = aps["wqk8"]

    def prep_qkv(pair):
        psl = slice(pair * 128, (pair + 1) * 128)
        wqk_c = []
        for cp in range(CCH // 2):
            wt = po_wqk.tile([128, 512], F8, tag="wqk", name="wt")
            nc.sync.dma_start(
                out=wt[:],
                in_=wqk8[cp * 128:(cp + 1) * 128,
                         pair * 512:(pair + 1) * 512])
            wqk_c.append(wt)
        bq_sb = po_bias.tile([128, 1], F32, tag=f"bq{pair}", name=f"bq{pair}")
        nc.sync.dma_start(out=bq_sb[:], in_=bq[psl, :])
        bk_sb = po_bias.tile([128, 1], F32, tag=f"bk{pair}", name=f"bk{pair}")
        nc.sync.dma_start(out=bk_sb[:], in_=bk[psl, :])
        qT = po_qkt.tile([128, T], F16, tag="qT", name="qT")
        kT = po_qkt.tile([128, T], F16, tag="kT", name="kT")
        return dict(wqk=wqk_c, bq=bq_sb, bk=bk_sb, qT=qT, kT=kT)

    def qkv_units(st8):
        units = []
        for tb in range(NTB):
            def unit(tb=tb):
                ncp = CCH // 2
                ps2 = pp_main.tile([128, 1024], F32, tag="main", name="psqk")
                psq, psk = ps2[:, 0:512], ps2[:, 512:1024]
                for qk in range(2):
                    dst = psq if qk == 0 else psk
                    for cp in range(ncp):
                        wt = st8["wqk"][cp]
                        lhsT = bass.AP(
                            tensor=wt.tensor,
                            offset=wt[:].offset + qk * 256,
                            ap=[list(wt[:].ap[0]), [128, 2], [1, 128]])
                        x8 = xT8[cp]
                        rhs = bass.AP(
                            tensor=x8.tensor,
                            offset=x8[:].offset + tb * 512,
                            ap=[list(x8[:].ap[0]), [T, 2], [1, 512]])
                        nc.tensor.matmul(dst, lhsT, rhs, start=(cp == 0),
                                         stop=(cp == ncp - 1), perf_mode=DR,
                                         skip_group_check=True)
                tsl = slice(tb * 512, (tb + 1) * 512)
                # psum*(1/(sqrt(D)*512)) + bq/sqrt(D)  (bq pre-scaled)
                nc.vector.tensor_scalar(
                    out=st8["qT"][:, tsl], in0=psq, scalar1=0.125 / 512.0,
                    scalar2=st8["bq"][:], op0=MULT, op1=ADD)
                nc.vector.tensor_scalar(
                    out=st8["kT"][:, tsl], in0=psk, scalar1=1.0 / 512.0,
                    scalar2=st8["bk"][:], op0=MULT, op1=ADD)
            units.append(unit)
        return units

    def attn_units(st8, yt):
        qT, kT = st8["qT"], st8["kT"]
        units = []
        for qb in range(NQB):
            nkt = 4 * qb + 4
            kpairs = [(2 * i, 2 * i + 1) for i in range(nkt // 2)]
            L = len(kpairs)
            state = {"st": {}, "et": {}, "pv": None}

            def offs_of(kp, qb=qb):
                return tuple(max(0, 128 * (kt - 4 * qb)) for kt in kp)

            def emit_scores(i, qb=qb, kpairs=kpairs, state=state,
                            offs_of=offs_of):
                kp = kpairs[i]
                offs = offs_of(kp)
                sts = []
                for s in range(2):
                    sts.append(pp_main.tile([128, 1024], F32, tag="main",
                                            name=f"st{s}"))
                for j in range(2):   # k-tile within pair, outer for adjacency
                    for s in range(2):  # head A/B adjacent -> concurrent
                        rq = slice(s * 64, s * 64 + 64)
                        kt, off = kp[j], offs[j]
                        nc.tensor.matmul(
                            sts[s][:, j * 512 + off:(j + 1) * 512],
                            kT[rq, kt * 128:(kt + 1) * 128],
                            qT[rq, qb * 512 + off:(qb + 1) * 512],
                            start=True, stop=True)
                for s in range(2):
                    if kp[0] >= 4 * qb:
                        # both diag tiles in one op: starts off0 and
                        # 512+off1 differ by 640 for either diag pair
                        st = sts[s]
                        m2 = bass.AP(
                            tensor=st.tensor,
                            offset=st[:].offset + offs[0],
                            ap=[list(st[:].ap[0]), [640, 2], [1, 128]])
                        mb = bass.AP(
                            tensor=mask_sb.tensor,
                            offset=mask_sb[:].offset,
                            ap=[list(mask_sb[:].ap[0]), [0, 2], [1, 128]])
                        nc.vector.tensor_add(m2, m2, mb)
                    elif kp[1] >= 4 * qb:
                        j, off = 1, offs[1]
                        nc.vector.tensor_add(
                            sts[s][:, j * 512 + off:j * 512 + off + 128],
                            sts[s][:, j * 512 + off:j * 512 + off + 128],
                            mask_sb[:])
                state["st"][i] = sts

            def emit_exp(i, qb=qb, kpairs=kpairs, state=state,
                         offs_of=offs_of):
                kp = kpairs[i]
                offs = offs_of(kp)
                lo = offs[0]  # 0 or 256; single op over [lo:1024]
                full = USE_DR and qb >= 1
                sts = state["st"].pop(i)
                ets = []
                for s in range(2):
                    et = po_et.tile([128, 1024], F8 if full else F16,
                                    tag="expT8" if full else "expT",
                                    name="et")
                    nc.scalar.activation(
                        et[:, lo:1024], sts[s][:, lo:1024],
                        mybir.ActivationFunctionType.Exp)
                    if full and kp[0] >= 4 * qb:
                        # DoubleRow reads full strips: zero the regions the
                        # narrowed score matmuls never computed.
                        if lo > 0:
                            nc.vector.memset(et[:, 0:lo], 0.0)
                        if offs[1] > 0:
                            nc.vector.memset(et[:, 512:512 + offs[1]], 0.0)
                    ets.append(et)
                state["et"][i] = ets

            def emit_pv(i, qb=qb, kpairs=kpairs, nkt=nkt, state=state,
                        offs_of=offs_of):
                kp = kpairs[i]
                offs = offs_of(kp)
                full = USE_DR and qb >= 1
                ets = state["et"].pop(i)
                pv = state["pv"]
                for s in range(2):
                    pair_h = st8["pair"] * 2 + s
                    if full:
                        # fp8 DoubleRow: both k-tiles in one matmul
                        v8t = v8[kp[0] // 2]
                        lhsT = bass.AP(
                            tensor=v8t.tensor,
                            offset=v8t[:].offset + pair_h * 65,
                            ap=[list(v8t[:].ap[0]), [528, 2], [1, 65]])
                        et = ets[s]
                        rhs = bass.AP(
                            tensor=et.tensor, offset=et[:].offset,
                            ap=[list(et[:].ap[0]), [512, 2], [1, 512]])
                        nc.tensor.matmul(
                            pv[s][0:65, 0:512], lhsT, rhs,
                            start=(kp[0] == 0), stop=(kp[1] == nkt - 1),
                            perf_mode=DR, skip_group_check=True)
                        continue
                    vsl = slice(pair_h * 65, pair_h * 65 + 65)
                    for j in range(2):
                        kt, off = kp[j], offs[j]
                        nc.tensor.matmul(
                            pv[s][0:65, off:512], v16[kt][:, vsl],
                            ets[s][:, j * 512 + off:(j + 1) * 512],
                            start=(kt == 0), stop=(kt == nkt - 1),
                            skip_group_check=True)

            def finish(qb=qb, state=state):
                pv = state["pv"]
                qsl = slice(qb * 512, (qb + 1) * 512)
                for s in range(2):
                    # evacuate pv early so the bank frees for the next qb
                    ysb = po_ysb.tile([128, 512], F32, tag="ysb",
                                      name="ysb")
                    nc.vector.tensor_copy(ysb[0:65, :], pv[s][0:65, :])
                    dscr = po_dram.tile([1, 512], F32, tag="dscr",
                                        name="dscr")
                    nc.sync.dma_start(out=dscr[:], in_=ysb[64:65, :])
                    rec = po_rec.tile([128, 512], F32, tag="recip",
                                      name="rec")
                    nc.gpsimd.dma_start(out=rec[0:64, :], in_=bass.AP(
                        tensor=dscr.tensor, offset=dscr[:].offset,
                        ap=[[0, 64]] + [list(a) for a in dscr[:].ap[1:]]))
                    nc.vector.reciprocal_approx_fast(rec[0:64, :],
                                                     rec[0:64, :])
                    if s == 0:
                        nc.vector.tensor_mul(yt[0:64, qsl], ysb[0:64, :],
                                             rec[0:64, :])
                    else:
                        # engines can't cross partitions; bounce via DMA
                        ytmp = po_ytmp.tile([128, 512], F16, tag="ytmp",
                                            name="ytmp")
                        nc.vector.tensor_mul(ytmp[0:64, :], ysb[0:64, :],
                                             rec[0:64, :])
                        (nc.gpsimd if USE_GPS_DMA else nc.sync).dma_start(out=yt[64:128, qsl],
                                            in_=ytmp[0:64, :])

            def unit(i, L=L, state=state, emit_exp=emit_exp,
                     emit_pv=emit_pv, emit_scores=emit_scores,
                     finish=finish):
                # pipeline: exp(i-1) first, then pv(i-2), then scores(i)
                if i == 0:
                    state["pv"] = [
                        pp_pv.tile([128, 512], F32, tag="pv", name=f"pv{s}")
                        for s in range(2)]
                if 1 <= i <= L:
                    emit_exp(i - 1)
                if 2 <= i <= L + 1:
                    emit_pv(i - 2)
                if i < L:
                    emit_scores(i)
                if i == L + 1:
                    finish()

            for i in range(L + 2):
                units.append(lambda i=i, u=unit: u(i))
        return units

    def prep_proj(couple):
        wp_sb = []
        for pq in range(2):
            for cb in range(2):
                prow = (couple * 2 + pq) * 128
                wt = po_wp.tile([128, 512], F16, tag="wp", name="wpt")
                nc.sync.dma_start(
                    out=wt[:],
                    in_=wp[prow:prow + 128, cb * 512:(cb + 1) * 512])
                wp_sb.append(wt)
        return wp_sb

    def proj_units(couple, wp_sb, yts):
        out_p = out_ab[couple]
        units = []
        for tt0 in range(0, NTT, 2):
            def unit(tt0=tt0):
                for tt in (tt0, tt0 + 1):
                    ot = po_ot.tile([128, C], F16, tag="ot", name="ot")
                    ps2 = pp_main.tile([128, 1024], F32, tag="main",
                                       name="pproj")
                    # pq outer: stationary (y tile) reused across cb
                    for pq in range(2):
                        for cb in range(2):
                            nc.tensor.matmul(
                                ps2[:, cb * 512:(cb + 1) * 512],
                                yts[pq][:, tt * 128:(tt + 1) * 128],
                                wp_sb[pq * 2 + cb][:],
                                start=(pq == 0), stop=(pq == 1))
                    for cb in range(2):
                        nc.vector.tensor_copy(
                            ot[:, cb * 512:(cb + 1) * 512],
                            ps2[:, cb * 512:(cb + 1) * 512])
                    nc.sync.dma_start(
                        out=out_p[tt * 128:(tt + 1) * 128, :], in_=ot[:])
            units.append(unit)
        return units

    def round_robin(*streams):
        # proportional interleave: each pop advances the stream whose
        # remaining fraction is largest, spreading short fill streams
        # evenly across the long attention stream
        streams = [list(s) for s in streams if s]
        totals = [len(s) for s in streams]
        while any(streams):
            frac = [(len(s) / t, i) for i, (s, t) in
                    enumerate(zip(streams, totals)) if s]
            _, i = max(frac)
            streams[i].pop(0)()

    vu = v_units()
    pair_state = []
    yts = []
    st0 = prep_qkv(0)
    st0["pair"] = 0
    pair_state.append(st0)
    for u in qkv_units(st0):
        u()
    for u in vu[0:8]:
        u()
    proj0_late = []
    for p in range(NPAIR):
        yt = po_yt.tile([128, T], F16, tag="yT", name="yt")
        yts.append(yt)
        streams = []
        if p == 0:
            streams.append(vu[8:16])
        if p + 1 < NPAIR:
            stn = prep_qkv(p + 1)
            stn["pair"] = p + 1
            pair_state.append(stn)
            streams.append(qkv_units(stn))
        if p == 2:
            wp_sb = prep_proj(0)
            pu = proj_units(0, wp_sb, yts[0:2])
            streams.append(pu[0:3])
            proj0_late = pu[3:]
        if p == 3:
            streams.append(proj0_late)
        streams.append(attn_units(pair_state[p], yt))
        round_robin(*streams)
    wp_sb = prep_proj(1)
    for u in proj_units(1, wp_sb, yts[2:4]):
        u()

    ctx.close()


_CACHE = {}


def _build():
    if "nc" in _CACHE:
        return _CACHE["nc"]
    nc = bacc.Bacc("TRN2", target_bir_lowering=False, debug=False,
                   enable_asserts=True, num_devices=N_CORES)
    aps = {
        "xt": nc.dram_tensor("xt", [C, T], F16, kind="ExternalInput").ap(),
        "xt8": nc.dram_tensor("xt8", [C // 2, 2 * T], F8,
                              kind="ExternalInput").ap(),
        "wqk8": nc.dram_tensor("wqk8", [C // 2, 4 * F], F8,
                               kind="ExternalInput").ap(),
        "wva": nc.dram_tensor("wva", [C, VW], F16, kind="ExternalInput").ap(),
        "bq": nc.dram_tensor("bq", [F, 1], F32, kind="ExternalInput").ap(),
        "bk": nc.dram_tensor("bk", [F, 1], F32, kind="ExternalInput").ap(),
        "bva2": nc.dram_tensor("bva2", [1, VW], F32,
                               kind="ExternalInput").ap(),
        "wp": nc.dram_tensor("wp", [F, C], F16, kind="ExternalInput").ap(),
        "cmask": nc.dram_tensor("cmask", [128, 128], F32,
                                kind="ExternalInput").ap(),
        "out_pa": nc.dram_tensor("out_pa", [T, C], F16,
                                 kind="ExternalOutput").ap(),
        "out_pb": nc.dram_tensor("out_pb", [T, C], F16,
                                 kind="ExternalOutput").ap(),
    }
    with tile.TileContext(nc) as tc:
        _emit(tc, aps)
    nc.compile()
    _CACHE["nc"] = nc
    return nc


def _make_in_maps(x, Wqkv, bqkv, Wproj):
    x = np.asarray(x, dtype=np.float32)
    Wqkv = np.asarray(Wqkv, dtype=np.float32)
    bqkv = np.asarray(bqkv, dtype=np.float32)
    Wproj = np.asarray(Wproj, dtype=np.float32)

    # triangular causal mask: M[p, f] = 0 if f >= p else -1e9
    p_idx = np.arange(128)[:, None]
    u_idx = np.arange(128)[None, :]
    cmask = np.where(u_idx >= p_idx, 0.0, -1e9).astype(np.float32)

    in_maps = []
    for core in range(N_CORES):
        b, g = divmod(core, 2)
        q0, k0, v0 = 512 * g, C + 512 * g, 2 * C + 512 * g
        wva = np.zeros((C, VW), dtype=np.float32)
        bva = np.zeros((1, VW), dtype=np.float32)
        for h in range(NH):
            src = v0 + D * h
            dst = 65 * h
            # per-head layout [v(64), one]
            wva[:, dst:dst + 64] = Wqkv[:, src:src + 64]
            bva[0, dst:dst + 64] = bqkv[src:src + 64]
            bva[0, dst + 64] = 1.0
        xTf8 = np.ascontiguousarray(x[b].T).astype(ml_dtypes.float8_e4m3fn)
        xr = xTf8.reshape(C // 256, 2, 128, T)
        xt8 = np.concatenate([xr[:, 0], xr[:, 1]], axis=2).reshape(
            C // 2, 2 * T)
        wq_s = (512.0 * Wqkv[:, q0:q0 + F]).astype(ml_dtypes.float8_e4m3fn)
        wk_s = (512.0 * Wqkv[:, k0:k0 + F]).astype(ml_dtypes.float8_e4m3fn)
        wqr = wq_s.reshape(C // 256, 2, 128, F)
        wkr = wk_s.reshape(C // 256, 2, 128, F)
        wqk8 = np.zeros((C // 2, 4 * F), dtype=ml_dtypes.float8_e4m3fn)
        for cp in range(C // 256):
            for pair in range(NPAIR):
                csl = slice(pair * 128, (pair + 1) * 128)
                blk = np.concatenate(
                    [wqr[cp, 0][:, csl], wqr[cp, 1][:, csl],
                     wkr[cp, 0][:, csl], wkr[cp, 1][:, csl]], axis=1)
                wqk8[cp * 128:(cp + 1) * 128,
                     pair * 512:(pair + 1) * 512] = blk
        in_maps.append({
            "xt": np.ascontiguousarray(x[b].T).astype(np.float16),
            "xt8": xt8,
            "wqk8": wqk8,
            "wva": np.ascontiguousarray(wva).astype(np.float16),
            "bq": np.ascontiguousarray(bqkv[q0:q0 + F].reshape(F, 1) * 0.125),
            "bk": np.ascontiguousarray(bqkv[k0:k0 + F].reshape(F, 1)),
            "bva2": np.ascontiguousarray(bva),
            "wp": np.ascontiguousarray(
                Wproj[512 * g:512 * g + F, :]).astype(np.float16),
            "cmask": cmask,
        })
    return in_maps


def run_sharded(x, Wqkv, bqkv, Wproj, bproj, trace=False):
    nc = _build()
    in_maps = _make_in_maps(x, Wqkv, bqkv, Wproj)
    res = run_bass_kernel_spmd(nc, in_maps, core_ids=list(range(N_CORES)),
                               trace=trace)
    bproj = np.asarray(bproj, dtype=np.float32)
    out = np.empty((B, T, C), dtype=np.float32)
    for b in range(B):
        acc = bproj[None, :].astype(np.float32).repeat(T, axis=0)
        for core in (2 * b, 2 * b + 1):
            acc = acc + res.results[core]["out_pa"].astype(np.float32) \
                + res.results[core]["out_pb"].astype(np.float32)
        out[b] = acc
    return out, res


def kernel(x, Wqkv, bqkv, Wproj, bproj):
    out, _ = run_sharded(x, Wqkv, bqkv, Wproj, bproj, trace=False)
    return out


# revision 29
# speedup vs baseline: 1.4030x; 1.0062x over previous
"""Causal self-attention (B=4, T=2048, C=1024, H=16, Dh=64) on 8 trn2 NeuronCores.

Sharding: core i <-> (batch b = i//2, head-group g = i%2). Each core computes
8 heads of one batch end-to-end (qkv slice, causal attention, partial output
projection); the host sums the head-group partials per batch and adds bproj.
No device collectives.

v2 layout: x arrives pre-transposed from the host ([C, T]) so no PE
transposes are needed. Scores use the transposed layout sT[tk, tq]; the two
heads of a pair occupy PE row-groups 0-63 / 64-127 and their score matmuls
are emitted adjacently so the 16x 32x32 sub-arrays run them concurrently.
Softmax denominators come from an extra ones column interleaved into v
(M=65 PV matmuls) and are broadcast across partitions with a DRAM-bounce
DMA. Causal masking only touches the 128 diagonal columns of each k-tile.
"""

import ml_dtypes
import numpy as np

import concourse.bass as bass
import concourse.tile as tile
from concourse import bacc, mybir
from concourse.bass_utils import run_bass_kernel_spmd

F32 = mybir.dt.float32
F16 = mybir.dt.float16
F8 = mybir.dt.float8e4
DR = mybir.MatmulPerfMode.DoubleRow
ADD = mybir.AluOpType.add
MULT = mybir.AluOpType.mult
USE_DR = True        # fp8 DoubleRow PV for off-diagonal k-pairs
USE_GPS_DMA = False  # issue bounce DMAs from the GpSimd queue

N_CORES = 8
B, T, C = 4, 2048, 1024
NH_TOT, D = 16, 64
F = 512            # features per core (8 heads)
NH = 8             # local heads
NPAIR = 4          # head pairs (128 feats each)
CCH = C // 128     # 8 contraction chunks
NTT = T // 128     # 16 t tiles
NTB = T // 512     # 4 t blocks (qkv production)
NQB = T // 512     # 4 q blocks (attention)
VW = NH * (D + 1)  # 520: augmented v width (per-head [v(64), one])


def _emit(tc, aps):
    from contextlib import ExitStack
    nc = tc.nc
    xt, wva, bq, bk, wp = (
        aps["xt"], aps["wva"], aps["bq"], aps["bk"], aps["wp"])
    cmask = aps["cmask"]
    out_ab = [aps["out_pa"], aps["out_pb"]]

    ctx = ExitStack()
    # PSUM: pp_main 3x[128,1024] = 6 banks, pp_pv 2x[128,512] = 2 banks
    pp_main = ctx.enter_context(tc.tile_pool(name="ps_main", bufs=3,
                                             space="PSUM"))
    pp_pv = ctx.enter_context(tc.tile_pool(name="ps_pv", bufs=2, space="PSUM"))
    po_xt = ctx.enter_context(tc.tile_pool(name="xt", bufs=1))
    po_xt8 = ctx.enter_context(tc.tile_pool(name="xt8", bufs=1))
    po_v = ctx.enter_context(tc.tile_pool(name="v_all", bufs=1))
    po_v8 = ctx.enter_context(tc.tile_pool(name="v8", bufs=1))
    po_ysb = ctx.enter_context(tc.tile_pool(name="ysb", bufs=3))
    po_mask = ctx.enter_context(tc.tile_pool(name="mask", bufs=1))
    po_wva = ctx.enter_context(tc.tile_pool(name="wva", bufs=8))
    po_qkt = ctx.enter_context(tc.tile_pool(name="qkT", bufs=4))
    po_bias = ctx.enter_context(tc.tile_pool(name="bias", bufs=1))
    po_wqk = ctx.enter_context(tc.tile_pool(name="wqk", bufs=8))
    po_yt = ctx.enter_context(tc.tile_pool(name="yT", bufs=4))
    po_et = ctx.enter_context(tc.tile_pool(name="expT", bufs=6))
    po_rec = ctx.enter_context(tc.tile_pool(name="recip", bufs=3))
    po_den = ctx.enter_context(tc.tile_pool(name="den", bufs=2))
    po_ytmp = ctx.enter_context(tc.tile_pool(name="ytmp", bufs=2))
    po_ot = ctx.enter_context(tc.tile_pool(name="ot", bufs=3))
    po_wp = ctx.enter_context(tc.tile_pool(name="wp", bufs=4))
    po_dram = ctx.enter_context(tc.tile_pool(name="dram_scr", bufs=4,
                                             space="DRAM"))

    mask_sb = po_mask.tile([128, 128], F32, tag="mask")
    nc.sync.dma_start(out=mask_sb[:], in_=cmask[:])
    # bva broadcast to all 128 partitions straight from DRAM ([1,VW] src)
    bva_bc = po_bias.tile([128, VW], F32, tag="bva_bc")
    bva2 = aps["bva2"]
    nc.sync.dma_start(out=bva_bc[:], in_=bass.AP(
        tensor=bva2.tensor, offset=bva2.offset,
        ap=[[0, 128]] + [list(a) for a in bva2.ap[1:]]))

    # ---- per head pair: qkv -> attention -> partial proj ----
    # wqk8 row-block cp, col-block pair: [q_even|q_odd|k_even|k_odd] fp8,
    # weights pre-scaled x512 on host (e4m3 subnormal avoidance).
    wqk8 = aps["wqk8"]

    def prep_qkv(pair):
        psl = slice(pair * 128, (pair + 1) * 128)
        wqk_c = []
        for cp in range(CCH // 2):
            wt = po_wqk.tile([128, 512], F8, tag="wqk", name="wt")
            nc.sync.dma_start(
                out=wt[:],
                in_=wqk8[cp * 128:(cp + 1) * 128,
                         pair * 512:(pair + 1) * 512])
            wqk_c.append(wt)
        bq_sb = po_bias.tile([128, 1], F32, tag=f"bq{pair}", name=f"bq{pair}")
        nc.sync.dma_start(out=bq_sb[:], in_=bq[psl, :])
        bk_sb = po_bias.tile([128, 1], F32, tag=f"bk{pair}", name=f"bk{pair}")
        nc.sync.dma_start(out=bk_sb[:], in_=bk[psl, :])
        qT = po_qkt.tile([128, T], F16, tag="qT", name="qT")
        kT = po_qkt.tile([128, T], F16, tag="kT", name="kT")
        return dict(wqk=wqk_c, bq=bq_sb, bk=bk_sb, qT=qT, kT=kT)

    # ---- phase 0: pair-0 qkv weights then fp8 x on the sync queue; the
    # big fp16 xT (v path) streams on the scalar+gpsimd queues so qkv can
    # start the PE early.
    st0 = prep_qkv(0)
    st0["pair"] = 0
    xt8 = aps["xt8"]
    xT8 = [po_xt8.tile([128, 2 * T], F8, tag=f"xT8_{cp}", name=f"xT8_{cp}")
           for cp in range(CCH // 2)]
    for cp in range(CCH // 2):
        nc.sync.dma_start(
            out=xT8[cp][:], in_=xt8[cp * 128:(cp + 1) * 128, :])
    dqs = [nc.scalar, nc.gpsimd]
    xT = [po_xt.tile([128, T], F16, tag=f"xT{c}", name=f"xT{c}")
          for c in range(CCH)]
    for ch in range(2):
        csl = slice(ch * (T // 2), (ch + 1) * (T // 2))
        for c in range(CCH):
            dqs[c % 2].dma_start(out=xT[c][:, csl],
                                 in_=xt[c * 128:(c + 1) * 128, csl])

    # ---- phase 0b: v tiles [128, 520] = 8 heads x [v(64) | one],
    # wva host-interleaved with ones columns.  v8: fp8 copy, k-tile pairs
    # packed for DoubleRow ([0:520] = even kt, [528:1048] = odd kt). ----
    v16 = [po_v.tile([128, VW], F16, tag=f"v{tt}", name=f"v{tt}")
           for tt in range(NTT)]
    v8 = [po_v8.tile([128, 1056], F8, tag=f"v8_{kp}", name=f"v8_{kp}")
          for kp in range(NTT // 2)]
    wva_sb = []
    for c in range(CCH):
        wt = po_wva.tile([128, VW], F16, tag="wva")
        nc.sync.dma_start(out=wt[:],
                          in_=wva[c * 128:(c + 1) * 128, :])
        wva_sb.append(wt)

    def v_units():
        units = []
        for tt in range(NTT):
            def unit(tt=tt):
                ps2 = pp_main.tile([128, 1024], F32, tag="main", name="psv")
                for half in range(2):
                    cs = slice(half * 260, half * 260 + 260)
                    ps = ps2[:, half * 512:half * 512 + 260]
                    for c in range(CCH):
                        nc.tensor.matmul(
                            ps, xT[c][:, tt * 128:(tt + 1) * 128],
                            wva_sb[c][:, cs], start=(c == 0),
                            stop=(c == CCH - 1))
                    nc.vector.tensor_add(v16[tt][:, cs], ps, bva_bc[:, cs])
                if USE_DR:
                    # fp8 copy of the finished v16 tile (SBUF->SBUF)
                    o = (tt % 2) * 528
                    nc.vector.tensor_copy(v8[tt // 2][:, o:o + VW],
                                          v16[tt][:, 0:VW])
            units.append(unit)
        return units

    def qkv_units(st8):
        units = []
        for tb in range(NTB):
            def unit(tb=tb):
                ncp = CCH // 2
                ps2 = pp_main.tile([128, 1024], F32, tag="main", name="psqk")
                psq, psk = ps2[:, 0:512], ps2[:, 512:1024]
                for qk in range(2):
                    dst = psq if qk == 0 else psk
                    for cp in range(ncp):
                        wt = st8["wqk"][cp]
                        lhsT = bass.AP(
                            tensor=wt.tensor,
                            offset=wt[:].offset + qk * 256,
                            ap=[list(wt[:].ap[0]), [128, 2], [1, 128]])
                        x8 = xT8[cp]
                        rhs = bass.AP(
                            tensor=x8.tensor,
                            offset=x8[:].offset + tb * 512,
                            ap=[list(x8[:].ap[0]), [T, 2], [1, 512]])
                        nc.tensor.matmul(dst, lhsT, rhs, start=(cp == 0),
                                         stop=(cp == ncp - 1), perf_mode=DR,
                                         skip_group_check=True)
                tsl = slice(tb * 512, (tb + 1) * 512)
                # psum*(1/(sqrt(D)*512)) + bq/sqrt(D)  (bq pre-scaled)
                nc.vector.tensor_scalar(
                    out=st8["qT"][:, tsl], in0=psq, scalar1=0.125 / 512.0,
                    scalar2=st8["bq"][:], op0=MULT, op1=ADD)
                nc.vector.tensor_scalar(
                    out=st8["kT"][:, tsl], in0=psk, scalar1=1.0 / 512.0,
                    scalar2=st8["bk"][:], op0=MULT, op1=ADD)
            units.append(unit)
        return units

    def attn_units(st8, yt):
        qT, kT = st8["qT"], st8["kT"]
        units = []
        for qb in range(NQB):
            nkt = 4 * qb + 4
            kpairs = [(2 * i, 2 * i + 1) for i in range(nkt // 2)]
            L = len(kpairs)
            state = {"st": {}, "et": {}, "pv": None}

            def offs_of(kp, qb=qb):
                return tuple(max(0, 128 * (kt - 4 * qb)) for kt in kp)

            def emit_scores(i, qb=qb, kpairs=kpairs, state=state,
                            offs_of=offs_of):
                kp = kpairs[i]
                offs = offs_of(kp)
                sts = []
                for s in range(2):
                    sts.append(pp_main.tile([128, 1024], F32, tag="main",
                                            name=f"st{s}"))
                for j in range(2):   # k-tile within pair, outer for adjacency
                    for s in range(2):  # head A/B adjacent -> concurrent
                        rq = slice(s * 64, s * 64 + 64)
                        kt, off = kp[j], offs[j]
                        nc.tensor.matmul(
                            sts[s][:, j * 512 + off:(j + 1) * 512],
                            kT[rq, kt * 128:(kt + 1) * 128],
                            qT[rq, qb * 512 + off:(qb + 1) * 512],
                            start=True, stop=True)
                for s in range(2):
                    if kp[0] >= 4 * qb:
                        # both diag tiles in one op: starts off0 and
                        # 512+off1 differ by 640 for either diag pair
                        st = sts[s]
                        m2 = bass.AP(
                            tensor=st.tensor,
                            offset=st[:].offset + offs[0],
                            ap=[list(st[:].ap[0]), [640, 2], [1, 128]])
                        mb = bass.AP(
                            tensor=mask_sb.tensor,
                            offset=mask_sb[:].offset,
                            ap=[list(mask_sb[:].ap[0]), [0, 2], [1, 128]])
                        nc.vector.tensor_add(m2, m2, mb)
                    elif kp[1] >= 4 * qb:
                        j, off = 1, offs[1]
                        nc.vector.tensor_add(
                            sts[s][:, j * 512 + off:j * 512 + off + 128],
                            sts[s][:, j * 512 + off:j * 512 + off + 128],
                            mask_sb[:])
                state["st"][i] = sts

            def emit_exp(i, qb=qb, kpairs=kpairs, state=state,
                         offs_of=offs_of):
                kp = kpairs[i]
                offs = offs_of(kp)
                lo = offs[0]  # 0 or 256; single op over [lo:1024]
                full = USE_DR and qb >= 1
                sts = state["st"].pop(i)
                ets = []
                for s in range(2):
                    et = po_et.tile([128, 1024], F8 if full else F16,
                                    tag="expT8" if full else "expT",
                                    name="et")
                    nc.scalar.activation(
                        et[:, lo:1024], sts[s][:, lo:1024],
                        mybir.ActivationFunctionType.Exp)
                    if full and kp[0] >= 4 * qb:
                        # DoubleRow reads full strips: zero the regions the
                        # narrowed score matmuls never computed.
                        if lo > 0:
                            nc.vector.memset(et[:, 0:lo], 0.0)
                        if offs[1] > 0:
                            nc.vector.memset(et[:, 512:512 + offs[1]], 0.0)
                    ets.append(et)
                state["et"][i] = ets

            def emit_pv(i, qb=qb, kpairs=kpairs, nkt=nkt, state=state,
                        offs_of=offs_of):
                kp = kpairs[i]
                offs = offs_of(kp)
                full = USE_DR and qb >= 1
                ets = state["et"].pop(i)
                pv = state["pv"]
                for s in range(2):
                    pair_h = st8["pair"] * 2 + s
                    if full:
                        # fp8 DoubleRow: both k-tiles in one matmul
                        v8t = v8[kp[0] // 2]
                        lhsT = bass.AP(
                            tensor=v8t.tensor,
                            offset=v8t[:].offset + pair_h * 65,
                            ap=[list(v8t[:].ap[0]), [528, 2], [1, 65]])
                        et = ets[s]
                        rhs = bass.AP(
                            tensor=et.tensor, offset=et[:].offset,
                            ap=[list(et[:].ap[0]), [512, 2], [1, 512]])
                        nc.tensor.matmul(
                            pv[s][0:65, 0:512], lhsT, rhs,
                            start=(kp[0] == 0), stop=(kp[1] == nkt - 1),
                            perf_mode=DR, skip_group_check=True)
                        continue
                    vsl = slice(pair_h * 65, pair_h * 65 + 65)
                    for j in range(2):
                        kt, off = kp[j], offs[j]
                        nc.tensor.matmul(
                            pv[s][0:65, off:512], v16[kt][:, vsl],
                            ets[s][:, j * 512 + off:(j + 1) * 512],
                            start=(kt == 0), stop=(kt == nkt - 1),
                            skip_group_check=True)

            def finish(qb=qb, state=state):
                pv = state["pv"]
                qsl = slice(qb * 512, (qb + 1) * 512)
                for s in range(2):
                    # evacuate pv early so the bank frees for the next qb
                    ysb = po_ysb.tile([128, 512], F32, tag="ysb",
                                      name="ysb")
                    nc.vector.tensor_copy(ysb[0:65, :], pv[s][0:65, :])
                    dscr = po_dram.tile([1, 512], F32, tag="dscr",
                                        name="dscr")
                    nc.sync.dma_start(out=dscr[:], in_=ysb[64:65, :])
                    rec = po_rec.tile([128, 512], F32, tag="recip",
                                      name="rec")
                    nc.gpsimd.dma_start(out=rec[0:64, :], in_=bass.AP(
                        tensor=dscr.tensor, offset=dscr[:].offset,
                        ap=[[0, 64]] + [list(a) for a in dscr[:].ap[1:]]))
                    nc.vector.reciprocal_approx_fast(rec[0:64, :],
                                                     rec[0:64, :])
                    if s == 0:
                        nc.vector.tensor_mul(yt[0:64, qsl], ysb[0:64, :],
                                             rec[0:64, :])
                    else:
                        # engines can't cross partitions; bounce via DMA
                        ytmp = po_ytmp.tile([128, 512], F16, tag="ytmp",
                                            name="ytmp")
                        nc.vector.tensor_mul(ytmp[0:64, :], ysb[0:64, :],
                                             rec[0:64, :])
                        (nc.gpsimd if USE_GPS_DMA else nc.sync).dma_start(out=yt[64:128, qsl],
                                            in_=ytmp[0:64, :])

            def unit(i, L=L, state=state, emit_exp=emit_exp,
                     emit_pv=emit_pv, emit_scores=emit_scores,
                     finish=finish):
                # pipeline: exp(i-1) first, then pv(i-2), then scores(i)
                if i == 0:
                    state["pv"] = [
                        pp_pv.tile([128, 512], F32, tag="pv", name=f"pv{s}")
                        for s in range(2)]
                if 1 <= i <= L:
                    emit_exp(i - 1)
                if 2 <= i <= L + 1:
                    emit_pv(i - 2)
                if i < L:
                    emit_scores(i)
                if i == L + 1:
                    finish()

            for i in range(L + 2):
                units.append(lambda i=i, u=unit: u(i))
        return units

    def prep_proj(couple):
        wp_sb = []
        for pq in range(2):
            for cb in range(2):
                prow = (couple * 2 + pq) * 128
                wt = po_wp.tile([128, 512], F16, tag="wp", name="wpt")
                nc.sync.dma_start(
                    out=wt[:],
                    in_=wp[prow:prow + 128, cb * 512:(cb + 1) * 512])
                wp_sb.append(wt)
        return wp_sb

    def proj_units(couple, wp_sb, yts):
        out_p = out_ab[couple]
        units = []
        for tt0 in range(0, NTT, 2):
            def unit(tt0=tt0):
                for tt in (tt0, tt0 + 1):
                    ot = po_ot.tile([128, C], F16, tag="ot", name="ot")
                    ps2 = pp_main.tile([128, 1024], F32, tag="main",
                                       name="pproj")
                    # pq outer: stationary (y tile) reused across cb
                    for pq in range(2):
                        for cb in range(2):
                            nc.tensor.matmul(
                                ps2[:, cb * 512:(cb + 1) * 512],
                                yts[pq][:, tt * 128:(tt + 1) * 128],
                                wp_sb[pq * 2 + cb][:],
                                start=(pq == 0), stop=(pq == 1))
                    for cb in range(2):
                        nc.vector.tensor_copy(
                            ot[:, cb * 512:(cb + 1) * 512],
                            ps2[:, cb * 512:(cb + 1) * 512])
                    nc.sync.dma_start(
                        out=out_p[tt * 128:(tt + 1) * 128, :], in_=ot[:])
            units.append(unit)
        return units

    def round_robin(*streams):
        # proportional interleave: each pop advances the stream whose
        # remaining fraction is largest, spreading short fill streams
        # evenly across the long attention stream
        streams = [list(s) for s in streams if s]
        totals = [len(s) for s in streams]
        while any(streams):
            frac = [(len(s) / t, i) for i, (s, t) in
                    enumerate(zip(streams, totals)) if s]
            _, i = max(frac)
            streams[i].pop(0)()

    vu = v_units()
    pair_state = []
    yts = []
    pair_state.append(st0)
    for u in qkv_units(st0):
        u()
    for u in vu[0:8]:
        u()
    proj0_late = []
    for p in range(NPAIR):
        yt = po_yt.tile([128, T], F16, tag="yT", name="yt")
        yts.append(yt)
        streams = []
        if p == 0:
            streams.append(vu[8:16])
        if p + 1 < NPAIR:
            stn = prep_qkv(p + 1)
            stn["pair"] = p + 1
            pair_state.append(stn)
            streams.append(qkv_units(stn))
        if p == 2:
            wp_sb = prep_proj(0)
            pu = proj_units(0, wp_sb, yts[0:2])
            streams.append(pu[0:3])
            proj0_late = pu[3:]
        if p == 3:
            streams.append(proj0_late)
        streams.append(attn_units(pair_state[p], yt))
        round_robin(*streams)
    wp_sb = prep_proj(1)
    for u in proj_units(1, wp_sb, yts[2:4]):
        u()

    ctx.close()


_CACHE = {}


def _build():
    if "nc" in _CACHE:
        return _CACHE["nc"]
    nc = bacc.Bacc("TRN2", target_bir_lowering=False, debug=False,
                   enable_asserts=True, num_devices=N_CORES)
    aps = {
        "xt": nc.dram_tensor("xt", [C, T], F16, kind="ExternalInput").ap(),
        "xt8": nc.dram_tensor("xt8", [C // 2, 2 * T], F8,
                              kind="ExternalInput").ap(),
        "wqk8": nc.dram_tensor("wqk8", [C // 2, 4 * F], F8,
                               kind="ExternalInput").ap(),
        "wva": nc.dram_tensor("wva", [C, VW], F16, kind="ExternalInput").ap(),
        "bq": nc.dram_tensor("bq", [F, 1], F32, kind="ExternalInput").ap(),
        "bk": nc.dram_tensor("bk", [F, 1], F32, kind="ExternalInput").ap(),
        "bva2": nc.dram_tensor("bva2", [1, VW], F32,
                               kind="ExternalInput").ap(),
        "wp": nc.dram_tensor("wp", [F, C], F16, kind="ExternalInput").ap(),
        "cmask": nc.dram_tensor("cmask", [128, 128], F32,
                                kind="ExternalInput").ap(),
        "out_pa": nc.dram_tensor("out_pa", [T, C], F16,
                                 kind="ExternalOutput").ap(),
        "out_pb": nc.dram_tensor("out_pb", [T, C], F16,
                                 kind="ExternalOutput").ap(),
    }
    with tile.TileContext(nc) as tc:
        _emit(tc, aps)
    nc.compile()
    _CACHE["nc"] = nc
    return nc


def _make_in_maps(x, Wqkv, bqkv, Wproj):
    x = np.asarray(x, dtype=np.float32)
    Wqkv = np.asarray(Wqkv, dtype=np.float32)
    bqkv = np.asarray(bqkv, dtype=np.float32)
    Wproj = np.asarray(Wproj, dtype=np.float32)

    # triangular causal mask: M[p, f] = 0 if f >= p else -1e9
    p_idx = np.arange(128)[:, None]
    u_idx = np.arange(128)[None, :]
    cmask = np.where(u_idx >= p_idx, 0.0, -1e9).astype(np.float32)

    in_maps = []
    for core in range(N_CORES):
        b, g = divmod(core, 2)
        q0, k0, v0 = 512 * g, C + 512 * g, 2 * C + 512 * g
        wva = np.zeros((C, VW), dtype=np.float32)
        bva = np.zeros((1, VW), dtype=np.float32)
        for h in range(NH):
            src = v0 + D * h
            dst = 65 * h
            # per-head layout [v(64), one]
            wva[:, dst:dst + 64] = Wqkv[:, src:src + 64]
            bva[0, dst:dst + 64] = bqkv[src:src + 64]
            bva[0, dst + 64] = 1.0
        xTf8 = np.ascontiguousarray(x[b].T).astype(ml_dtypes.float8_e4m3fn)
        xr = xTf8.reshape(C // 256, 2, 128, T)
        xt8 = np.concatenate([xr[:, 0], xr[:, 1]], axis=2).reshape(
            C // 2, 2 * T)
        wq_s = (512.0 * Wqkv[:, q0:q0 + F]).astype(ml_dtypes.float8_e4m3fn)
        wk_s = (512.0 * Wqkv[:, k0:k0 + F]).astype(ml_dtypes.float8_e4m3fn)
        wqr = wq_s.reshape(C // 256, 2, 128, F)
        wkr = wk_s.reshape(C // 256, 2, 128, F)
        wqk8 = np.zeros((C // 2, 4 * F), dtype=ml_dtypes.float8_e4m3fn)
        for cp in range(C // 256):
            for pair in range(NPAIR):
                csl = slice(pair * 128, (pair + 1) * 128)
                blk = np.concatenate(
                    [wqr[cp, 0][:, csl], wqr[cp, 1][:, csl],
                     wkr[cp, 0][:, csl], wkr[cp, 1][:, csl]], axis=1)
                wqk8[cp * 128:(cp + 1) * 128,
                     pair * 512:(pair + 1) * 512] = blk
        in_maps.append({
            "xt": np.ascontiguousarray(x[b].T).astype(np.float16),
            "xt8": xt8,
            "wqk8": wqk8,
            "wva": np.ascontiguousarray(wva).astype(np.float16),
            "bq": np.ascontiguousarray(bqkv[q0:q0 + F].reshape(F, 1) * 0.125),
            "bk": np.ascontiguousarray(bqkv[k0:k0 + F].reshape(F, 1)),
            "bva2": np.ascontiguousarray(bva),
            "wp": np.ascontiguousarray(
                Wproj[512 * g:512 * g + F, :]).astype(np.float16),
            "cmask": cmask,
        })
    return in_maps


def run_sharded(x, Wqkv, bqkv, Wproj, bproj, trace=False):
    nc = _build()
    in_maps = _make_in_maps(x, Wqkv, bqkv, Wproj)
    res = run_bass_kernel_spmd(nc, in_maps, core_ids=list(range(N_CORES)),
                               trace=trace)
    bproj = np.asarray(bproj, dtype=np.float32)
    out = np.empty((B, T, C), dtype=np.float32)
    for b in range(B):
        acc = bproj[None, :].astype(np.float32).repeat(T, axis=0)
        for core in (2 * b, 2 * b + 1):
            acc = acc + res.results[core]["out_pa"].astype(np.float32) \
                + res.results[core]["out_pb"].astype(np.float32)
        out[b] = acc
    return out, res


def kernel(x, Wqkv, bqkv, Wproj, bproj):
    out, _ = run_sharded(x, Wqkv, bqkv, Wproj, bproj, trace=False)
    return out


# revision 32
# speedup vs baseline: 1.4075x; 1.0033x over previous
"""Causal self-attention (B=4, T=2048, C=1024, H=16, Dh=64) on 8 trn2 NeuronCores.

Sharding: core i <-> (batch b = i//2, head-group g = i%2). Each core computes
8 heads of one batch end-to-end (qkv slice, causal attention, partial output
projection); the host sums the head-group partials per batch and adds bproj.
No device collectives.

v2 layout: x arrives pre-transposed from the host ([C, T]) so no PE
transposes are needed. Scores use the transposed layout sT[tk, tq]; the two
heads of a pair occupy PE row-groups 0-63 / 64-127 and their score matmuls
are emitted adjacently so the 16x 32x32 sub-arrays run them concurrently.
Softmax denominators come from an extra ones column interleaved into v
(M=65 PV matmuls) and are broadcast across partitions with a DRAM-bounce
DMA. Causal masking only touches the 128 diagonal columns of each k-tile.
"""

import ml_dtypes
import numpy as np

import concourse.bass as bass
import concourse.tile as tile
from concourse import bacc, mybir
from concourse.bass_utils import run_bass_kernel_spmd

F32 = mybir.dt.float32
F16 = mybir.dt.float16
F8 = mybir.dt.float8e4
DR = mybir.MatmulPerfMode.DoubleRow
ADD = mybir.AluOpType.add
MULT = mybir.AluOpType.mult
USE_DR = True        # fp8 DoubleRow PV for off-diagonal k-pairs
USE_GPS_DMA = False  # issue bounce DMAs from the GpSimd queue

N_CORES = 8
B, T, C = 4, 2048, 1024
NH_TOT, D = 16, 64
F = 512            # features per core (8 heads)
NH = 8             # local heads
NPAIR = 4          # head pairs (128 feats each)
CCH = C // 128     # 8 contraction chunks
NTT = T // 128     # 16 t tiles
NTB = T // 512     # 4 t blocks (qkv production)
NQB = T // 512     # 4 q blocks (attention)
VW = NH * (D + 1)  # 520: augmented v width (per-head [v(64), one])


def _emit(tc, aps):
    from contextlib import ExitStack
    nc = tc.nc
    xt, wva, bq, bk, wp = (
        aps["xt"], aps["wva"], aps["bq"], aps["bk"], aps["wp"])
    cmask = aps["cmask"]
    out_ab = [aps["out_pa"], aps["out_pb"]]

    ctx = ExitStack()
    # PSUM: pp_main 3x[128,1024] = 6 banks, pp_pv 2x[128,512] = 2 banks
    pp_main = ctx.enter_context(tc.tile_pool(name="ps_main", bufs=3,
                                             space="PSUM"))
    pp_pv = ctx.enter_context(tc.tile_pool(name="ps_pv", bufs=2, space="PSUM"))
    po_xt = ctx.enter_context(tc.tile_pool(name="xt", bufs=1))
    po_xt8 = ctx.enter_context(tc.tile_pool(name="xt8", bufs=1))
    po_v = ctx.enter_context(tc.tile_pool(name="v_all", bufs=1))
    po_v8 = ctx.enter_context(tc.tile_pool(name="v8", bufs=1))
    po_ysb = ctx.enter_context(tc.tile_pool(name="ysb", bufs=3))
    po_mask = ctx.enter_context(tc.tile_pool(name="mask", bufs=1))
    po_wva = ctx.enter_context(tc.tile_pool(name="wva", bufs=8))
    po_qkt = ctx.enter_context(tc.tile_pool(name="qkT", bufs=4))
    po_bias = ctx.enter_context(tc.tile_pool(name="bias", bufs=1))
    po_wqk = ctx.enter_context(tc.tile_pool(name="wqk", bufs=8))
    po_yt = ctx.enter_context(tc.tile_pool(name="yT", bufs=4))
    po_et = ctx.enter_context(tc.tile_pool(name="expT", bufs=6))
    po_rec = ctx.enter_context(tc.tile_pool(name="recip", bufs=3))
    po_den = ctx.enter_context(tc.tile_pool(name="den", bufs=2))
    po_ytmp = ctx.enter_context(tc.tile_pool(name="ytmp", bufs=2))
    po_ot = ctx.enter_context(tc.tile_pool(name="ot", bufs=3))
    po_wp = ctx.enter_context(tc.tile_pool(name="wp", bufs=4))
    po_dram = ctx.enter_context(tc.tile_pool(name="dram_scr", bufs=4,
                                             space="DRAM"))

    mask_sb = po_mask.tile([128, 128], F32, tag="mask")
    nc.sync.dma_start(out=mask_sb[:], in_=cmask[:])
    # bva broadcast to all 128 partitions straight from DRAM ([1,VW] src)
    bva_bc = po_bias.tile([128, VW], F32, tag="bva_bc")
    bva2 = aps["bva2"]
    nc.sync.dma_start(out=bva_bc[:], in_=bass.AP(
        tensor=bva2.tensor, offset=bva2.offset,
        ap=[[0, 128]] + [list(a) for a in bva2.ap[1:]]))

    # ---- per head pair: qkv -> attention -> partial proj ----
    # wqk8 row-block cp, col-block pair: [q_even|q_odd|k_even|k_odd] fp8,
    # weights pre-scaled x512 on host (e4m3 subnormal avoidance).
    wqk8 = aps["wqk8"]

    def prep_qkv(pair):
        psl = slice(pair * 128, (pair + 1) * 128)
        wqk_c = []
        for cp in range(CCH // 2):
            wt = po_wqk.tile([128, 512], F8, tag="wqk", name="wt")
            nc.sync.dma_start(
                out=wt[:],
                in_=wqk8[cp * 128:(cp + 1) * 128,
                         pair * 512:(pair + 1) * 512])
            wqk_c.append(wt)
        bq_sb = po_bias.tile([128, 1], F32, tag=f"bq{pair}", name=f"bq{pair}")
        nc.sync.dma_start(out=bq_sb[:], in_=bq[psl, :])
        bk_sb = po_bias.tile([128, 1], F32, tag=f"bk{pair}", name=f"bk{pair}")
        nc.sync.dma_start(out=bk_sb[:], in_=bk[psl, :])
        qT = po_qkt.tile([128, T], F16, tag="qT", name="qT")
        kT = po_qkt.tile([128, T], F16, tag="kT", name="kT")
        return dict(wqk=wqk_c, bq=bq_sb, bk=bk_sb, qT=qT, kT=kT)

    # ---- phase 0: pair-0 qkv weights then fp8 x on the sync queue; the
    # big fp16 xT (v path) streams on the scalar+gpsimd queues so qkv can
    # start the PE early.
    st0 = prep_qkv(0)
    st0["pair"] = 0
    xt8 = aps["xt8"]
    xT8 = [po_xt8.tile([128, 2 * T], F8, tag=f"xT8_{cp}", name=f"xT8_{cp}")
           for cp in range(CCH // 2)]
    for cp in range(CCH // 2):
        nc.sync.dma_start(
            out=xT8[cp][:], in_=xt8[cp * 128:(cp + 1) * 128, :])
    dqs = [nc.scalar, nc.gpsimd]
    xT = [po_xt.tile([128, T], F16, tag=f"xT{c}", name=f"xT{c}")
          for c in range(CCH)]
    for ch in range(2):
        csl = slice(ch * (T // 2), (ch + 1) * (T // 2))
        for c in range(CCH):
            dqs[c % 2].dma_start(out=xT[c][:, csl],
                                 in_=xt[c * 128:(c + 1) * 128, csl])

    # ---- phase 0b: v tiles [128, 520] = 8 heads x [v(64) | one],
    # wva host-interleaved with ones columns.  v8: fp8 copy, k-tile pairs
    # packed for DoubleRow ([0:520] = even kt, [528:1048] = odd kt). ----
    v16 = [po_v.tile([128, VW], F16, tag=f"v{tt}", name=f"v{tt}")
           for tt in range(NTT)]
    v8 = [po_v8.tile([128, 1056], F8, tag=f"v8_{kp}", name=f"v8_{kp}")
          for kp in range(NTT // 2)]
    wva_sb = []
    for c in range(CCH):
        wt = po_wva.tile([128, VW], F16, tag="wva")
        nc.sync.dma_start(out=wt[:],
                          in_=wva[c * 128:(c + 1) * 128, :])
        wva_sb.append(wt)

    def v_units():
        units = []
        for tt in range(NTT):
            def unit(tt=tt):
                ps2 = pp_main.tile([128, 1024], F32, tag="main", name="psv")
                for half in range(2):
                    cs = slice(half * 260, half * 260 + 260)
                    ps = ps2[:, half * 512:half * 512 + 260]
                    for c in range(CCH):
                        nc.tensor.matmul(
                            ps, xT[c][:, tt * 128:(tt + 1) * 128],
                            wva_sb[c][:, cs], start=(c == 0),
                            stop=(c == CCH - 1))
                    nc.vector.tensor_add(v16[tt][:, cs], ps, bva_bc[:, cs])
                if USE_DR:
                    # fp8 copy of the finished v16 tile (SBUF->SBUF)
                    o = (tt % 2) * 528
                    nc.vector.tensor_copy(v8[tt // 2][:, o:o + VW],
                                          v16[tt][:, 0:VW])
            units.append(unit)
        return units

    def qkv_units(st8):
        units = []
        for tb in range(NTB):
            def unit(tb=tb):
                ncp = CCH // 2
                ps2 = pp_main.tile([128, 1024], F32, tag="main", name="psqk")
                psq, psk = ps2[:, 0:512], ps2[:, 512:1024]
                for qk in range(2):
                    dst = psq if qk == 0 else psk
                    for cp in range(ncp):
                        wt = st8["wqk"][cp]
                        lhsT = bass.AP(
                            tensor=wt.tensor,
                            offset=wt[:].offset + qk * 256,
                            ap=[list(wt[:].ap[0]), [128, 2], [1, 128]])
                        x8 = xT8[cp]
                        rhs = bass.AP(
                            tensor=x8.tensor,
                            offset=x8[:].offset + tb * 512,
                            ap=[list(x8[:].ap[0]), [T, 2], [1, 512]])
                        nc.tensor.matmul(dst, lhsT, rhs, start=(cp == 0),
                                         stop=(cp == ncp - 1), perf_mode=DR,
                                         skip_group_check=True)
                tsl = slice(tb * 512, (tb + 1) * 512)
                # psum*(1/(sqrt(D)*512)) + bq/sqrt(D)  (bq pre-scaled)
                nc.vector.tensor_scalar(
                    out=st8["qT"][:, tsl], in0=psq, scalar1=0.125 / 512.0,
                    scalar2=st8["bq"][:], op0=MULT, op1=ADD)
                nc.vector.tensor_scalar(
                    out=st8["kT"][:, tsl], in0=psk, scalar1=1.0 / 512.0,
                    scalar2=st8["bk"][:], op0=MULT, op1=ADD)
            units.append(unit)
        return units

    def attn_units(st8, yt):
        qT, kT = st8["qT"], st8["kT"]
        units = []
        for qb in range(NQB):
            nkt = 4 * qb + 4
            kpairs = [(2 * i, 2 * i + 1) for i in range(nkt // 2)]
            L = len(kpairs)
            state = {"st": {}, "et": {}, "pv": None}

            def offs_of(kp, qb=qb):
                return tuple(max(0, 128 * (kt - 4 * qb)) for kt in kp)

            def emit_scores(i, qb=qb, kpairs=kpairs, state=state,
                            offs_of=offs_of):
                kp = kpairs[i]
                offs = offs_of(kp)
                sts = []
                for s in range(2):
                    sts.append(pp_main.tile([128, 1024], F32, tag="main",
                                            name=f"st{s}"))
                for j in range(2):   # k-tile within pair, outer for adjacency
                    for s in range(2):  # head A/B adjacent -> concurrent
                        rq = slice(s * 64, s * 64 + 64)
                        kt, off = kp[j], offs[j]
                        nc.tensor.matmul(
                            sts[s][:, j * 512 + off:(j + 1) * 512],
                            kT[rq, kt * 128:(kt + 1) * 128],
                            qT[rq, qb * 512 + off:(qb + 1) * 512],
                            start=True, stop=True)
                for s in range(2):
                    if kp[0] >= 4 * qb:
                        # both diag tiles in one op: starts off0 and
                        # 512+off1 differ by 640 for either diag pair
                        st = sts[s]
                        m2 = bass.AP(
                            tensor=st.tensor,
                            offset=st[:].offset + offs[0],
                            ap=[list(st[:].ap[0]), [640, 2], [1, 128]])
                        mb = bass.AP(
                            tensor=mask_sb.tensor,
                            offset=mask_sb[:].offset,
                            ap=[list(mask_sb[:].ap[0]), [0, 2], [1, 128]])
                        nc.vector.tensor_add(m2, m2, mb)
                    elif kp[1] >= 4 * qb:
                        j, off = 1, offs[1]
                        nc.vector.tensor_add(
                            sts[s][:, j * 512 + off:j * 512 + off + 128],
                            sts[s][:, j * 512 + off:j * 512 + off + 128],
                            mask_sb[:])
                state["st"][i] = sts

            def emit_exp(i, qb=qb, kpairs=kpairs, state=state,
                         offs_of=offs_of):
                kp = kpairs[i]
                offs = offs_of(kp)
                lo = offs[0]  # 0 or 256; single op over [lo:1024]
                full = USE_DR and qb >= 1
                sts = state["st"].pop(i)
                ets = []
                for s in range(2):
                    et = po_et.tile([128, 1024], F8 if full else F16,
                                    tag="expT8" if full else "expT",
                                    name="et")
                    nc.scalar.activation(
                        et[:, lo:1024], sts[s][:, lo:1024],
                        mybir.ActivationFunctionType.Exp)
                    if full and kp[0] >= 4 * qb:
                        # DoubleRow reads full strips: zero the regions the
                        # narrowed score matmuls never computed.
                        if lo > 0:
                            nc.vector.memset(et[:, 0:lo], 0.0)
                        if offs[1] > 0:
                            nc.vector.memset(et[:, 512:512 + offs[1]], 0.0)
                    ets.append(et)
                state["et"][i] = ets

            def emit_pv(i, qb=qb, kpairs=kpairs, nkt=nkt, state=state,
                        offs_of=offs_of):
                kp = kpairs[i]
                offs = offs_of(kp)
                full = USE_DR and qb >= 1
                ets = state["et"].pop(i)
                pv = state["pv"]
                for s in range(2):
                    pair_h = st8["pair"] * 2 + s
                    if full:
                        # fp8 DoubleRow: both k-tiles in one matmul
                        v8t = v8[kp[0] // 2]
                        lhsT = bass.AP(
                            tensor=v8t.tensor,
                            offset=v8t[:].offset + pair_h * 65,
                            ap=[list(v8t[:].ap[0]), [528, 2], [1, 65]])
                        et = ets[s]
                        rhs = bass.AP(
                            tensor=et.tensor, offset=et[:].offset,
                            ap=[list(et[:].ap[0]), [512, 2], [1, 512]])
                        nc.tensor.matmul(
                            pv[s][0:65, 0:512], lhsT, rhs,
                            start=(kp[0] == 0), stop=(kp[1] == nkt - 1),
                            perf_mode=DR, skip_group_check=True)
                        continue
                    vsl = slice(pair_h * 65, pair_h * 65 + 65)
                    for j in range(2):
                        kt, off = kp[j], offs[j]
                        nc.tensor.matmul(
                            pv[s][0:65, off:512], v16[kt][:, vsl],
                            ets[s][:, j * 512 + off:(j + 1) * 512],
                            start=(kt == 0), stop=(kt == nkt - 1),
                            skip_group_check=True)

            def finish(qb=qb, state=state):
                pv = state["pv"]
                qsl = slice(qb * 512, (qb + 1) * 512)
                for s in range(2):
                    # evacuate pv early so the bank frees for the next qb
                    ysb = po_ysb.tile([128, 512], F32, tag="ysb",
                                      name="ysb")
                    nc.vector.tensor_copy(ysb[0:65, :], pv[s][0:65, :])
                    dscr = po_dram.tile([1, 512], F32, tag="dscr",
                                        name="dscr")
                    nc.sync.dma_start(out=dscr[:], in_=ysb[64:65, :])
                    rec = po_rec.tile([128, 512], F32, tag="recip",
                                      name="rec")
                    nc.gpsimd.dma_start(out=rec[0:64, :], in_=bass.AP(
                        tensor=dscr.tensor, offset=dscr[:].offset,
                        ap=[[0, 64]] + [list(a) for a in dscr[:].ap[1:]]))
                    nc.vector.reciprocal_approx_fast(rec[0:64, :],
                                                     rec[0:64, :])
                    if s == 0:
                        nc.vector.tensor_mul(yt[0:64, qsl], ysb[0:64, :],
                                             rec[0:64, :])
                    else:
                        # engines can't cross partitions; bounce via DMA
                        ytmp = po_ytmp.tile([128, 512], F16, tag="ytmp",
                                            name="ytmp")
                        nc.vector.tensor_mul(ytmp[0:64, :], ysb[0:64, :],
                                             rec[0:64, :])
                        (nc.gpsimd if USE_GPS_DMA else nc.sync).dma_start(out=yt[64:128, qsl],
                                            in_=ytmp[0:64, :])

            def unit(i, L=L, state=state, emit_exp=emit_exp,
                     emit_pv=emit_pv, emit_scores=emit_scores,
                     finish=finish):
                # pipeline: exp(i-1) first, then pv(i-2), then scores(i)
                if i == 0:
                    state["pv"] = [
                        pp_pv.tile([128, 512], F32, tag="pv", name=f"pv{s}")
                        for s in range(2)]
                if 1 <= i <= L:
                    emit_exp(i - 1)
                if 2 <= i <= L + 1:
                    emit_pv(i - 2)
                if i < L:
                    emit_scores(i)
                if i == L + 1:
                    finish()

            for i in range(L + 2):
                units.append(lambda i=i, u=unit: u(i))
        return units

    def prep_proj(couple):
        wp_sb = []
        for pq in range(2):
            for cb in range(2):
                prow = (couple * 2 + pq) * 128
                wt = po_wp.tile([128, 512], F16, tag="wp", name="wpt")
                nc.sync.dma_start(
                    out=wt[:],
                    in_=wp[prow:prow + 128, cb * 512:(cb + 1) * 512])
                wp_sb.append(wt)
        return wp_sb

    def proj_units(couple, wp_sb, yts):
        out_p = out_ab[couple]
        units = []
        for tt0 in range(0, NTT, 2):
            def unit(tt0=tt0):
                for tt in (tt0, tt0 + 1):
                    ot = po_ot.tile([128, C], F16, tag="ot", name="ot")
                    ps2 = pp_main.tile([128, 1024], F32, tag="main",
                                       name="pproj")
                    # pq outer: stationary (y tile) reused across cb
                    for pq in range(2):
                        for cb in range(2):
                            nc.tensor.matmul(
                                ps2[:, cb * 512:(cb + 1) * 512],
                                yts[pq][:, tt * 128:(tt + 1) * 128],
                                wp_sb[pq * 2 + cb][:],
                                start=(pq == 0), stop=(pq == 1))
                    for cb in range(2):
                        nc.vector.tensor_copy(
                            ot[:, cb * 512:(cb + 1) * 512],
                            ps2[:, cb * 512:(cb + 1) * 512])
                    nc.sync.dma_start(
                        out=out_p[tt * 128:(tt + 1) * 128, :], in_=ot[:])
            units.append(unit)
        return units

    def round_robin(*streams):
        # proportional interleave: each pop advances the stream whose
        # remaining fraction is largest, spreading short fill streams
        # evenly across the long attention stream
        streams = [list(s) for s in streams if s]
        totals = [len(s) for s in streams]
        while any(streams):
            frac = [(len(s) / t, i) for i, (s, t) in
                    enumerate(zip(streams, totals)) if s]
            _, i = max(frac)
            streams[i].pop(0)()

    vu = v_units()
    pair_state = []
    yts = []
    pair_state.append(st0)
    for u in qkv_units(st0):
        u()
    for u in vu[0:8]:
        u()
    proj0_late = []
    for p in range(NPAIR):
        yt = po_yt.tile([128, T], F16, tag="yT", name="yt")
        yts.append(yt)
        streams = []
        if p == 0:
            streams.append(vu[8:16])
        if p + 1 < NPAIR:
            stn = prep_qkv(p + 1)
            stn["pair"] = p + 1
            pair_state.append(stn)
            streams.append(qkv_units(stn))
        if p == 2:
            wp_sb = prep_proj(0)
            pu = proj_units(0, wp_sb, yts[0:2])
            streams.append(pu[0:3])
            proj0_late = pu[3:]
        if p == 3:
            streams.append(proj0_late)
        streams.append(attn_units(pair_state[p], yt))
        round_robin(*streams)
    wp_sb = prep_proj(1)
    for u in proj_units(1, wp_sb, yts[2:4]):
        u()

    ctx.close()


_CACHE = {}


def _build():
    if "nc" in _CACHE:
        return _CACHE["nc"]
    nc = bacc.Bacc("TRN2", target_bir_lowering=False, debug=False,
                   enable_asserts=True, num_devices=N_CORES)
    aps = {
        "xt": nc.dram_tensor("xt", [C, T], F16, kind="ExternalInput").ap(),
        "xt8": nc.dram_tensor("xt8", [C // 2, 2 * T], F8,
                              kind="ExternalInput").ap(),
        "wqk8": nc.dram_tensor("wqk8", [C // 2, 4 * F], F8,
                               kind="ExternalInput").ap(),
        "wva": nc.dram_tensor("wva", [C, VW], F16, kind="ExternalInput").ap(),
        "bq": nc.dram_tensor("bq", [F, 1], F32, kind="ExternalInput").ap(),
        "bk": nc.dram_tensor("bk", [F, 1], F32, kind="ExternalInput").ap(),
        "bva2": nc.dram_tensor("bva2", [1, VW], F32,
                               kind="ExternalInput").ap(),
        "wp": nc.dram_tensor("wp", [F, C], F16, kind="ExternalInput").ap(),
        "cmask": nc.dram_tensor("cmask", [128, 128], F32,
                                kind="ExternalInput").ap(),
        "out_pa": nc.dram_tensor("out_pa", [T, C], F16,
                                 kind="ExternalOutput").ap(),
        "out_pb": nc.dram_tensor("out_pb", [T, C], F16,
                                 kind="ExternalOutput").ap(),
    }
    with tile.TileContext(nc) as tc:
        _emit(tc, aps)
    nc.compile()
    _CACHE["nc"] = nc
    return nc


def _make_in_maps(x, Wqkv, bqkv, Wproj):
    x = np.asarray(x, dtype=np.float32)
    Wqkv = np.asarray(Wqkv, dtype=np.float32)
    bqkv = np.asarray(bqkv, dtype=np.float32)
    Wproj = np.asarray(Wproj, dtype=np.float32)

    # triangular causal mask: M[p, f] = 0 if f >= p else -1e9
    p_idx = np.arange(128)[:, None]
    u_idx = np.arange(128)[None, :]
    cmask = np.where(u_idx >= p_idx, 0.0, -1e9).astype(np.float32)

    in_maps = []
    for core in range(N_CORES):
        b, g = divmod(core, 2)
        q0, k0, v0 = 512 * g, C + 512 * g, 2 * C + 512 * g
        wva = np.zeros((C, VW), dtype=np.float32)
        bva = np.zeros((1, VW), dtype=np.float32)
        for h in range(NH):
            src = v0 + D * h
            dst = 65 * h
            # per-head layout [v(64), one]
            wva[:, dst:dst + 64] = Wqkv[:, src:src + 64]
            bva[0, dst:dst + 64] = bqkv[src:src + 64]
            bva[0, dst + 64] = 1.0
        xTf8 = np.ascontiguousarray(x[b].T).astype(ml_dtypes.float8_e4m3fn)
        xr = xTf8.reshape(C // 256, 2, 128, T)
        xt8 = np.concatenate([xr[:, 0], xr[:, 1]], axis=2).reshape(
            C // 2, 2 * T)
        wq_s = (512.0 * Wqkv[:, q0:q0 + F]).astype(ml_dtypes.float8_e4m3fn)
        wk_s = (512.0 * Wqkv[:, k0:k0 + F]).astype(ml_dtypes.float8_e4m3fn)
        wqr = wq_s.reshape(C // 256, 2, 128, F)
        wkr = wk_s.reshape(C // 256, 2, 128, F)
        wqk8 = np.zeros((C // 2, 4 * F), dtype=ml_dtypes.float8_e4m3fn)
        for cp in range(C // 256):
            for pair in range(NPAIR):
                csl = slice(pair * 128, (pair + 1) * 128)
                blk = np.concatenate(
                    [wqr[cp, 0][:, csl], wqr[cp, 1][:, csl],
                     wkr[cp, 0][:, csl], wkr[cp, 1][:, csl]], axis=1)
                wqk8[cp * 128:(cp + 1) * 128,
                     pair * 512:(pair + 1) * 512] = blk
        in_maps.append({
            "xt": np.ascontiguousarray(x[b].T).astype(np.float16),
            "xt8": xt8,
            "wqk8": wqk8,
            "wva": np.ascontiguousarray(wva).astype(np.float16),
            "bq": np.ascontiguousarray(bqkv[q0:q0 + F].reshape(F, 1) * 0.125),
            "bk": np.ascontiguousarray(bqkv[k0:k0 + F].reshape(F, 1)),
            "bva2": np.ascontiguousarray(bva),
            "wp": np.ascontiguousarray(
                Wproj[512 * g:512 * g + F, :]).astype(np.float16),
            "cmask": cmask,
        })
    return in_maps


def run_sharded(x, Wqkv, bqkv, Wproj, bproj, trace=False):
    nc = _build()
    in_maps = _make_in_maps(x, Wqkv, bqkv, Wproj)
    res = run_bass_kernel_spmd(nc, in_maps, core_ids=list(range(N_CORES)),
                               trace=trace)
    bproj = np.asarray(bproj, dtype=np.float32)
    out = np.empty((B, T, C), dtype=np.float32)
    for b in range(B):
        acc = bproj[None, :].astype(np.float32).repeat(T, axis=0)
        for core in (2 * b, 2 * b + 1):
            acc = acc + res.results[core]["out_pa"].astype(np.float32) \
                + res.results[core]["out_pb"].astype(np.float32)
        out[b] = acc
    return out, res


def kernel(x, Wqkv, bqkv, Wproj, bproj):
    out, _ = run_sharded(x, Wqkv, bqkv, Wproj, bproj, trace=False)
    return out


# revision 33
# speedup vs baseline: 1.4627x; 1.0392x over previous
"""Causal self-attention (B=4, T=2048, C=1024, H=16, Dh=64) on 8 trn2 NeuronCores.

Sharding: core i <-> (batch b = i//2, head-group g = i%2). Each core computes
8 heads of one batch end-to-end (qkv slice, causal attention, partial output
projection); the host sums the head-group partials per batch and adds bproj.
No device collectives.

v2 layout: x arrives pre-transposed from the host ([C, T]) so no PE
transposes are needed. Scores use the transposed layout sT[tk, tq]; the two
heads of a pair occupy PE row-groups 0-63 / 64-127 and their score matmuls
are emitted adjacently so the 16x 32x32 sub-arrays run them concurrently.
Softmax denominators come from an extra ones column interleaved into v
(M=65 PV matmuls) and are broadcast across partitions with a DRAM-bounce
DMA. Causal masking only touches the 128 diagonal columns of each k-tile.
"""

import ml_dtypes
import numpy as np

import concourse.bass as bass
import concourse.tile as tile
from concourse import bacc, mybir
from concourse.bass_utils import run_bass_kernel_spmd

F32 = mybir.dt.float32
F16 = mybir.dt.float16
F8 = mybir.dt.float8e4
DR = mybir.MatmulPerfMode.DoubleRow
ADD = mybir.AluOpType.add
MULT = mybir.AluOpType.mult
USE_DR = True        # fp8 DoubleRow PV for off-diagonal k-pairs
USE_GPS_DMA = False  # issue bounce DMAs from the GpSimd queue

N_CORES = 8
B, T, C = 4, 2048, 1024
NH_TOT, D = 16, 64
F = 512            # features per core (8 heads)
NH = 8             # local heads
NPAIR = 4          # head pairs (128 feats each)
CCH = C // 128     # 8 contraction chunks
NTT = T // 128     # 16 t tiles
NTB = T // 512     # 4 t blocks (qkv production)
NQB = T // 512     # 4 q blocks (attention)
VW = NH * (D + 1)  # 520: augmented v width (per-head [v(64), one])


def _emit(tc, aps):
    from contextlib import ExitStack
    nc = tc.nc
    xt, wva, bq, bk, wp = (
        aps["xt"], aps["wva"], aps["bq"], aps["bk"], aps["wp"])
    cmask = aps["cmask"]
    out_ab = [aps["out_pa"], aps["out_pb"]]

    ctx = ExitStack()
    # PSUM: pp_main 3x[128,1024] = 6 banks, pp_pv 2x[128,512] = 2 banks
    pp_main = ctx.enter_context(tc.tile_pool(name="ps_main", bufs=3,
                                             space="PSUM"))
    pp_pv = ctx.enter_context(tc.tile_pool(name="ps_pv", bufs=2, space="PSUM"))
    po_xt = ctx.enter_context(tc.tile_pool(name="xt", bufs=1))
    po_xt8 = ctx.enter_context(tc.tile_pool(name="xt8", bufs=1))
    po_v = ctx.enter_context(tc.tile_pool(name="v_all", bufs=1))
    po_v8 = ctx.enter_context(tc.tile_pool(name="v8", bufs=1))
    po_ysb = ctx.enter_context(tc.tile_pool(name="ysb", bufs=3))
    po_mask = ctx.enter_context(tc.tile_pool(name="mask", bufs=1))
    po_wva = ctx.enter_context(tc.tile_pool(name="wva", bufs=8))
    po_qkt = ctx.enter_context(tc.tile_pool(name="qkT", bufs=4))
    po_bias = ctx.enter_context(tc.tile_pool(name="bias", bufs=1))
    po_wqk = ctx.enter_context(tc.tile_pool(name="wqk", bufs=8))
    po_yt = ctx.enter_context(tc.tile_pool(name="yT", bufs=4))
    po_et = ctx.enter_context(tc.tile_pool(name="expT", bufs=6))
    po_rec = ctx.enter_context(tc.tile_pool(name="recip", bufs=3))
    po_den = ctx.enter_context(tc.tile_pool(name="den", bufs=2))
    po_ytmp = ctx.enter_context(tc.tile_pool(name="ytmp", bufs=2))
    po_ot = ctx.enter_context(tc.tile_pool(name="ot", bufs=3))
    po_wp = ctx.enter_context(tc.tile_pool(name="wp", bufs=4))
    po_dram = ctx.enter_context(tc.tile_pool(name="dram_scr", bufs=4,
                                             space="DRAM"))

    mask_sb = po_mask.tile([128, 128], F32, tag="mask")
    nc.sync.dma_start(out=mask_sb[:], in_=cmask[:])
    # bva broadcast to all 128 partitions straight from DRAM ([1,VW] src)
    bva_bc = po_bias.tile([128, VW], F32, tag="bva_bc")
    bva2 = aps["bva2"]
    nc.sync.dma_start(out=bva_bc[:], in_=bass.AP(
        tensor=bva2.tensor, offset=bva2.offset,
        ap=[[0, 128]] + [list(a) for a in bva2.ap[1:]]))

    # ---- per head pair: qkv -> attention -> partial proj ----
    # wqk8 row-block cp, col-block pair: [q_even|q_odd|k_even|k_odd] fp8,
    # weights pre-scaled x512 on host (e4m3 subnormal avoidance).
    wqk8 = aps["wqk8"]

    def prep_qkv(pair):
        psl = slice(pair * 128, (pair + 1) * 128)
        wqk_c = []
        for cp in range(CCH // 2):
            wt = po_wqk.tile([128, 512], F8, tag="wqk", name="wt")
            nc.sync.dma_start(
                out=wt[:],
                in_=wqk8[cp * 128:(cp + 1) * 128,
                         pair * 512:(pair + 1) * 512])
            wqk_c.append(wt)
        bq_sb = po_bias.tile([128, 1], F32, tag=f"bq{pair}", name=f"bq{pair}")
        nc.sync.dma_start(out=bq_sb[:], in_=bq[psl, :])
        bk_sb = po_bias.tile([128, 1], F32, tag=f"bk{pair}", name=f"bk{pair}")
        nc.sync.dma_start(out=bk_sb[:], in_=bk[psl, :])
        qT = po_qkt.tile([128, T], F16, tag="qT", name="qT")
        kT = po_qkt.tile([128, T], F16, tag="kT", name="kT")
        return dict(wqk=wqk_c, bq=bq_sb, bk=bk_sb, qT=qT, kT=kT)

    # ---- phase 0: pair-0 qkv weights then fp8 x on the sync queue; the
    # big fp16 xT (v path) streams on the scalar+gpsimd queues so qkv can
    # start the PE early.
    st0 = prep_qkv(0)
    st0["pair"] = 0
    xt8 = aps["xt8"]
    xT8 = [po_xt8.tile([128, 2 * T], F8, tag=f"xT8_{cp}", name=f"xT8_{cp}")
           for cp in range(CCH // 2)]
    for cp in range(CCH // 2):
        nc.sync.dma_start(
            out=xT8[cp][:], in_=xt8[cp * 128:(cp + 1) * 128, :])
    dqs = [nc.scalar, nc.gpsimd]
    xT = [po_xt.tile([128, T], F16, tag=f"xT{c}", name=f"xT{c}")
          for c in range(CCH)]
    for ch in range(2):
        csl = slice(ch * (T // 2), (ch + 1) * (T // 2))
        for c in range(CCH):
            dqs[c % 2].dma_start(out=xT[c][:, csl],
                                 in_=xt[c * 128:(c + 1) * 128, csl])

    # ---- phase 0b: v tiles [128, 520] = 8 heads x [v(64) | one],
    # wva host-interleaved with ones columns.  v8: fp8 copy, k-tile pairs
    # packed for DoubleRow ([0:520] = even kt, [528:1048] = odd kt). ----
    v16 = [po_v.tile([128, VW], F16, tag=f"v{tt}", name=f"v{tt}")
           for tt in range(NTT)]
    v8 = [po_v8.tile([128, 1056], F8, tag=f"v8_{kp}", name=f"v8_{kp}")
          for kp in range(NTT // 2)]
    wva_sb = []
    for c in range(CCH):
        wt = po_wva.tile([128, VW], F16, tag="wva")
        dqs[c % 2].dma_start(out=wt[:],
                             in_=wva[c * 128:(c + 1) * 128, :])
        wva_sb.append(wt)

    def v_units():
        units = []
        for tt in range(NTT):
            def unit(tt=tt):
                ps2 = pp_main.tile([128, 1024], F32, tag="main", name="psv")
                for half in range(2):
                    cs = slice(half * 260, half * 260 + 260)
                    ps = ps2[:, half * 512:half * 512 + 260]
                    for c in range(CCH):
                        nc.tensor.matmul(
                            ps, xT[c][:, tt * 128:(tt + 1) * 128],
                            wva_sb[c][:, cs], start=(c == 0),
                            stop=(c == CCH - 1))
                    nc.vector.tensor_add(v16[tt][:, cs], ps, bva_bc[:, cs])
                if USE_DR:
                    # fp8 copy of the finished v16 tile (SBUF->SBUF)
                    o = (tt % 2) * 528
                    nc.vector.tensor_copy(v8[tt // 2][:, o:o + VW],
                                          v16[tt][:, 0:VW])
            units.append(unit)
        return units

    def qkv_units(st8):
        units = []
        for tb in range(NTB):
            def unit(tb=tb):
                ncp = CCH // 2
                ps2 = pp_main.tile([128, 1024], F32, tag="main", name="psqk")
                psq, psk = ps2[:, 0:512], ps2[:, 512:1024]
                for qk in range(2):
                    dst = psq if qk == 0 else psk
                    for cp in range(ncp):
                        wt = st8["wqk"][cp]
                        lhsT = bass.AP(
                            tensor=wt.tensor,
                            offset=wt[:].offset + qk * 256,
                            ap=[list(wt[:].ap[0]), [128, 2], [1, 128]])
                        x8 = xT8[cp]
                        rhs = bass.AP(
                            tensor=x8.tensor,
                            offset=x8[:].offset + tb * 512,
                            ap=[list(x8[:].ap[0]), [T, 2], [1, 512]])
                        nc.tensor.matmul(dst, lhsT, rhs, start=(cp == 0),
                                         stop=(cp == ncp - 1), perf_mode=DR,
                                         skip_group_check=True)
                tsl = slice(tb * 512, (tb + 1) * 512)
                # psum*(1/(sqrt(D)*512)) + bq/sqrt(D)  (bq pre-scaled)
                nc.vector.tensor_scalar(
                    out=st8["qT"][:, tsl], in0=psq, scalar1=0.125 / 512.0,
                    scalar2=st8["bq"][:], op0=MULT, op1=ADD)
                nc.vector.tensor_scalar(
                    out=st8["kT"][:, tsl], in0=psk, scalar1=1.0 / 512.0,
                    scalar2=st8["bk"][:], op0=MULT, op1=ADD)
            units.append(unit)
        return units

    def attn_units(st8, yt):
        qT, kT = st8["qT"], st8["kT"]
        units = []
        for qb in range(NQB):
            nkt = 4 * qb + 4
            kpairs = [(2 * i, 2 * i + 1) for i in range(nkt // 2)]
            L = len(kpairs)
            state = {"st": {}, "et": {}, "pv": None}

            def offs_of(kp, qb=qb):
                return tuple(max(0, 128 * (kt - 4 * qb)) for kt in kp)

            def emit_scores(i, qb=qb, kpairs=kpairs, state=state,
                            offs_of=offs_of):
                kp = kpairs[i]
                offs = offs_of(kp)
                sts = []
                for s in range(2):
                    sts.append(pp_main.tile([128, 1024], F32, tag="main",
                                            name=f"st{s}"))
                for j in range(2):   # k-tile within pair, outer for adjacency
                    for s in range(2):  # head A/B adjacent -> concurrent
                        rq = slice(s * 64, s * 64 + 64)
                        kt, off = kp[j], offs[j]
                        nc.tensor.matmul(
                            sts[s][:, j * 512 + off:(j + 1) * 512],
                            kT[rq, kt * 128:(kt + 1) * 128],
                            qT[rq, qb * 512 + off:(qb + 1) * 512],
                            start=True, stop=True)
                for s in range(2):
                    if kp[0] >= 4 * qb:
                        # both diag tiles in one op: starts off0 and
                        # 512+off1 differ by 640 for either diag pair
                        st = sts[s]
                        m2 = bass.AP(
                            tensor=st.tensor,
                            offset=st[:].offset + offs[0],
                            ap=[list(st[:].ap[0]), [640, 2], [1, 128]])
                        mb = bass.AP(
                            tensor=mask_sb.tensor,
                            offset=mask_sb[:].offset,
                            ap=[list(mask_sb[:].ap[0]), [0, 2], [1, 128]])
                        nc.vector.tensor_add(m2, m2, mb)
                    elif kp[1] >= 4 * qb:
                        j, off = 1, offs[1]
                        nc.vector.tensor_add(
                            sts[s][:, j * 512 + off:j * 512 + off + 128],
                            sts[s][:, j * 512 + off:j * 512 + off + 128],
                            mask_sb[:])
                state["st"][i] = sts

            def emit_exp(i, qb=qb, kpairs=kpairs, state=state,
                         offs_of=offs_of):
                kp = kpairs[i]
                offs = offs_of(kp)
                lo = offs[0]  # 0 or 256; single op over [lo:1024]
                full = USE_DR and qb >= 1
                sts = state["st"].pop(i)
                ets = []
                for s in range(2):
                    et = po_et.tile([128, 1024], F8 if full else F16,
                                    tag="expT8" if full else "expT",
                                    name="et")
                    nc.scalar.activation(
                        et[:, lo:1024], sts[s][:, lo:1024],
                        mybir.ActivationFunctionType.Exp)
                    if full and kp[0] >= 4 * qb:
                        # DoubleRow reads full strips: zero the regions the
                        # narrowed score matmuls never computed.
                        if lo > 0:
                            nc.gpsimd.memset(et[:, 0:lo], 0.0)
                        if offs[1] > 0:
                            nc.gpsimd.memset(et[:, 512:512 + offs[1]], 0.0)
                    ets.append(et)
                state["et"][i] = ets

            def emit_pv(i, qb=qb, kpairs=kpairs, nkt=nkt, state=state,
                        offs_of=offs_of):
                kp = kpairs[i]
                offs = offs_of(kp)
                full = USE_DR and qb >= 1
                ets = state["et"].pop(i)
                pv = state["pv"]
                for s in range(2):
                    pair_h = st8["pair"] * 2 + s
                    if full:
                        # fp8 DoubleRow: both k-tiles in one matmul
                        v8t = v8[kp[0] // 2]
                        lhsT = bass.AP(
                            tensor=v8t.tensor,
                            offset=v8t[:].offset + pair_h * 65,
                            ap=[list(v8t[:].ap[0]), [528, 2], [1, 65]])
                        et = ets[s]
                        rhs = bass.AP(
                            tensor=et.tensor, offset=et[:].offset,
                            ap=[list(et[:].ap[0]), [512, 2], [1, 512]])
                        nc.tensor.matmul(
                            pv[s][0:65, 0:512], lhsT, rhs,
                            start=(kp[0] == 0), stop=(kp[1] == nkt - 1),
                            perf_mode=DR, skip_group_check=True)
                        continue
                    vsl = slice(pair_h * 65, pair_h * 65 + 65)
                    for j in range(2):
                        kt, off = kp[j], offs[j]
                        nc.tensor.matmul(
                            pv[s][0:65, off:512], v16[kt][:, vsl],
                            ets[s][:, j * 512 + off:(j + 1) * 512],
                            start=(kt == 0), stop=(kt == nkt - 1),
                            skip_group_check=True)

            def finish(qb=qb, state=state):
                pv = state["pv"]
                qsl = slice(qb * 512, (qb + 1) * 512)
                for s in range(2):
                    # evacuate pv early so the bank frees for the next qb
                    ysb = po_ysb.tile([128, 512], F32, tag="ysb",
                                      name="ysb")
                    nc.vector.tensor_copy(ysb[0:65, :], pv[s][0:65, :])
                    dscr = po_dram.tile([1, 512], F32, tag="dscr",
                                        name="dscr")
                    nc.sync.dma_start(out=dscr[:], in_=ysb[64:65, :])
                    rec = po_rec.tile([128, 512], F32, tag="recip",
                                      name="rec")
                    nc.gpsimd.dma_start(out=rec[0:64, :], in_=bass.AP(
                        tensor=dscr.tensor, offset=dscr[:].offset,
                        ap=[[0, 64]] + [list(a) for a in dscr[:].ap[1:]]))
                    nc.vector.reciprocal_approx_fast(rec[0:64, :],
                                                     rec[0:64, :])
                    if s == 0:
                        nc.vector.tensor_mul(yt[0:64, qsl], ysb[0:64, :],
                                             rec[0:64, :])
                    else:
                        # engines can't cross partitions; bounce via DMA
                        ytmp = po_ytmp.tile([128, 512], F16, tag="ytmp",
                                            name="ytmp")
                        nc.vector.tensor_mul(ytmp[0:64, :], ysb[0:64, :],
                                             rec[0:64, :])
                        (nc.gpsimd if USE_GPS_DMA else nc.sync).dma_start(out=yt[64:128, qsl],
                                            in_=ytmp[0:64, :])

            def unit(i, L=L, state=state, emit_exp=emit_exp,
                     emit_pv=emit_pv, emit_scores=emit_scores,
                     finish=finish):
                # pipeline: exp(i-1) first, then pv(i-2), then scores(i)
                if i == 0:
                    state["pv"] = [
                        pp_pv.tile([128, 512], F32, tag="pv", name=f"pv{s}")
                        for s in range(2)]
                if 1 <= i <= L:
                    emit_exp(i - 1)
                if 2 <= i <= L + 1:
                    emit_pv(i - 2)
                if i < L:
                    emit_scores(i)
                if i == L + 1:
                    finish()

            for i in range(L + 2):
                units.append(lambda i=i, u=unit: u(i))
        return units

    def prep_proj(couple):
        wp_sb = []
        for pq in range(2):
            for cb in range(2):
                prow = (couple * 2 + pq) * 128
                wt = po_wp.tile([128, 512], F16, tag="wp", name="wpt")
                nc.sync.dma_start(
                    out=wt[:],
                    in_=wp[prow:prow + 128, cb * 512:(cb + 1) * 512])
                wp_sb.append(wt)
        return wp_sb

    def proj_units(couple, wp_sb, yts):
        out_p = out_ab[couple]
        units = []
        for tt0 in range(0, NTT, 2):
            def unit(tt0=tt0, couple=couple):
                for tt in (tt0, tt0 + 1):
                    ot = po_ot.tile([128, C], F16, tag="ot", name="ot")
                    ps2 = pp_main.tile([128, 1024], F32, tag="main",
                                       name="pproj")
                    # pq outer: stationary (y tile) reused across cb
                    for pq in range(2):
                        for cb in range(2):
                            nc.tensor.matmul(
                                ps2[:, cb * 512:(cb + 1) * 512],
                                yts[pq][:, tt * 128:(tt + 1) * 128],
                                wp_sb[pq * 2 + cb][:],
                                start=(pq == 0), stop=(pq == 1))
                    for cb in range(2):
                        osl = slice(cb * 512, (cb + 1) * 512)
                        if couple == 1:
                            # Scalar is idle once the last exp retires
                            nc.scalar.activation(
                                ot[:, osl], ps2[:, osl],
                                mybir.ActivationFunctionType.Copy)
                        else:
                            nc.vector.tensor_copy(ot[:, osl], ps2[:, osl])
                    nc.sync.dma_start(
                        out=out_p[tt * 128:(tt + 1) * 128, :], in_=ot[:])
            units.append(unit)
        return units

    def round_robin(*streams):
        # proportional interleave: each pop advances the stream whose
        # remaining fraction is largest, spreading short fill streams
        # evenly across the long attention stream
        streams = [list(s) for s in streams if s]
        totals = [len(s) for s in streams]
        while any(streams):
            frac = [(len(s) / t, i) for i, (s, t) in
                    enumerate(zip(streams, totals)) if s]
            _, i = max(frac)
            streams[i].pop(0)()

    vu = v_units()
    pair_state = []
    yts = []
    pair_state.append(st0)
    for u in qkv_units(st0):
        u()
    for u in vu[0:8]:
        u()
    proj0_late = []
    for p in range(NPAIR):
        yt = po_yt.tile([128, T], F16, tag="yT", name="yt")
        yts.append(yt)
        streams = []
        if p == 0:
            streams.append(vu[8:16])
        if p + 1 < NPAIR:
            stn = prep_qkv(p + 1)
            stn["pair"] = p + 1
            pair_state.append(stn)
            streams.append(qkv_units(stn))
        if p == 2:
            wp_sb = prep_proj(0)
            pu = proj_units(0, wp_sb, yts[0:2])
            streams.append(pu[0:3])
            proj0_late = pu[3:]
        if p == 3:
            streams.append(proj0_late)
        streams.append(attn_units(pair_state[p], yt))
        round_robin(*streams)
    wp_sb = prep_proj(1)
    for u in proj_units(1, wp_sb, yts[2:4]):
        u()

    ctx.close()


_CACHE = {}


def _build():
    if "nc" in _CACHE:
        return _CACHE["nc"]
    nc = bacc.Bacc("TRN2", target_bir_lowering=False, debug=False,
                   enable_asserts=True, num_devices=N_CORES)
    aps = {
        "xt": nc.dram_tensor("xt", [C, T], F16, kind="ExternalInput").ap(),
        "xt8": nc.dram_tensor("xt8", [C // 2, 2 * T], F8,
                              kind="ExternalInput").ap(),
        "wqk8": nc.dram_tensor("wqk8", [C // 2, 4 * F], F8,
                               kind="ExternalInput").ap(),
        "wva": nc.dram_tensor("wva", [C, VW], F16, kind="ExternalInput").ap(),
        "bq": nc.dram_tensor("bq", [F, 1], F32, kind="ExternalInput").ap(),
        "bk": nc.dram_tensor("bk", [F, 1], F32, kind="ExternalInput").ap(),
        "bva2": nc.dram_tensor("bva2", [1, VW], F32,
                               kind="ExternalInput").ap(),
        "wp": nc.dram_tensor("wp", [F, C], F16, kind="ExternalInput").ap(),
        "cmask": nc.dram_tensor("cmask", [128, 128], F32,
                                kind="ExternalInput").ap(),
        "out_pa": nc.dram_tensor("out_pa", [T, C], F16,
                                 kind="ExternalOutput").ap(),
        "out_pb": nc.dram_tensor("out_pb", [T, C], F16,
                                 kind="ExternalOutput").ap(),
    }
    with tile.TileContext(nc) as tc:
        _emit(tc, aps)
    nc.compile()
    _CACHE["nc"] = nc
    return nc


def _make_in_maps(x, Wqkv, bqkv, Wproj):
    x = np.asarray(x, dtype=np.float32)
    Wqkv = np.asarray(Wqkv, dtype=np.float32)
    bqkv = np.asarray(bqkv, dtype=np.float32)
    Wproj = np.asarray(Wproj, dtype=np.float32)

    # triangular causal mask: M[p, f] = 0 if f >= p else -1e9
    p_idx = np.arange(128)[:, None]
    u_idx = np.arange(128)[None, :]
    cmask = np.where(u_idx >= p_idx, 0.0, -1e9).astype(np.float32)

    in_maps = []
    for core in range(N_CORES):
        b, g = divmod(core, 2)
        q0, k0, v0 = 512 * g, C + 512 * g, 2 * C + 512 * g
        wva = np.zeros((C, VW), dtype=np.float32)
        bva = np.zeros((1, VW), dtype=np.float32)
        for h in range(NH):
            src = v0 + D * h
            dst = 65 * h
            # per-head layout [v(64), one]
            wva[:, dst:dst + 64] = Wqkv[:, src:src + 64]
            bva[0, dst:dst + 64] = bqkv[src:src + 64]
            bva[0, dst + 64] = 1.0
        xTf8 = np.ascontiguousarray(x[b].T).astype(ml_dtypes.float8_e4m3fn)
        xr = xTf8.reshape(C // 256, 2, 128, T)
        xt8 = np.concatenate([xr[:, 0], xr[:, 1]], axis=2).reshape(
            C // 2, 2 * T)
        wq_s = (512.0 * Wqkv[:, q0:q0 + F]).astype(ml_dtypes.float8_e4m3fn)
        wk_s = (512.0 * Wqkv[:, k0:k0 + F]).astype(ml_dtypes.float8_e4m3fn)
        wqr = wq_s.reshape(C // 256, 2, 128, F)
        wkr = wk_s.reshape(C // 256, 2, 128, F)
        wqk8 = np.zeros((C // 2, 4 * F), dtype=ml_dtypes.float8_e4m3fn)
        for cp in range(C // 256):
            for pair in range(NPAIR):
                csl = slice(pair * 128, (pair + 1) * 128)
                blk = np.concatenate(
                    [wqr[cp, 0][:, csl], wqr[cp, 1][:, csl],
                     wkr[cp, 0][:, csl], wkr[cp, 1][:, csl]], axis=1)
                wqk8[cp * 128:(cp + 1) * 128,
                     pair * 512:(pair + 1) * 512] = blk
        in_maps.append({
            "xt": np.ascontiguousarray(x[b].T).astype(np.float16),
            "xt8": xt8,
            "wqk8": wqk8,
            "wva": np.ascontiguousarray(wva).astype(np.float16),
            "bq": np.ascontiguousarray(bqkv[q0:q0 + F].reshape(F, 1) * 0.125),
            "bk": np.ascontiguousarray(bqkv[k0:k0 + F].reshape(F, 1)),
            "bva2": np.ascontiguousarray(bva),
            "wp": np.ascontiguousarray(
                Wproj[512 * g:512 * g + F, :]).astype(np.float16),
            "cmask": cmask,
        })
    return in_maps


def run_sharded(x, Wqkv, bqkv, Wproj, bproj, trace=False):
    nc = _build()
    in_maps = _make_in_maps(x, Wqkv, bqkv, Wproj)
    res = run_bass_kernel_spmd(nc, in_maps, core_ids=list(range(N_CORES)),
                               trace=trace)
    bproj = np.asarray(bproj, dtype=np.float32)
    out = np.empty((B, T, C), dtype=np.float32)
    for b in range(B):
        acc = bproj[None, :].astype(np.float32).repeat(T, axis=0)
        for core in (2 * b, 2 * b + 1):
            acc = acc + res.results[core]["out_pa"].astype(np.float32) \
                + res.results[core]["out_pb"].astype(np.float32)
        out[b] = acc
    return out, res


def kernel(x, Wqkv, bqkv, Wproj, bproj):
    out, _ = run_sharded(x, Wqkv, bqkv, Wproj, bproj, trace=False)
    return out
